# revision 1
# baseline (speedup 1.0000x reference)
"""Trainium2 Bass kernel for nn_Encoder_base (5x ChebConv GNN + pool + MLP).

Distribution over 8 NeuronCores:
  - level-0 ChebConv props: edge-sharded by destination (all 32 batches,
    96 = 32*3 features), selection-matmul scatter + 2 DRAM AllGathers
  - levels 1-3: batch-sharded (4 batches/core, 128 = 4*32 features);
    level-1 props sparse (indirect-DMA row gathers + selection matmuls),
    levels 2-3 dense-S matmuls
  - channel mixes as block-diagonal matmuls in feature-major layout
  - MLP: output-feature sharded (each core owns 512 cols of W6/W7/W8 and
    512 rows of W9), BatchNorm is local per feature; activations AllGathered
"""
import numpy as np
import concourse.bass as bass
import concourse.bacc as bacc
import concourse.tile as tile
from concourse import mybir, bass_utils
from concourse.masks import make_identity

F32 = mybir.dt.float32
I32 = mybir.dt.int32
I16 = mybir.dt.int16
AF = mybir.ActivationFunctionType
ALU = mybir.AluOpType
AX = mybir.AxisListType
RG = [list(range(8))]
NCORES = 8
N0, N1, N2, N3 = 16384, 4096, 1024, 128
EPS = 1e-5

_CACHE = {}


# ---------------------------------------------------------------- host prep
def _prep_prop(row, col, we, n_dest, n_shard):
    """Sorted-by-dest edges -> 128-dest windows, 128-edge chunks, padded so
    chunk counts per window match across shards (one SPMD program)."""
    window = 128
    order = np.argsort(row, kind="stable")
    row, col, we = row[order], col[order], we[order]
    per = n_dest // n_shard
    nwin = per // window
    counts = np.zeros((n_shard, nwin), np.int64)
    lists = {}
    for s in range(n_shard):
        lo = s * per
        for wi in range(nwin):
            wlo = lo + wi * window
            a = np.searchsorted(row, wlo, side="left")
            b = np.searchsorted(row, wlo + window, side="left")
            lists[(s, wi)] = (row[a:b] - wlo, col[a:b], we[a:b])
            counts[s, wi] = (b - a + 127) // 128
    ncw = np.maximum(counts.max(axis=0), 1)
    C = int(ncw.sum())
    src = np.zeros((n_shard, C, 128), np.int32)
    dst = np.full((n_shard, C, 128), 200.0, np.float32)
    wea = np.zeros((n_shard, C, 128), np.float32)
    for s in range(n_shard):
        base = 0
        for wi in range(nwin):
            dl, cl, wl = lists[(s, wi)]
            n = len(dl)
            k = int(ncw[wi])
            src[s, base:base + k].reshape(-1)[:n] = cl
            dst[s, base:base + k].reshape(-1)[:n] = dl
            wea[s, base:base + k].reshape(-1)[:n] = wl
            base += k
    return [int(x) for x in ncw], src, dst, wea


def _edge_we(e, n):
    row, col = np.asarray(e[0], np.int64), np.asarray(e[1], np.int64)
    deg = np.bincount(row, minlength=n).astype(np.float32)
    dis = np.where(deg > 0, 1.0 / np.sqrt(np.maximum(deg, 1.0)), 0.0).astype(np.float32)
    return row, col, -(dis[row] * dis[col]).astype(np.float32)


def _sub_edges(row, col, we, pool_idx):
    order = np.argsort(row, kind="stable")
    row, col, we = row[order], col[order], we[order]
    starts = np.searchsorted(row, pool_idx, side="left")
    ends = np.searchsorted(row, pool_idx, side="right")
    nr, ncl, nw = [], [], []
    for i in range(len(pool_idx)):
        s, e = starts[i], ends[i]
        if e > s:
            nr.append(np.full(e - s, i, np.int64))
            ncl.append(col[s:e])
            nw.append(we[s:e])
    return np.concatenate(nr), np.concatenate(ncl), np.concatenate(nw)


def _dense_s(row, col, we, n):
    s = np.zeros((n, n), np.float32)
    np.add.at(s, (row, col), we)
    return s


def _tile_w(w, pack):
    """[K, M] -> [K//(128*pack) * 128, pack*M]: pack K-blocks side by side."""
    k, m = w.shape
    nb = k // 128
    t = w.reshape(nb // pack, pack, 128, m).transpose(0, 2, 1, 3)
    return np.ascontiguousarray(t.reshape((nb // pack) * 128, pack * m))


def _host_prep(inputs):
    d = {k: np.asarray(v) for k, v in inputs.items()}
    x = d["x"].astype(np.float32)
    l0 = np.asarray(d["l0"], np.int64)
    l1 = np.asarray(d["l1"], np.int64)
    l2 = np.asarray(d["l2"], np.int64)

    X0 = np.ascontiguousarray(x.transpose(1, 0, 2).reshape(N0, 96))
    X0p = np.zeros((N0, 128), np.float32)
    X0p[:, :96] = X0
    X0l0T = np.ascontiguousarray(X0[l0].T)  # [96, 4096]

    r0, c0, w0 = _edge_we(d["e0"], N0)
    ncw_p1, src_p1, dst_p1, we_p1 = _prep_prop(r0, c0, w0, N0, NCORES)
    r0s, c0s, w0s = _sub_edges(r0, c0, w0, l0)
    ncw_p2, src_p2, dst_p2, we_p2 = _prep_prop(r0s, c0s, w0s, N1, NCORES)

    r1, c1, w1 = _edge_we(d["e1"], N1)
    ncw_q1, src_q1, dst_q1, we_q1 = _prep_prop(r1, c1, w1, N1, 1)
    r1s, c1s, w1s = _sub_edges(r1, c1, w1, l1)
    ncw_q2, src_q2, dst_q2, we_q2 = _prep_prop(r1s, c1s, w1s, N2, 1)

    r2, c2, w2 = _edge_we(d["e2"], N2)
    S2 = _dense_s(r2, c2, w2, N2)
    S2T = _tile_w(np.ascontiguousarray(S2.T), 8)       # [128, 8192]
    S2l2T = _tile_w(np.ascontiguousarray(S2[l2].T), 8)  # [128, 1024]
    P_l2 = np.zeros((N2, 128), np.float32)
    P_l2[l2, np.arange(128)] = 1.0
    P_l2 = _tile_w(P_l2, 8)                             # [128, 1024]

    r3, c3, w3 = _edge_we(d["e3"], N3)
    S3T = np.ascontiguousarray(_dense_s(r3, c3, w3, N3).T)

    def wmod(W):
        return W[0] - W[2], W[1], 2.0 * W[2]

    Wm1 = wmod(d["Wc1"].astype(np.float32))
    Wm = [wmod(d[f"Wc{i}"].astype(np.float32)) for i in (2, 3, 4, 5)]
    eye4 = np.eye(4, dtype=np.float32)

    per_core = []
    for k in range(NCORES):
        m = {}
        m["X0"] = X0p
        m["X0l0T"] = X0l0T
        m["iota"] = np.tile(np.arange(128, dtype=np.float32), (128, 1))
        m["epsv"] = np.full((128, 1), EPS, np.float32)
        m["l0_idx"] = np.ascontiguousarray(
            np.tile(l0.astype(np.int16).reshape(-1, 16).T, (8, 1)))
        m["l1_idx"] = np.ascontiguousarray(
            np.tile(l1.astype(np.int16).reshape(-1, 16).T, (8, 1)))
        for pref, (src, dst, wea) in (
            ("p1", (src_p1[k], dst_p1[k], we_p1[k])),
            ("p2", (src_p2[k], dst_p2[k], we_p2[k])),
            ("q1", (src_q1[0], dst_q1[0], we_q1[0])),
            ("q2", (src_q2[0], dst_q2[0], we_q2[0])),
        ):
            flat = src.reshape(-1).astype(np.int16)
            m[pref + "_srcw"] = np.ascontiguousarray(
                np.tile(flat.reshape(-1, 16).T, (8, 1)))
            m[pref + "_dst"] = np.ascontiguousarray(dst.transpose(1, 0))
            m[pref + "_we"] = np.ascontiguousarray(wea.transpose(1, 0))
        m["S2T"] = S2T
        m["S2l2T"] = S2l2T
        m["P_l2"] = P_l2
        m["S3T"] = S3T
        for t in range(3):
            bw = np.zeros((96, 128), np.float32)
            for j in range(4):
                bg = 4 * k + j
                bw[3 * bg:3 * bg + 3, 32 * j:32 * j + 32] = Wm1[t]
            m[f"bigw0_{t}"] = bw
        for lev in range(4):
            for t in range(3):
                m[f"bigw{lev + 1}_{t}"] = np.kron(eye4, Wm[lev][t])
        for lev, nm in ((1, "b1"), (2, "b2"), (3, "b3"), (4, "b4"), (5, "b5")):
            m[f"bias{lev}"] = np.tile(d[nm].astype(np.float32), 4).reshape(128, 1)
        for li in (6, 7, 8):
            W = d[f"W{li}"].astype(np.float32)[:, 512 * k:512 * k + 512]
            m[f"w{li}"] = _tile_w(W, 8)  # [512, 4096]
            m[f"g{li}"] = np.ascontiguousarray(
                d[f"g{li}"].astype(np.float32)[512 * k:512 * k + 512].reshape(4, 128).T)
            m[f"be{li}"] = np.ascontiguousarray(
                d[f"be{li}"].astype(np.float32)[512 * k:512 * k + 512].reshape(4, 128).T)
        m["w9"] = _tile_w(d["W9"].astype(np.float32)[512 * k:512 * k + 512], 4)  # [128, 512]
        per_core.append(m)

    meta = {"p1": ncw_p1, "p2": ncw_p2, "q1": ncw_q1, "q2": ncw_q2}
    return per_core, meta


# ---------------------------------------------------------------- device program
def _build_nc(meta, shapes):
    nc = bacc.Bacc("TRN2", target_bir_lowering=False, debug=False, num_devices=NCORES)
    ein = {}
    for name, arr in shapes.items():
        dt = {np.dtype(np.int32): I32, np.dtype(np.int16): I16}.get(arr.dtype, F32)
        ein[name] = nc.dram_tensor(name, list(arr.shape), dt, kind="ExternalInput")
    out_mu = nc.dram_tensor("mu", [128, 32], F32, kind="ExternalOutput")

    tx1_loc = nc.dram_tensor("tx1_loc", [N0 // 8, 128], F32)
    tx1_all = nc.dram_tensor("tx1_all", [N0, 128], F32)
    p2t_loc = nc.dram_tensor("p2t_loc", [96, 512], F32)
    p2t_all = nc.dram_tensor("p2t_all", [8 * 96, 512], F32)
    z1_dram = nc.dram_tensor("z1_dram", [N1, 128], F32)
    t1l1_dram = nc.dram_tensor("t1l1_dram", [N1, 128], F32)
    x6_loc = nc.dram_tensor("x6_loc", [4096, 4], F32)
    x6_all = nc.dram_tensor("x6_all", [8 * 4096, 4], F32)
    h6_loc = nc.dram_tensor("h6_loc", [512, 32], F32)
    h6_all = nc.dram_tensor("h6_all", [4096, 32], F32)
    h7_loc = nc.dram_tensor("h7_loc", [512, 32], F32)
    h7_all = nc.dram_tensor("h7_all", [4096, 32], F32)
    mu_loc = nc.dram_tensor("mu_loc", [128, 32], F32)
    mu_all = nc.dram_tensor("mu_all", [8 * 128, 32], F32)

    with tile.TileContext(nc) as tc:
        with (
            tc.tile_pool(name="const", bufs=1) as cpool,
            tc.tile_pool(name="big", bufs=1) as bigpool,
            tc.tile_pool(name="work", bufs=3) as wpool,
            tc.tile_pool(name="wload", bufs=2) as wlpool,
            tc.tile_pool(name="psA", bufs=3, space="PSUM") as ppool,
            tc.tile_pool(name="psB", bufs=1, space="PSUM") as apool,
        ):
            ident = cpool.tile([128, 128], F32, tag="ident", name="ident")
            make_identity(nc, ident[:])
            iota_t = cpool.tile([128, 128], F32, tag="iota", name="iota")
            nc.sync.dma_start(out=iota_t[:], in_=ein["iota"][:, :])
            eps_t = cpool.tile([128, 1], F32, tag="epsv", name="epsv")
            nc.sync.dma_start(out=eps_t[:], in_=ein["epsv"][:, :])

            def load_const(name):
                t = cpool.tile(list(shapes[name].shape), F32, tag=name)
                nc.sync.dma_start(out=t[:], in_=ein[name][:, :])
                return t

            def load_chunk_arrs(pref, C):
                s = cpool.tile([128, C * 8], I16, tag=pref + "s", name=pref + "s")
                dd = cpool.tile([128, C], F32, tag=pref + "d", name=pref + "d")
                w = cpool.tile([128, C], F32, tag=pref + "w", name=pref + "w")
                nc.sync.dma_start(out=s[:], in_=ein[pref + "_srcw"][:, :])
                nc.sync.dma_start(out=dd[:], in_=ein[pref + "_dst"][:, :])
                nc.sync.dma_start(out=w[:], in_=ein[pref + "_we"][:, :])
                return s, dd, w

            GRP = 16

            def grp_gather(idx_sb, g0, gc, gather_src):
                zb = wpool.tile([128, GRP * 128], F32, tag="zb", name="zb", bufs=3)
                nc.gpsimd.dma_gather(
                    out_ap=zb[:, :gc * 128].rearrange("p (c e) -> p c e", e=128),
                    in_ap=gather_src[:, :],
                    idxs_ap=idx_sb[:, g0 * 8:(g0 + gc) * 8],
                    num_idxs=gc * 128, num_idxs_reg=gc * 128, elem_size=128,
                    single_packet=False)
                return zb

            def mk_sel(eng, dst_ap, we_ap):
                sel = wpool.tile([128, 128], F32, tag="sel", name="sel")
                eng.tensor_scalar(out=sel[:], in0=iota_t[:], scalar1=dst_ap,
                                  scalar2=we_ap, op0=ALU.is_equal, op1=ALU.mult)
                return sel

            def prop_nodemajor(ncw, pref, gather_src, D, evac):
                C = sum(ncw)
                s, dd, w = load_chunk_arrs(pref, C)
                zbs = {}
                for g0 in range(0, C, GRP):
                    gc = min(GRP, C - g0)
                    zbs[g0] = grp_gather(s, g0, gc, gather_src)
                base = 0
                for wi, nch in enumerate(ncw):
                    ps = ppool.tile([128, 512], F32, tag="ps", name="ps")
                    for c in range(nch):
                        cc = base + c
                        zb = zbs[(cc // GRP) * GRP]
                        lo = (cc % GRP) * 128
                        sel = mk_sel(nc.vector, dd[:, cc:cc + 1], w[:, cc:cc + 1])
                        nc.tensor.matmul(out=ps[:, :D], lhsT=sel[:],
                                         rhs=zb[:, lo:lo + D],
                                         start=(c == 0), stop=(c == nch - 1))
                    evac(wi, ps[:, :D])
                    base += nch

            def transp(src_ap, dst_ap):
                p, f = src_ap.shape
                ps = ppool.tile([128, 512], F32, tag="ps", name="ps")
                nc.tensor.transpose(out=ps[:f, :p], in_=src_ap, identity=ident[:])
                nc.scalar.activation(out=dst_ap, in_=ps[:f, :p], func=AF.Copy)

            def gather_T(idx_t, chunks, gather_src, D, outT):
                chunks = list(chunks)
                zb = grp_gather(idx_t, chunks[0], len(chunks), gather_src)
                for ci in range(len(chunks)):
                    transp(zb[:, ci * 128:ci * 128 + D],
                           outT[:, ci * 128:(ci + 1) * 128])

            def einsum_win(bigw, taps, Din, width, out_ap, func, bias_ap):
                ps = ppool.tile([128, 512], F32, tag="ps", name="ps")
                for t in range(3):
                    nc.tensor.matmul(out=ps[:, :width], lhsT=bigw[t][:Din, :],
                                     rhs=taps[t], start=(t == 0), stop=(t == 2))
                f2 = AF.Identity if func == AF.Copy else func
                nc.scalar.activation(out=out_ap, in_=ps[:, :width], func=f2, bias=bias_ap)

            # ================= LEVEL 0 =================
            with nc.named_scope("l0_prop1"):
                def evac_p1(wi, ps_ap):
                    t = wpool.tile([128, 96], F32, tag="ev", name="ev", bufs=6)
                    nc.scalar.activation(out=t[:], in_=ps_ap, func=AF.Copy)
                    nc.sync.dma_start(out=tx1_loc[wi * 128:(wi + 1) * 128, :96], in_=t[:])
                prop_nodemajor(meta["p1"], "p1", ein["X0"], 96, evac_p1)
            with nc.named_scope("ag1"):
                nc.gpsimd.collective_compute(
                    "AllGather", ALU.bypass, replica_groups=RG,
                    ins=[tx1_loc.ap().opt()], outs=[tx1_all.ap().opt()])

            with nc.named_scope("l0_prop2"):
                C2 = sum(meta["p2"])
                s2c, d2c, w2c = load_chunk_arrs("p2", C2)
                zbs2 = {}
                for g0 in range(0, C2, GRP):
                    gc = min(GRP, C2 - g0)
                    zbs2[g0] = grp_gather(s2c, g0, gc, tx1_all)
                p2t_sb = bigpool.tile([96, 512], F32, tag="p2t_sb", name="p2t_sb")
                base = 0
                for wi, nch in enumerate(meta["p2"]):
                    ps = ppool.tile([128, 512], F32, tag="ps", name="ps")
                    for c in range(nch):
                        cc = base + c
                        zb = zbs2[(cc // GRP) * GRP]
                        lo = (cc % GRP) * 128
                        sel = mk_sel(nc.vector, d2c[:, cc:cc + 1], w2c[:, cc:cc + 1])
                        nc.tensor.matmul(out=ps[:96, :128],
                                         lhsT=zb[:, lo:lo + 96], rhs=sel[:],
                                         start=(c == 0), stop=(c == nch - 1))
                    nc.scalar.activation(out=p2t_sb[:, wi * 128:(wi + 1) * 128],
                                         in_=ps[:96, :128], func=AF.Copy)
                    base += nch
                nc.sync.dma_start(out=p2t_loc[:, :], in_=p2t_sb[:])
            with nc.named_scope("ag2"):
                nc.gpsimd.collective_compute(
                    "AllGather", ALU.bypass, replica_groups=RG,
                    ins=[p2t_loc.ap().opt()], outs=[p2t_all.ap().opt()])

            with nc.named_scope("l0_einsum"):
                l0i = cpool.tile([128, 32 * 8], I16, tag="l0i", name="l0i")
                nc.sync.dma_start(out=l0i[:], in_=ein["l0_idx"][:, :])
                bw0 = [load_const(f"bigw0_{t}") for t in range(3)]
                bias1 = load_const("bias1")
                for w in range(8):
                    g0w = wpool.tile([96, 512], F32, tag="g0w", name="g0w")
                    nc.sync.dma_start(out=g0w[:], in_=ein["X0l0T"][:, 512 * w:512 * (w + 1)])
                    g1w = wpool.tile([96, 512], F32, tag="g1w", name="g1w")
                    gather_T(l0i, range(4 * w, 4 * w + 4), tx1_all, 96, g1w)
                    p2w = wpool.tile([96, 512], F32, tag="p2w", name="p2w")
                    nc.sync.dma_start(out=p2w[:], in_=p2t_all[96 * w:96 * (w + 1), :])
                    z1Tw = wpool.tile([128, 512], F32, tag="z1Tw", name="z1Tw")
                    einsum_win(bw0, [g0w[:], g1w[:], p2w[:]], 96, 512,
                               z1Tw[:], AF.Copy, bias1[:, 0:1])
                    for c in range(4):
                        t = wpool.tile([128, 128], F32, tag="z1nc", name="z1nc")
                        transp(z1Tw[:, c * 128:(c + 1) * 128], t[:])
                        r = w * 512 + c * 128
                        nc.sync.dma_start(out=z1_dram[r:r + 128, :], in_=t[:])

            # ================= LEVEL 1 =================
            with nc.named_scope("l1_prop1"):
                def evac_q1(wi, ps_ap):
                    t = wpool.tile([128, 128], F32, tag="ev", name="ev", bufs=6)
                    nc.scalar.activation(out=t[:], in_=ps_ap, func=AF.Copy)
                    nc.sync.dma_start(out=t1l1_dram[wi * 128:(wi + 1) * 128, :], in_=t[:])
                prop_nodemajor(meta["q1"], "q1", z1_dram, 128, evac_q1)

            p2n_l1 = bigpool.tile([128, 8 * 128], F32, tag="p2n_l1", name="p2n_l1")
            with nc.named_scope("l1_prop2"):
                def evac_q2(wi, ps_ap):
                    nc.scalar.activation(out=p2n_l1[:, wi * 128:(wi + 1) * 128],
                                         in_=ps_ap, func=AF.Copy)
                prop_nodemajor(meta["q2"], "q2", t1l1_dram, 128, evac_q2)

            z2n = bigpool.tile([128, 8 * 128], F32, tag="z2n", name="z2n")
            with nc.named_scope("l1_einsum"):
                l1i = cpool.tile([128, 8 * 8], I16, tag="l1i", name="l1i")
                nc.sync.dma_start(out=l1i[:], in_=ein["l1_idx"][:, :])
                z1l1T = bigpool.tile([128, 1024], F32, tag="z1l1T", name="z1l1T")
                gather_T(l1i, range(8), z1_dram, 128, z1l1T)
                t1l1T = bigpool.tile([128, 1024], F32, tag="t1l1T", name="t1l1T")
                gather_T(l1i, range(8), t1l1_dram, 128, t1l1T)
                p2l1T = bigpool.tile([128, 1024], F32, tag="p2l1T", name="p2l1T")
                for c in range(8):
                    transp(p2n_l1[:, c * 128:(c + 1) * 128], p2l1T[:, c * 128:(c + 1) * 128])
                bw1 = [load_const(f"bigw1_{t}") for t in range(3)]
                bias2 = load_const("bias2")
                z2T = bigpool.tile([128, 1024], F32, tag="z2T", name="z2T")
                for w in range(2):
                    einsum_win(bw1, [z1l1T[:, 512 * w:512 * (w + 1)],
                                     t1l1T[:, 512 * w:512 * (w + 1)],
                                     p2l1T[:, 512 * w:512 * (w + 1)]],
                               128, 512, z2T[:, 512 * w:512 * (w + 1)], AF.Tanh, bias2[:, 0:1])
                for c in range(8):
                    transp(z2T[:, c * 128:(c + 1) * 128], z2n[:, c * 128:(c + 1) * 128])

            # ================= LEVEL 2 (dense) =================
            with nc.named_scope("l2"):
                t1_l2 = bigpool.tile([128, 8 * 128], F32, tag="t1_l2", name="t1_l2")
                for half in range(2):
                    s2t = wlpool.tile([128, 4096], F32, tag="wld", name="wld")
                    nc.sync.dma_start(out=s2t[:], in_=ein["S2T"][:, 4096 * half:4096 * (half + 1)])
                    for dc in range(8):
                        ps = ppool.tile([128, 512], F32, tag="ps", name="ps")
                        for kk in range(4):
                            kc = half * 4 + kk
                            nc.tensor.matmul(
                                out=ps[:, :128],
                                lhsT=s2t[:, kk * 1024 + dc * 128: kk * 1024 + dc * 128 + 128],
                                rhs=z2n[:, kc * 128:(kc + 1) * 128],
                                start=(kk == 0), stop=(kk == 3))
                        if half == 0:
                            nc.scalar.activation(out=t1_l2[:, dc * 128:(dc + 1) * 128],
                                                 in_=ps[:, :128], func=AF.Copy)
                        else:
                            nc.vector.tensor_add(t1_l2[:, dc * 128:(dc + 1) * 128],
                                                 t1_l2[:, dc * 128:(dc + 1) * 128],
                                                 ps[:, :128])
                s2l2 = cpool.tile([128, 1024], F32, tag="s2l2", name="s2l2")
                nc.sync.dma_start(out=s2l2[:], in_=ein["S2l2T"][:, :])
                ps = ppool.tile([128, 512], F32, tag="ps", name="ps")
                for kc in range(8):
                    nc.tensor.matmul(out=ps[:, :128], lhsT=s2l2[:, kc * 128:(kc + 1) * 128],
                                     rhs=t1_l2[:, kc * 128:(kc + 1) * 128],
                                     start=(kc == 0), stop=(kc == 7))
                p2n_l2 = wpool.tile([128, 128], F32, tag="p2n_l2", name="p2n_l2")
                nc.scalar.activation(out=p2n_l2[:], in_=ps[:, :128], func=AF.Copy)
                pl2 = cpool.tile([128, 1024], F32, tag="pl2", name="pl2")
                nc.sync.dma_start(out=pl2[:], in_=ein["P_l2"][:, :])
                z2l2T = wpool.tile([128, 128], F32, tag="z2l2T", name="z2l2T")
                psg = ppool.tile([128, 512], F32, tag="ps", name="ps")
                for kc in range(8):
                    nc.tensor.matmul(out=psg[:, :128], lhsT=z2n[:, kc * 128:(kc + 1) * 128],
                                     rhs=pl2[:, kc * 128:(kc + 1) * 128],
                                     start=(kc == 0), stop=(kc == 7))
                nc.scalar.activation(out=z2l2T[:], in_=psg[:, :128], func=AF.Copy)
                t1l2T = wpool.tile([128, 128], F32, tag="t1l2T", name="t1l2T")
                psg2 = ppool.tile([128, 512], F32, tag="ps", name="ps")
                for kc in range(8):
                    nc.tensor.matmul(out=psg2[:, :128], lhsT=t1_l2[:, kc * 128:(kc + 1) * 128],
                                     rhs=pl2[:, kc * 128:(kc + 1) * 128],
                                     start=(kc == 0), stop=(kc == 7))
                nc.scalar.activation(out=t1l2T[:], in_=psg2[:, :128], func=AF.Copy)
                p2l2T = wpool.tile([128, 128], F32, tag="p2l2T", name="p2l2T")
                transp(p2n_l2[:], p2l2T[:])
                bw2 = [load_const(f"bigw2_{t}") for t in range(3)]
                bias3 = load_const("bias3")
                z3T = wpool.tile([128, 128], F32, tag="z3T", name="z3T")
                einsum_win(bw2, [z2l2T[:], t1l2T[:], p2l2T[:]], 128, 128,
                           z3T[:], AF.Tanh, bias3[:, 0:1])
                z3n = wpool.tile([128, 128], F32, tag="z3n", name="z3n")
                transp(z3T[:], z3n[:])

            # ================= LEVEL 3 =================
            with nc.named_scope("l3"):
                s3t = cpool.tile([128, 128], F32, tag="s3t", name="s3t")
                nc.sync.dma_start(out=s3t[:], in_=ein["S3T"][:, :])
                bias4 = load_const("bias4")
                bias5 = load_const("bias5")

                def conv_l3(zn, zT, bw_pref, bias_t, func, keep):
                    t1T = wpool.tile([128, 128], F32, tag=keep + "t1T", name=keep + "t1T")
                    ps = ppool.tile([128, 512], F32, tag="ps", name="ps")
                    nc.tensor.matmul(out=ps[:, :128], lhsT=zn, rhs=s3t[:], start=True, stop=True)
                    nc.scalar.activation(out=t1T[:], in_=ps[:, :128], func=AF.Copy)
                    t1n_ = wpool.tile([128, 128], F32, tag=keep + "t1n", name=keep + "t1n")
                    transp(t1T[:], t1n_[:])
                    p2T_ = wpool.tile([128, 128], F32, tag=keep + "p2T", name=keep + "p2T")
                    ps2 = ppool.tile([128, 512], F32, tag="ps", name="ps")
                    nc.tensor.matmul(out=ps2[:, :128], lhsT=t1n_[:], rhs=s3t[:], start=True, stop=True)
                    nc.scalar.activation(out=p2T_[:], in_=ps2[:, :128], func=AF.Copy)
                    bw = [load_const(f"{bw_pref}_{t}") for t in range(3)]
                    outT = wpool.tile([128, 128], F32, tag=keep + "oT", name=keep + "oT")
                    einsum_win(bw, [zT, t1T[:], p2T_[:]], 128, 128, outT[:], func, bias_t[:, 0:1])
                    outn = wpool.tile([128, 128], F32, tag=keep + "on", name=keep + "on")
                    transp(outT[:], outn[:])
                    return outn, outT

                z4n, z4T = conv_l3(z3n[:], z3T[:], "bigw3", bias4, AF.Tanh, "c4")
                o5n, o5T = conv_l3(z4n[:], z4T[:], "bigw4", bias5, AF.Copy, "c5")

            # ================= MLP input assembly =================
            with nc.named_scope("mlp_in"):
                for j in range(4):
                    ap_out = x6_loc.ap()[:, j:j + 1].rearrange("(n h) o -> n (h o)", h=32)
                    nc.sync.dma_start(out=ap_out, in_=o5n[:, 32 * j:32 * j + 32])
                nc.gpsimd.collective_compute(
                    "AllGather", ALU.bypass, replica_groups=RG,
                    ins=[x6_loc.ap().opt()], outs=[x6_all.ap().opt()])

            # ================= MLP =================
            def mlp_layer(nm, src_sb, out_sb):
                g_t = load_const("g" + nm[1])
                be_t = load_const("be" + nm[1])
                pss = [apool.tile([128, 32], F32, tag=f"acc{m}", name=f"acc{m}") for m in range(4)]
                for i in range(4):
                    wt = wlpool.tile([128, 4096], F32, tag="wld", name="wld")
                    nc.sync.dma_start(out=wt[:], in_=ein[nm][128 * i:128 * (i + 1), :])
                    for a in range(8):
                        kc = i * 8 + a
                        for mm in range(4):
                            nc.tensor.matmul(
                                out=pss[mm][:],
                                lhsT=wt[:, a * 512 + mm * 128: a * 512 + mm * 128 + 128],
                                rhs=src_sb[:, 32 * kc:32 * kc + 32],
                                start=(kc == 0), stop=(kc == 31))
                for mm in range(4):
                    t = wpool.tile([128, 32], F32, tag="b_t", name="b_t")
                    nc.vector.tensor_copy(t[:], pss[mm][:])
                    s1 = wpool.tile([128, 1], F32, tag="b_s1", name="b_s1")
                    nc.vector.tensor_reduce(out=s1[:], in_=t[:], axis=AX.X, op=ALU.add)
                    mu_ = wpool.tile([128, 1], F32, tag="b_mu", name="b_mu")
                    nc.vector.tensor_scalar_mul(mu_[:], s1[:], 1.0 / 32.0)
                    sq = wpool.tile([128, 32], F32, tag="b_sq", name="b_sq")
                    nc.vector.tensor_mul(sq[:], t[:], t[:])
                    s2_ = wpool.tile([128, 1], F32, tag="b_s2", name="b_s2")
                    nc.vector.tensor_reduce(out=s2_[:], in_=sq[:], axis=AX.X, op=ALU.add)
                    var = wpool.tile([128, 1], F32, tag="b_var", name="b_var")
                    nc.vector.scalar_tensor_tensor(out=var[:], in0=mu_[:], scalar=-1.0,
                                                   in1=mu_[:], op0=ALU.mult, op1=ALU.mult)
                    nc.vector.scalar_tensor_tensor(out=var[:], in0=s2_[:], scalar=1.0 / 32.0,
                                                   in1=var[:], op0=ALU.mult, op1=ALU.add)
                    sd = wpool.tile([128, 1], F32, tag="b_sd", name="b_sd")
                    nc.scalar.activation(out=sd[:], in_=var[:], func=AF.Sqrt, bias=eps_t[:, 0:1])
                    rs = wpool.tile([128, 1], F32, tag="b_rs", name="b_rs")
                    nc.vector.reciprocal(rs[:], sd[:])
                    a_ = wpool.tile([128, 1], F32, tag="b_a", name="b_a")
                    nc.vector.tensor_mul(a_[:], rs[:], g_t[:, mm:mm + 1])
                    sh = wpool.tile([128, 1], F32, tag="b_sh", name="b_sh")
                    nc.vector.scalar_tensor_tensor(out=sh[:], in0=mu_[:], scalar=-1.0,
                                                   in1=a_[:], op0=ALU.mult, op1=ALU.mult)
                    nc.vector.tensor_add(sh[:], sh[:], be_t[:, mm:mm + 1])
                    nc.scalar.activation(out=out_sb[:, 32 * mm:32 * mm + 32], in_=t[:],
                                         func=AF.Relu, scale=a_[:, 0:1], bias=sh[:, 0:1])

            with nc.named_scope("mlp6"):
                x6T = bigpool.tile([128, 1024], F32, tag="x6T", name="x6T")
                for kk in range(8):
                    nc.sync.dma_start(
                        out=x6T[:].rearrange("p (c r) -> p c r", r=32)[:, :, 4 * kk:4 * kk + 4],
                        in_=x6_all[4096 * kk:4096 * (kk + 1), :].rearrange(
                            "(c p) j -> p c j", p=128))
                h6 = bigpool.tile([128, 128], F32, tag="h6sb", name="h6sb")
                mlp_layer("w6", x6T, h6)
                nc.sync.dma_start(out=h6_loc.ap().rearrange("(m p) b -> p m b", p=128),
                                  in_=h6[:].rearrange("p (m b) -> p m b", b=32))
                nc.gpsimd.collective_compute(
                    "AllGather", ALU.bypass, replica_groups=RG,
                    ins=[h6_loc.ap().opt()], outs=[h6_all.ap().opt()])
            with nc.named_scope("mlp7"):
                x7T = bigpool.tile([128, 1024], F32, tag="x7T", name="x7T")
                nc.sync.dma_start(out=x7T[:].rearrange("p (c b) -> p c b", b=32),
                                  in_=h6_all[:, :].rearrange("(c p) b -> p c b", p=128))
                h7 = bigpool.tile([128, 128], F32, tag="h7sb", name="h7sb")
                mlp_layer("w7", x7T, h7)
                nc.sync.dma_start(out=h7_loc.ap().rearrange("(m p) b -> p m b", p=128),
                                  in_=h7[:].rearrange("p (m b) -> p m b", b=32))
                nc.gpsimd.collective_compute(
                    "AllGather", ALU.bypass, replica_groups=RG,
                    ins=[h7_loc.ap().opt()], outs=[h7_all.ap().opt()])
            with nc.named_scope("mlp8"):
                x8T = bigpool.tile([128, 1024], F32, tag="x8T", name="x8T")
                nc.sync.dma_start(out=x8T[:].rearrange("p (c b) -> p c b", b=32),
                                  in_=h7_all[:, :].rearrange("(c p) b -> p c b", p=128))
                h8 = bigpool.tile([128, 128], F32, tag="h8sb", name="h8sb")
                mlp_layer("w8", x8T, h8)

            with nc.named_scope("mlp9"):
                w9t = cpool.tile([128, 512], F32, tag="w9t", name="w9t")
                nc.sync.dma_start(out=w9t[:], in_=ein["w9"][:, :])
                ps9 = apool.tile([128, 32], F32, tag="acc0", name="acc0")
                for kc in range(4):
                    nc.tensor.matmul(out=ps9[:], lhsT=w9t[:, kc * 128:(kc + 1) * 128],
                                     rhs=h8[:, 32 * kc:32 * kc + 32],
                                     start=(kc == 0), stop=(kc == 3))
                mu_sb = wpool.tile([128, 32], F32, tag="mu_sb", name="mu_sb")
                nc.scalar.activation(out=mu_sb[:], in_=ps9[:], func=AF.Copy)
                nc.sync.dma_start(out=mu_loc[:, :], in_=mu_sb[:])
                nc.gpsimd.collective_compute(
                    "AllGather", ALU.bypass, replica_groups=RG,
                    ins=[mu_loc.ap().opt()], outs=[mu_all.ap().opt()])
                tot = wpool.tile([128, 32], F32, tag="f_tot", name="f_tot")
                nc.sync.dma_start(out=tot[:], in_=mu_all[0:128, :])
                for k in range(1, 8):
                    pk = wpool.tile([128, 32], F32, tag="f_pk", name="f_pk")
                    nc.sync.dma_start(out=pk[:], in_=mu_all[k * 128:(k + 1) * 128, :])
                    nc.vector.tensor_add(tot[:], tot[:], pk[:])
                s1 = wpool.tile([128, 1], F32, tag="f_s1", name="f_s1")
                nc.vector.tensor_reduce(out=s1[:], in_=tot[:], axis=AX.X, op=ALU.add)
                mu_ = wpool.tile([128, 1], F32, tag="f_mu", name="f_mu")
                nc.vector.tensor_scalar_mul(mu_[:], s1[:], 1.0 / 32.0)
                sq = wpool.tile([128, 32], F32, tag="f_sq", name="f_sq")
                nc.vector.tensor_mul(sq[:], tot[:], tot[:])
                s2_ = wpool.tile([128, 1], F32, tag="f_s2", name="f_s2")
                nc.vector.tensor_reduce(out=s2_[:], in_=sq[:], axis=AX.X, op=ALU.add)
                var = wpool.tile([128, 1], F32, tag="f_var", name="f_var")
                nc.vector.scalar_tensor_tensor(out=var[:], in0=mu_[:], scalar=-1.0,
                                               in1=mu_[:], op0=ALU.mult, op1=ALU.mult)
                nc.vector.scalar_tensor_tensor(out=var[:], in0=s2_[:], scalar=1.0 / 32.0,
                                               in1=var[:], op0=ALU.mult, op1=ALU.add)
                sdf = wpool.tile([128, 1], F32, tag="f_sd", name="f_sd")
                nc.scalar.activation(out=sdf[:], in_=var[:], func=AF.Sqrt, bias=eps_t[:, 0:1])
                rs = wpool.tile([128, 1], F32, tag="f_rs", name="f_rs")
                nc.vector.reciprocal(rs[:], sdf[:])
                neg = wpool.tile([128, 1], F32, tag="f_neg", name="f_neg")
                nc.vector.scalar_tensor_tensor(out=neg[:], in0=mu_[:], scalar=-1.0,
                                               in1=rs[:], op0=ALU.mult, op1=ALU.mult)
                outt = wpool.tile([128, 32], F32, tag="f_out", name="f_out")
                nc.scalar.activation(out=outt[:], in_=tot[:], func=AF.Identity,
                                     scale=rs[:, 0:1], bias=neg[:, 0:1])
                nc.sync.dma_start(out=out_mu[:, :], in_=outt[:])

    nc.compile()
    return nc


# ---------------------------------------------------------------- entry point
def kernel(**inputs) -> np.ndarray:
    per_core, meta = _host_prep(inputs)
    if "prog" not in _CACHE:
        _CACHE["prog"] = _build_nc(meta, per_core[0])
    nc = _CACHE["prog"]
    res = bass_utils.run_bass_kernel_spmd(nc, per_core, core_ids=list(range(NCORES)))
    return np.ascontiguousarray(res.results[0]["mu"].T)



# revision 21
# speedup vs baseline: 2.0442x; 2.0442x over previous
"""Trainium2 Bass kernel for nn_Encoder_base (5x ChebConv GNN + pool + MLP).

Distribution over 8 NeuronCores (v2, bf16 data path):
  - level-0 prop1: edge-sharded by destination; source rows PRE-GATHERED on
    the host (x is a kernel input), selection matrices host-built in bf16
  - level-0 prop2 + level-1 props: destination-sharded with full-width
    (all-batch) node rows -> few fat dma_gather indices instead of many thin
    ones; AllGather of z1/t1 between stages
  - einsums (channel mixes) node-sharded, 8 batch-group passes each
  - levels 2-3: batch-sharded dense-S matmuls (as v1) in bf16
  - MLP: output-feature sharded bf16 weights (FWL), BatchNorm local per
    feature; final BN in fp32
"""
import numpy as np
import concourse.bass as bass
import concourse.bacc as bacc
import concourse.tile as tile
from concourse import mybir, bass_utils
from concourse.masks import make_identity

F32 = mybir.dt.float32
BF = mybir.dt.bfloat16
I32 = mybir.dt.int32
I16 = mybir.dt.int16
NPBF = mybir.dt.np(BF)
AF = mybir.ActivationFunctionType
ALU = mybir.AluOpType
AX = mybir.AxisListType
RG = [list(range(8))]
NCORES = 8
N0, N1, N2, N3 = 16384, 4096, 1024, 128
EPS = 1e-5

_CACHE = {}


# ---------------------------------------------------------------- host prep
def _prep_prop(row, col, we, n_dest, n_shard):
    """Sorted-by-dest edges -> 128-dest windows, 128-edge chunks, padded so
    chunk counts per window match across shards (one SPMD program)."""
    window = 128
    order = np.argsort(row, kind="stable")
    row, col, we = row[order], col[order], we[order]
    per = n_dest // n_shard
    nwin = per // window
    counts = np.zeros((n_shard, nwin), np.int64)
    lists = {}
    for s in range(n_shard):
        lo = s * per
        for wi in range(nwin):
            wlo = lo + wi * window
            a = np.searchsorted(row, wlo, side="left")
            b = np.searchsorted(row, wlo + window, side="left")
            lists[(s, wi)] = (row[a:b] - wlo, col[a:b], we[a:b])
            counts[s, wi] = (b - a + 127) // 128
    ncw = np.maximum(counts.max(axis=0), 1)
    C = int(ncw.sum())
    src = np.zeros((n_shard, C, 128), np.int32)
    dst = np.full((n_shard, C, 128), 200.0, np.float32)
    wea = np.zeros((n_shard, C, 128), np.float32)
    for s in range(n_shard):
        base = 0
        for wi in range(nwin):
            dl, cl, wl = lists[(s, wi)]
            n = len(dl)
            k = int(ncw[wi])
            src[s, base:base + k].reshape(-1)[:n] = cl
            dst[s, base:base + k].reshape(-1)[:n] = dl
            wea[s, base:base + k].reshape(-1)[:n] = wl
            base += k
    return [int(x) for x in ncw], src, dst, wea


def _edge_we(e, n):
    row, col = np.asarray(e[0], np.int64), np.asarray(e[1], np.int64)
    deg = np.bincount(row, minlength=n).astype(np.float32)
    dis = np.where(deg > 0, 1.0 / np.sqrt(np.maximum(deg, 1.0)), 0.0).astype(np.float32)
    return row, col, -(dis[row] * dis[col]).astype(np.float32)


def _sub_edges(row, col, we, pool_idx):
    order = np.argsort(row, kind="stable")
    row, col, we = row[order], col[order], we[order]
    starts = np.searchsorted(row, pool_idx, side="left")
    ends = np.searchsorted(row, pool_idx, side="right")
    nr, ncl, nw = [], [], []
    for i in range(len(pool_idx)):
        s, e = starts[i], ends[i]
        if e > s:
            nr.append(np.full(e - s, i, np.int64))
            ncl.append(col[s:e])
            nw.append(we[s:e])
    return np.concatenate(nr), np.concatenate(ncl), np.concatenate(nw)


def _dense_s(row, col, we, n):
    s = np.zeros((n, n), np.float32)
    np.add.at(s, (row, col), we)
    return s


def _tile_w(w, pack):
    """[K, M] -> [K//(128*pack) * 128, pack*M]: pack K-blocks side by side."""
    k, m = w.shape
    nb = k // 128
    t = w.reshape(nb // pack, pack, 128, m).transpose(0, 2, 1, 3)
    return np.ascontiguousarray(t.reshape((nb // pack) * 128, pack * m))


def _sel_pack(dst, wea):
    """dst/wea [C, 128] -> bf16 selection blocks [128, C*128]."""
    C = dst.shape[0]
    sel = np.zeros((C, 128, 128), np.float32)
    c_idx, p_idx = np.meshgrid(np.arange(C), np.arange(128), indexing="ij")
    valid = dst < 128
    sel[c_idx[valid], p_idx[valid], dst[valid].astype(np.int64)] = wea[valid]
    return np.ascontiguousarray(
        sel.transpose(1, 0, 2).reshape(128, C * 128)).astype(NPBF)


def _rows_pack(table, src, width):
    """Pre-gathered rows: table [N, width], src [C, 128] -> [128, C*width]."""
    C = src.shape[0]
    g = table[src.reshape(-1)].reshape(C, 128, width)
    return np.ascontiguousarray(
        g.transpose(1, 0, 2).reshape(128, C * width)).astype(NPBF)


def _idx_pack(flat):
    return np.ascontiguousarray(
        np.tile(flat.astype(np.int16).reshape(-1, 16).T, (8, 1)))


def _host_prep(inputs):
    d = {k: np.asarray(v) for k, v in inputs.items()}
    x = d["x"].astype(np.float32)
    l0 = np.asarray(d["l0"], np.int64)
    l1 = np.asarray(d["l1"], np.int64)
    l2 = np.asarray(d["l2"], np.int64)

    X0 = np.ascontiguousarray(x.transpose(1, 0, 2).reshape(N0, 96))
    X0bf = X0.astype(NPBF)

    r0, c0, w0 = _edge_we(d["e0"], N0)
    ncw_p1, src_p1, dst_p1, we_p1 = _prep_prop(r0, c0, w0, N0, NCORES)
    r0s, c0s, w0s = _sub_edges(r0, c0, w0, l0)
    ncw_p2, src_p2, dst_p2, we_p2 = _prep_prop(r0s, c0s, w0s, N1, NCORES)

    r1, c1, w1 = _edge_we(d["e1"], N1)
    ncw_q1, src_q1, dst_q1, we_q1 = _prep_prop(r1, c1, w1, N1, NCORES)
    r1s, c1s, w1s = _sub_edges(r1, c1, w1, l1)
    ncw_q2, src_q2, dst_q2, we_q2 = _prep_prop(r1s, c1s, w1s, N2, NCORES)

    r2, c2, w2 = _edge_we(d["e2"], N2)
    S2 = _dense_s(r2, c2, w2, N2)
    S2T = _tile_w(np.ascontiguousarray(S2.T), 8).astype(NPBF)       # [128, 8192]
    S2l2T = _tile_w(np.ascontiguousarray(S2[l2].T), 8).astype(NPBF)  # [128, 1024]
    P_l2 = np.zeros((N2, 128), np.float32)
    P_l2[l2, np.arange(128)] = 1.0
    P_l2 = _tile_w(P_l2, 8).astype(NPBF)                             # [128, 1024]

    r3, c3, w3 = _edge_we(d["e3"], N3)
    S3T = np.ascontiguousarray(_dense_s(r3, c3, w3, N3).T).astype(NPBF)

    def wmod(W):
        return W[0] - W[2], W[1], 2.0 * W[2]

    Wm1 = wmod(d["Wc1"].astype(np.float32))
    Wm = [wmod(d[f"Wc{i}"].astype(np.float32)) for i in (2, 3, 4, 5)]
    eye4 = np.eye(4, dtype=np.float32)

    per_core = []
    for k in range(NCORES):
        m = {}
        m["epsv"] = np.full((128, 1), EPS, np.float32)
        # ---- p1: host-gathered x rows + host sel blocks
        m["p1_xg"] = _rows_pack(X0bf, src_p1[k], 96)
        m["p1_sel"] = _sel_pack(dst_p1[k], we_p1[k])
        # ---- p2: gather idx (into tx1_all) + sel
        m["p2_idx"] = _idx_pack(src_p2[k].reshape(-1))
        m["p2_sel"] = _sel_pack(dst_p2[k], we_p2[k])
        # ---- q1 / q2
        m["q1_idx"] = _idx_pack(src_q1[k].reshape(-1))
        m["q1_sel"] = _sel_pack(dst_q1[k], we_q1[k])
        m["q2_idx"] = _idx_pack(src_q2[k].reshape(-1))
        m["q2_sel"] = _sel_pack(dst_q2[k], we_q2[k])
        # ---- einsum l0 (node shard 512k..512k+512)
        l0s = l0[512 * k:512 * (k + 1)]
        m["g0T"] = np.ascontiguousarray(X0[l0s].T).astype(NPBF)  # [96, 512]
        m["l0_idx"] = _idx_pack(l0s)
        for g in range(8):
            for t in range(3):
                bw = np.zeros((96, 128), np.float32)
                for j in range(4):
                    b = 4 * g + j
                    bw[3 * b:3 * b + 3, 32 * j:32 * j + 32] = Wm1[t]
                m[f"bw0_{g}_{t}"] = bw.astype(NPBF)
        # ---- einsum l1 (node shard 128k..128k+128)
        m["l1_idx"] = _idx_pack(l1[128 * k:128 * (k + 1)])
        for lev in range(4):
            for t in range(3):
                m[f"bigw{lev + 1}_{t}"] = np.kron(eye4, Wm[lev][t]).astype(NPBF)
        for lev, nm in ((1, "b1"), (2, "b2"), (3, "b3"), (4, "b4"), (5, "b5")):
            m[f"bias{lev}"] = np.tile(d[nm].astype(np.float32), 4).reshape(128, 1)
        # ---- level 2/3 dense
        m["S2T"] = S2T
        m["S2l2T"] = S2l2T
        m["P_l2"] = P_l2
        m["S3T"] = S3T
        # ---- MLP (feature shard 512k..512k+512)
        for li in (6, 7, 8):
            W = d[f"W{li}"].astype(np.float32)[:, 512 * k:512 * k + 512]
            m[f"w{li}"] = _tile_w(W, 8).astype(NPBF)  # [512, 4096]
            m[f"g{li}"] = np.ascontiguousarray(
                d[f"g{li}"].astype(np.float32)[512 * k:512 * k + 512].reshape(4, 128).T)
            m[f"be{li}"] = np.ascontiguousarray(
                d[f"be{li}"].astype(np.float32)[512 * k:512 * k + 512].reshape(4, 128).T)
        m["w9"] = _tile_w(
            d["W9"].astype(np.float32)[512 * k:512 * k + 512], 4).astype(NPBF)
        per_core.append(m)

    meta = {"p1": ncw_p1, "p2": ncw_p2, "q1": ncw_q1, "q2": ncw_q2}
    return per_core, meta


# ---------------------------------------------------------------- device program
def _build_nc(meta, shapes):
    nc = bacc.Bacc("TRN2", target_bir_lowering=False, debug=False, num_devices=NCORES)
    dtmap = {np.dtype(np.int32): I32, np.dtype(np.int16): I16,
             np.dtype(NPBF): BF, np.dtype(np.float32): F32}
    ein = {}
    for name, arr in shapes.items():
        ein[name] = nc.dram_tensor(name, list(arr.shape), dtmap[arr.dtype],
                                   kind="ExternalInput")
    out_mu = nc.dram_tensor("mu", [128, 32], F32, kind="ExternalOutput")

    tx1_loc = nc.dram_tensor("tx1_loc", [N0 // 8, 128], BF)
    tx1_all = nc.dram_tensor("tx1_all", [N0, 128], BF, addr_space="Shared")
    z1_loc = nc.dram_tensor("z1_loc", [512, 1024], BF)
    z1_all = nc.dram_tensor("z1_all", [N1, 1024], BF, addr_space="Shared")
    t1_loc = nc.dram_tensor("t1_loc", [512, 1024], BF)
    t1_all = nc.dram_tensor("t1_all", [N1, 1024], BF, addr_space="Shared")
    z2_a2a_in = nc.dram_tensor("z2_a2a_in", [1024, 128], BF)
    z2_a2a_out = nc.dram_tensor("z2_a2a_out", [1024, 128], BF)
    x6_loc = nc.dram_tensor("x6_loc", [4096, 4], BF)
    x6_all = nc.dram_tensor("x6_all", [8 * 4096, 4], BF, addr_space="Shared")
    h6_loc = nc.dram_tensor("h6_loc", [512, 32], BF)
    h6_all = nc.dram_tensor("h6_all", [4096, 32], BF, addr_space="Shared")
    h7_loc = nc.dram_tensor("h7_loc", [512, 32], BF)
    h7_all = nc.dram_tensor("h7_all", [4096, 32], BF, addr_space="Shared")
    mu_loc = nc.dram_tensor("mu_loc", [128, 32], F32)
    mu_all = nc.dram_tensor("mu_all", [8 * 128, 32], F32, addr_space="Shared")

    C1 = sum(meta["p1"])
    C2 = sum(meta["p2"])
    C3 = sum(meta["q1"])
    C4 = sum(meta["q2"])

    with tile.TileContext(nc) as tc:
        with (
            tc.tile_pool(name="const", bufs=1) as cpool,
            tc.tile_pool(name="grp", bufs=2) as gpool,
            tc.tile_pool(name="zb", bufs=3) as zpool,
            tc.tile_pool(name="work", bufs=3) as wpool,
            tc.tile_pool(name="wload", bufs=3) as wlpool,
            tc.tile_pool(name="ps_s", bufs=2, space="PSUM") as pps,
        ):
            identf = cpool.tile([128, 128], F32, tag="identf", name="identf")
            make_identity(nc, identf[:])
            identb = cpool.tile([128, 128], BF, tag="identb", name="identb")
            nc.vector.tensor_copy(identb[:], identf[:])
            eps_t = cpool.tile([128, 1], F32, tag="epsv", name="epsv")
            nc.sync.dma_start(out=eps_t[:], in_=ein["epsv"][:, :])

            def load_const(name, dt=BF):
                t = cpool.tile(list(shapes[name].shape), dt, tag=name)
                nc.sync.dma_start(out=t[:], in_=ein[name][:, :])
                return t

            def load_idx(name, ncols):
                t = cpool.tile([128, ncols], I16, tag=name, name=name)
                nc.sync.dma_start(out=t[:], in_=ein[name][:, :])
                return t

            # group loader for host-packed per-chunk arrays ([128, C*w] in DRAM)
            def mk_loader(ein_name, w, nchunks, grp, tag, eng):
                tiles = {}

                def get(cc):
                    g0 = (cc // grp) * grp
                    if g0 not in tiles:
                        gc = min(grp, nchunks - g0)
                        t = gpool.tile([128, grp * w], BF, tag=tag, name=tag)
                        eng.dma_start(out=t[:, :gc * w],
                                      in_=ein[ein_name][:, g0 * w:(g0 + gc) * w])
                        tiles[g0] = t
                    return tiles[g0], (cc % grp) * w
                return get

            # gather groups: idx_sb [128, nchunks*8] (128 idx per chunk)
            def mk_gather(idx_sb, src_dram, w, nchunks, grp, tag):
                tiles = {}

                def get(cc):
                    g0 = (cc // grp) * grp
                    if g0 not in tiles:
                        gc = min(grp, nchunks - g0)
                        t = zpool.tile([128, grp * w], BF, tag=tag, name=tag)
                        nc.gpsimd.dma_gather(
                            out_ap=t[:, :gc * w].rearrange("p (c e) -> p c e", e=w),
                            in_ap=src_dram[:, :],
                            idxs_ap=idx_sb[:, g0 * 8:(g0 + gc) * 8],
                            num_idxs=gc * 128, num_idxs_reg=gc * 128, elem_size=w,
                            single_packet=False)
                        tiles[g0] = t
                    return tiles[g0], (cc % grp) * w
                return get

            def transp(src_ap, dst_ap):
                p, f = src_ap.shape
                ps = pps.tile([128, 128], BF, tag="tps", name="tps")
                nc.tensor.transpose(out=ps[:f, :p], in_=src_ap, identity=identb[:])
                nc.scalar.activation(out=dst_ap, in_=ps[:f, :p], func=AF.Copy)

            # ================= LEVEL 0: prop1 (host-gathered sources) ========
            with nc.named_scope("l0_prop1"):
                xg = mk_loader("p1_xg", 96, C1, 16, "p1xg", nc.sync)
                sl = mk_loader("p1_sel", 128, C1, 16, "p1sel", nc.scalar)
                base = 0
                for wi, nch in enumerate(meta["p1"]):
                    ps = pps.tile([128, 512], F32, tag="pp1", name="pp1")
                    for c in range(nch):
                        cc = base + c
                        xt, xo = xg(cc)
                        st, so = sl(cc)
                        nc.tensor.matmul(out=ps[:, :96],
                                         lhsT=st[:, so:so + 128],
                                         rhs=xt[:, xo:xo + 96],
                                         start=(c == 0), stop=(c == nch - 1))
                    ev = wpool.tile([128, 96], BF, tag="p1ev", name="p1ev", bufs=4)
                    nc.vector.tensor_copy(ev[:], ps[:, :96])
                    nc.sync.dma_start(out=tx1_loc[wi * 128:(wi + 1) * 128, :96], in_=ev[:])
                    base += nch
            with nc.named_scope("ag1"):
                nc.gpsimd.collective_compute(
                    "AllGather", ALU.bypass, replica_groups=RG,
                    ins=[tx1_loc.ap().opt()], outs=[tx1_all.ap().opt()])

            # ================= LEVEL 0: prop2 (dest = own l0 shard) ==========
            p2T_sb = cpool.tile([96, 512], BF, tag="p2T_sb", name="p2T_sb")
            with nc.named_scope("l0_prop2"):
                p2i = load_idx("p2_idx", C2 * 8)
                sl2 = mk_loader("p2_sel", 128, C2, 16, "p2sel", nc.scalar)
                gz = mk_gather(p2i, tx1_all, 128, C2, 16, "p2zb")
                base = 0
                for wi, nch in enumerate(meta["p2"]):
                    ps = pps.tile([128, 512], F32, tag="pp1", name="pp1")
                    for c in range(nch):
                        cc = base + c
                        zt, zo = gz(cc)
                        st, so = sl2(cc)
                        nc.tensor.matmul(out=ps[:96, :128],
                                         lhsT=zt[:, zo:zo + 96],
                                         rhs=st[:, so:so + 128],
                                         start=(c == 0), stop=(c == nch - 1))
                    nc.scalar.activation(out=p2T_sb[:, wi * 128:(wi + 1) * 128],
                                         in_=ps[:96, :128], func=AF.Copy)
                    base += nch

            # ================= LEVEL 0: einsum -> z1 =========================
            with nc.named_scope("l0_einsum"):
                g0T = load_const("g0T")                      # [96, 512]
                l0i = load_idx("l0_idx", 32)
                gz1 = mk_gather(l0i, tx1_all, 128, 4, 4, "g1zb")
                g1T = cpool.tile([96, 512], BF, tag="g1T", name="g1T")
                for c in range(4):
                    zt, zo = gz1(c)
                    transp(zt[:, zo:zo + 96], g1T[:, c * 128:(c + 1) * 128])
                bias1 = load_const("bias1", F32)
                for g in range(8):
                    bw = [load_const(f"bw0_{g}_{t}") for t in range(3)]
                    ps = pps.tile([128, 512], F32, tag="pp1", name="pp1")
                    for t, tap in enumerate((g0T, g1T, p2T_sb)):
                        nc.tensor.matmul(out=ps[:, :512], lhsT=bw[t][:, :],
                                         rhs=tap[:, :], start=(t == 0), stop=(t == 2))
                    z1Tg = wpool.tile([128, 512], BF, tag="z1Tg", name="z1Tg")
                    nc.scalar.activation(out=z1Tg[:], in_=ps[:, :512],
                                         func=AF.Identity, bias=bias1[:, 0:1])
                    for c in range(4):
                        tn = wpool.tile([128, 128], BF, tag="z1n", name="z1n", bufs=4)
                        transp(z1Tg[:, c * 128:(c + 1) * 128], tn[:])
                        nc.sync.dma_start(
                            out=z1_loc[c * 128:(c + 1) * 128, g * 128:(g + 1) * 128],
                            in_=tn[:])
            with nc.named_scope("ag_z1"):
                nc.gpsimd.collective_compute(
                    "AllGather", ALU.bypass, replica_groups=RG,
                    ins=[z1_loc.ap().opt()], outs=[z1_all.ap().opt()])

            # ================= LEVEL 1: prop1 (dest-sharded, fat rows) =======
            with nc.named_scope("l1_prop1"):
                q1i = load_idx("q1_idx", C3 * 8)
                slq1 = mk_loader("q1_sel", 128, C3, 8, "q1sel", nc.scalar)
                gq1 = mk_gather(q1i, z1_all, 1024, C3, 4, "q1zb")
                base = 0
                for wi, nch in enumerate(meta["q1"]):
                    psh = [pps.tile([128, 512], F32, tag="pp1", name="pp1")
                           for _ in range(2)]
                    for c in range(nch):
                        cc = base + c
                        zt, zo = gq1(cc)
                        st, so = slq1(cc)
                        for h in range(2):
                            nc.tensor.matmul(
                                out=psh[h][:, :512],
                                lhsT=st[:, so:so + 128],
                                rhs=zt[:, zo + h * 512:zo + (h + 1) * 512],
                                start=(c == 0), stop=(c == nch - 1))
                    ev = wpool.tile([128, 1024], BF, tag="q1ev", name="q1ev", bufs=3)
                    for h in range(2):
                        nc.scalar.activation(out=ev[:, h * 512:(h + 1) * 512],
                                             in_=psh[h][:, :512], func=AF.Copy)
                    nc.sync.dma_start(out=t1_loc[wi * 128:(wi + 1) * 128, :], in_=ev[:])
                    base += nch
            with nc.named_scope("ag_t1"):
                nc.gpsimd.collective_compute(
                    "AllGather", ALU.bypass, replica_groups=RG,
                    ins=[t1_loc.ap().opt()], outs=[t1_all.ap().opt()])

            # ================= LEVEL 1: prop2 (dest = own l1 shard) ==========
            p2q = cpool.tile([128, 1024], BF, tag="p2q", name="p2q")
            with nc.named_scope("l1_prop2"):
                q2i = load_idx("q2_idx", C4 * 8)
                slq2 = mk_loader("q2_sel", 128, C4, 8, "q2sel", nc.scalar)
                gq2 = mk_gather(q2i, t1_all, 1024, C4, 4, "q1zb")
                psh = [pps.tile([128, 512], F32, tag="pp1", name="pp1")
                       for _ in range(2)]
                for c in range(C4):
                    zt, zo = gq2(c)
                    st, so = slq2(c)
                    for h in range(2):
                        nc.tensor.matmul(
                            out=psh[h][:, :512],
                            lhsT=st[:, so:so + 128],
                            rhs=zt[:, zo + h * 512:zo + (h + 1) * 512],
                            start=(c == 0), stop=(c == C4 - 1))
                for h in range(2):
                    nc.scalar.activation(out=p2q[:, h * 512:(h + 1) * 512],
                                         in_=psh[h][:, :512], func=AF.Copy)

            # ================= LEVEL 1: einsum -> z2 =========================
            with nc.named_scope("l1_einsum"):
                l1i = load_idx("l1_idx", 8)
                gzl1 = mk_gather(l1i, z1_all, 1024, 1, 1, "el1a")
                gtl1 = mk_gather(l1i, t1_all, 1024, 1, 1, "el1b")
                z1l1T = cpool.tile([128, 1024], BF, tag="z1l1T", name="z1l1T")
                t1l1T = cpool.tile([128, 1024], BF, tag="t1l1T", name="t1l1T")
                p2qT = cpool.tile([128, 1024], BF, tag="p2qT", name="p2qT")
                zt, _ = gzl1(0)
                tt, _ = gtl1(0)
                for g in range(8):
                    transp(zt[:, g * 128:(g + 1) * 128], z1l1T[:, g * 128:(g + 1) * 128])
                    transp(tt[:, g * 128:(g + 1) * 128], t1l1T[:, g * 128:(g + 1) * 128])
                    transp(p2q[:, g * 128:(g + 1) * 128], p2qT[:, g * 128:(g + 1) * 128])
                bw1 = [load_const(f"bigw1_{t}") for t in range(3)]
                bias2 = load_const("bias2", F32)
                for g in range(8):
                    ps = pps.tile([128, 512], F32, tag="pp1", name="pp1")
                    for t, tap in enumerate((z1l1T, t1l1T, p2qT)):
                        nc.tensor.matmul(out=ps[:, :128], lhsT=bw1[t][:, :],
                                         rhs=tap[:, g * 128:(g + 1) * 128],
                                         start=(t == 0), stop=(t == 2))
                    z2Tg = wpool.tile([128, 128], BF, tag="z2Tg", name="z2Tg")
                    nc.scalar.activation(out=z2Tg[:], in_=ps[:, :128],
                                         func=AF.Tanh, bias=bias2[:, 0:1])
                    z2ng = wpool.tile([128, 128], BF, tag="z2ng", name="z2ng", bufs=4)
                    transp(z2Tg[:], z2ng[:])
                    nc.sync.dma_start(out=z2_a2a_in[g * 128:(g + 1) * 128, :],
                                      in_=z2ng[:])
            with nc.named_scope("a2a_z2"):
                nc.gpsimd.collective_compute(
                    "AllToAll", ALU.bypass, replica_groups=RG,
                    ins=[z2_a2a_in.ap().opt()], outs=[z2_a2a_out.ap().opt()])

            # ================= LEVEL 2 (dense, batch-sharded) ================
            with nc.named_scope("l2"):
                # z2n: [128 nodes, 128 own-feats] tiles, k-tile kc
                z2n = cpool.tile([128, 1024], BF, tag="z2n", name="z2n")
                nc.sync.dma_start(
                    out=z2n[:].rearrange("p (c f) -> p c f", f=128),
                    in_=z2_a2a_out.ap().rearrange("(c p) f -> p c f", p=128))
                t1_l2 = cpool.tile([128, 1024], BF, tag="t1_l2", name="t1_l2")
                s2t = load_const("S2T")  # [128, 8192]
                for dc in range(8):
                    ps = pps.tile([128, 512], F32, tag="pp1", name="pp1")
                    for kc in range(8):
                        nc.tensor.matmul(
                            out=ps[:, :128],
                            lhsT=s2t[:, kc * 1024 + dc * 128: kc * 1024 + dc * 128 + 128],
                            rhs=z2n[:, kc * 128:(kc + 1) * 128],
                            start=(kc == 0), stop=(kc == 7))
                    nc.scalar.activation(out=t1_l2[:, dc * 128:(dc + 1) * 128],
                                         in_=ps[:, :128], func=AF.Copy)
                s2l2 = load_const("S2l2T")
                ps = pps.tile([128, 512], F32, tag="pp1", name="pp1")
                for kc in range(8):
                    nc.tensor.matmul(out=ps[:, :128], lhsT=s2l2[:, kc * 128:(kc + 1) * 128],
                                     rhs=t1_l2[:, kc * 128:(kc + 1) * 128],
                                     start=(kc == 0), stop=(kc == 7))
                p2n_l2 = wpool.tile([128, 128], BF, tag="p2n_l2", name="p2n_l2")
                nc.scalar.activation(out=p2n_l2[:], in_=ps[:, :128], func=AF.Copy)
                pl2 = load_const("P_l2")
                z2l2T = wpool.tile([128, 128], BF, tag="z2l2T", name="z2l2T")
                psg = pps.tile([128, 512], F32, tag="pp1", name="pp1")
                for kc in range(8):
                    nc.tensor.matmul(out=psg[:, :128], lhsT=z2n[:, kc * 128:(kc + 1) * 128],
                                     rhs=pl2[:, kc * 128:(kc + 1) * 128],
                                     start=(kc == 0), stop=(kc == 7))
                nc.scalar.activation(out=z2l2T[:], in_=psg[:, :128], func=AF.Copy)
                t1l2T = wpool.tile([128, 128], BF, tag="t1l2T", name="t1l2T")
                psg2 = pps.tile([128, 512], F32, tag="pp1", name="pp1")
                for kc in range(8):
                    nc.tensor.matmul(out=psg2[:, :128], lhsT=t1_l2[:, kc * 128:(kc + 1) * 128],
                                     rhs=pl2[:, kc * 128:(kc + 1) * 128],
                                     start=(kc == 0), stop=(kc == 7))
                nc.scalar.activation(out=t1l2T[:], in_=psg2[:, :128], func=AF.Copy)
                p2l2T = wpool.tile([128, 128], BF, tag="p2l2T", name="p2l2T")
                transp(p2n_l2[:], p2l2T[:])
                bw2 = [load_const(f"bigw2_{t}") for t in range(3)]
                bias3 = load_const("bias3", F32)
                ps3 = pps.tile([128, 512], F32, tag="pp1", name="pp1")
                for t, tap in enumerate((z2l2T, t1l2T, p2l2T)):
                    nc.tensor.matmul(out=ps3[:, :128], lhsT=bw2[t][:, :], rhs=tap[:],
                                     start=(t == 0), stop=(t == 2))
                z3T = wpool.tile([128, 128], BF, tag="z3T", name="z3T")
                nc.scalar.activation(out=z3T[:], in_=ps3[:, :128],
                                     func=AF.Tanh, bias=bias3[:, 0:1])
                z3n = wpool.tile([128, 128], BF, tag="z3n", name="z3n")
                transp(z3T[:], z3n[:])

            # ================= LEVEL 3 =================
            with nc.named_scope("l3"):
                s3t = load_const("S3T")
                bias4 = load_const("bias4", F32)
                bias5 = load_const("bias5", F32)

                def conv_l3(zn, zT, bw_pref, bias_t, func, keep):
                    t1T = wpool.tile([128, 128], BF, tag=keep + "t1T", name=keep + "t1T")
                    ps = pps.tile([128, 512], F32, tag="pp1", name="pp1")
                    nc.tensor.matmul(out=ps[:, :128], lhsT=zn, rhs=s3t[:], start=True, stop=True)
                    nc.scalar.activation(out=t1T[:], in_=ps[:, :128], func=AF.Copy)
                    t1n_ = wpool.tile([128, 128], BF, tag=keep + "t1n", name=keep + "t1n")
                    transp(t1T[:], t1n_[:])
                    p2T_ = wpool.tile([128, 128], BF, tag=keep + "p2T", name=keep + "p2T")
                    ps2 = pps.tile([128, 512], F32, tag="pp1", name="pp1")
                    nc.tensor.matmul(out=ps2[:, :128], lhsT=t1n_[:], rhs=s3t[:], start=True, stop=True)
                    nc.scalar.activation(out=p2T_[:], in_=ps2[:, :128], func=AF.Copy)
                    bw = [load_const(f"{bw_pref}_{t}") for t in range(3)]
                    outT = wpool.tile([128, 128], BF, tag=keep + "oT", name=keep + "oT")
                    ps4 = pps.tile([128, 512], F32, tag="pp1", name="pp1")
                    for t, tap in enumerate((zT, t1T[:], p2T_[:])):
                        nc.tensor.matmul(out=ps4[:, :128], lhsT=bw[t][:, :], rhs=tap,
                                         start=(t == 0), stop=(t == 2))
                    f2 = AF.Identity if func == AF.Copy else func
                    nc.scalar.activation(out=outT[:], in_=ps4[:, :128], func=f2,
                                         bias=bias_t[:, 0:1])
                    outn = wpool.tile([128, 128], BF, tag=keep + "on", name=keep + "on")
                    transp(outT[:], outn[:])
                    return outn, outT

                z4n, z4T = conv_l3(z3n[:], z3T[:], "bigw3", bias4, AF.Tanh, "c4")
                o5n, o5T = conv_l3(z4n[:], z4T[:], "bigw4", bias5, AF.Copy, "c5")

            # ================= MLP input assembly =================
            with nc.named_scope("mlp_in"):
                for j in range(4):
                    ap_out = x6_loc.ap()[:, j:j + 1].rearrange("(n h) o -> n (h o)", h=32)
                    nc.sync.dma_start(out=ap_out, in_=o5n[:, 32 * j:32 * j + 32])
                nc.gpsimd.collective_compute(
                    "AllGather", ALU.bypass, replica_groups=RG,
                    ins=[x6_loc.ap().opt()], outs=[x6_all.ap().opt()])

            # ================= MLP =================
            def mlp_layer(nm, src_sb, out_sb):
                g_t = load_const("g" + nm[1], F32)
                be_t = load_const("be" + nm[1], F32)
                # single PSUM bank for all 4 m-slices: start=True only on the
                # very first matmul (clears the whole bank's has_written bits);
                # each slice's first write then overwrites, later ones add.
                acc = pps.tile([128, 128], F32, tag="macc", name="macc", bufs=1)
                for i in range(4):
                    wt = wlpool.tile([128, 4096], BF, tag="wld", name="wld")
                    nc.scalar.dma_start(out=wt[:], in_=ein[nm][128 * i:128 * (i + 1), :])
                    for a in range(8):
                        kc = i * 8 + a
                        for mm in range(4):
                            nc.tensor.matmul(
                                out=acc[:, 32 * mm:32 * mm + 32],
                                lhsT=wt[:, a * 512 + mm * 128: a * 512 + mm * 128 + 128],
                                rhs=src_sb[:, 32 * kc:32 * kc + 32],
                                start=(kc == 0 and mm == 0), stop=(kc == 31))
                for mm in range(4):
                    t = wpool.tile([128, 32], F32, tag="b_t", name="b_t", bufs=4)
                    nc.vector.tensor_copy(t[:], acc[:, 32 * mm:32 * mm + 32])
                    s1 = wpool.tile([128, 1], F32, tag="b_s1", name="b_s1", bufs=4)
                    nc.vector.tensor_reduce(out=s1[:], in_=t[:], axis=AX.X, op=ALU.add)
                    mu_ = wpool.tile([128, 1], F32, tag="b_mu", name="b_mu", bufs=4)
                    nc.vector.tensor_scalar_mul(mu_[:], s1[:], 1.0 / 32.0)
                    sq = wpool.tile([128, 32], F32, tag="b_sq", name="b_sq", bufs=4)
                    nc.vector.tensor_mul(sq[:], t[:], t[:])
                    s2_ = wpool.tile([128, 1], F32, tag="b_s2", name="b_s2", bufs=4)
                    nc.vector.tensor_reduce(out=s2_[:], in_=sq[:], axis=AX.X, op=ALU.add)
                    var = wpool.tile([128, 1], F32, tag="b_var", name="b_var", bufs=4)
                    nc.vector.scalar_tensor_tensor(out=var[:], in0=mu_[:], scalar=-1.0,
                                                   in1=mu_[:], op0=ALU.mult, op1=ALU.mult)
                    nc.vector.scalar_tensor_tensor(out=var[:], in0=s2_[:], scalar=1.0 / 32.0,
                                                   in1=var[:], op0=ALU.mult, op1=ALU.add)
                    sd = wpool.tile([128, 1], F32, tag="b_sd", name="b_sd", bufs=4)
                    nc.scalar.activation(out=sd[:], in_=var[:], func=AF.Sqrt, bias=eps_t[:, 0:1])
                    rs = wpool.tile([128, 1], F32, tag="b_rs", name="b_rs", bufs=4)
                    nc.vector.reciprocal(rs[:], sd[:])
                    a_ = wpool.tile([128, 1], F32, tag="b_a", name="b_a", bufs=4)
                    nc.vector.tensor_mul(a_[:], rs[:], g_t[:, mm:mm + 1])
                    sh = wpool.tile([128, 1], F32, tag="b_sh", name="b_sh", bufs=4)
                    nc.vector.scalar_tensor_tensor(out=sh[:], in0=mu_[:], scalar=-1.0,
                                                   in1=a_[:], op0=ALU.mult, op1=ALU.mult)
                    nc.vector.tensor_add(sh[:], sh[:], be_t[:, mm:mm + 1])
                    nc.scalar.activation(out=out_sb[:, 32 * mm:32 * mm + 32], in_=t[:],
                                         func=AF.Relu, scale=a_[:, 0:1], bias=sh[:, 0:1])

            with nc.named_scope("mlp6"):
                x6T = cpool.tile([128, 1024], BF, tag="x6T", name="x6T")
                for kk in range(8):
                    nc.sync.dma_start(
                        out=x6T[:].rearrange("p (c r) -> p c r", r=32)[:, :, 4 * kk:4 * kk + 4],
                        in_=x6_all[4096 * kk:4096 * (kk + 1), :].rearrange(
                            "(c p) j -> p c j", p=128))
                h6 = cpool.tile([128, 128], BF, tag="h6sb", name="h6sb")
                mlp_layer("w6", x6T, h6)
                nc.sync.dma_start(out=h6_loc.ap().rearrange("(m p) b -> p m b", p=128),
                                  in_=h6[:].rearrange("p (m b) -> p m b", b=32))
                nc.gpsimd.collective_compute(
                    "AllGather", ALU.bypass, replica_groups=RG,
                    ins=[h6_loc.ap().opt()], outs=[h6_all.ap().opt()])
            with nc.named_scope("mlp7"):
                x7T = cpool.tile([128, 1024], BF, tag="x7T", name="x7T")
                nc.sync.dma_start(out=x7T[:].rearrange("p (c b) -> p c b", b=32),
                                  in_=h6_all[:, :].rearrange("(c p) b -> p c b", p=128))
                h7 = cpool.tile([128, 128], BF, tag="h7sb", name="h7sb")
                mlp_layer("w7", x7T, h7)
                nc.sync.dma_start(out=h7_loc.ap().rearrange("(m p) b -> p m b", p=128),
                                  in_=h7[:].rearrange("p (m b) -> p m b", b=32))
                nc.gpsimd.collective_compute(
                    "AllGather", ALU.bypass, replica_groups=RG,
                    ins=[h7_loc.ap().opt()], outs=[h7_all.ap().opt()])
            with nc.named_scope("mlp8"):
                x8T = cpool.tile([128, 1024], BF, tag="x8T", name="x8T")
                nc.sync.dma_start(out=x8T[:].rearrange("p (c b) -> p c b", b=32),
                                  in_=h7_all[:, :].rearrange("(c p) b -> p c b", p=128))
                h8 = cpool.tile([128, 128], BF, tag="h8sb", name="h8sb")
                mlp_layer("w8", x8T, h8)

            with nc.named_scope("mlp9"):
                w9t = load_const("w9")
                ps9 = pps.tile([128, 128], F32, tag="macc", name="macc", bufs=1)
                for kc in range(4):
                    nc.tensor.matmul(out=ps9[:, :32], lhsT=w9t[:, kc * 128:(kc + 1) * 128],
                                     rhs=h8[:, 32 * kc:32 * kc + 32],
                                     start=(kc == 0), stop=(kc == 3))
                mu_sb = wpool.tile([128, 32], F32, tag="mu_sb", name="mu_sb")
                nc.vector.tensor_copy(mu_sb[:], ps9[:, :32])
                nc.sync.dma_start(out=mu_loc[:, :], in_=mu_sb[:])
                nc.gpsimd.collective_compute(
                    "AllGather", ALU.bypass, replica_groups=RG,
                    ins=[mu_loc.ap().opt()], outs=[mu_all.ap().opt()])
                tot = wpool.tile([128, 32], F32, tag="f_tot", name="f_tot")
                nc.sync.dma_start(out=tot[:], in_=mu_all[0:128, :])
                for k in range(1, 8):
                    pk = wpool.tile([128, 32], F32, tag="f_pk", name="f_pk")
                    nc.sync.dma_start(out=pk[:], in_=mu_all[k * 128:(k + 1) * 128, :])
                    nc.vector.tensor_add(tot[:], tot[:], pk[:])
                s1 = wpool.tile([128, 1], F32, tag="f_s1", name="f_s1")
                nc.vector.tensor_reduce(out=s1[:], in_=tot[:], axis=AX.X, op=ALU.add)
                mu_ = wpool.tile([128, 1], F32, tag="f_mu", name="f_mu")
                nc.vector.tensor_scalar_mul(mu_[:], s1[:], 1.0 / 32.0)
                sq = wpool.tile([128, 32], F32, tag="f_sq", name="f_sq")
                nc.vector.tensor_mul(sq[:], tot[:], tot[:])
                s2_ = wpool.tile([128, 1], F32, tag="f_s2", name="f_s2")
                nc.vector.tensor_reduce(out=s2_[:], in_=sq[:], axis=AX.X, op=ALU.add)
                var = wpool.tile([128, 1], F32, tag="f_var", name="f_var")
                nc.vector.scalar_tensor_tensor(out=var[:], in0=mu_[:], scalar=-1.0,
                                               in1=mu_[:], op0=ALU.mult, op1=ALU.mult)
                nc.vector.scalar_tensor_tensor(out=var[:], in0=s2_[:], scalar=1.0 / 32.0,
                                               in1=var[:], op0=ALU.mult, op1=ALU.add)
                sdf = wpool.tile([128, 1], F32, tag="f_sd", name="f_sd")
                nc.scalar.activation(out=sdf[:], in_=var[:], func=AF.Sqrt, bias=eps_t[:, 0:1])
                rs = wpool.tile([128, 1], F32, tag="f_rs", name="f_rs")
                nc.vector.reciprocal(rs[:], sdf[:])
                neg = wpool.tile([128, 1], F32, tag="f_neg", name="f_neg")
                nc.vector.scalar_tensor_tensor(out=neg[:], in0=mu_[:], scalar=-1.0,
                                               in1=rs[:], op0=ALU.mult, op1=ALU.mult)
                outt = wpool.tile([128, 32], F32, tag="f_out", name="f_out")
                nc.scalar.activation(out=outt[:], in_=tot[:], func=AF.Identity,
                                     scale=rs[:, 0:1], bias=neg[:, 0:1])
                nc.sync.dma_start(out=out_mu[:, :], in_=outt[:])

    nc.compile()
    return nc


# ---------------------------------------------------------------- entry point
def kernel(**inputs) -> np.ndarray:
    per_core, meta = _host_prep(inputs)
    if "prog" not in _CACHE:
        _CACHE["prog"] = _build_nc(meta, per_core[0])
    nc = _CACHE["prog"]
    res = bass_utils.run_bass_kernel_spmd(nc, per_core, core_ids=list(range(NCORES)))
    return np.ascontiguousarray(res.results[0]["mu"].T)


# revision 27
# speedup vs baseline: 2.0768x; 1.0160x over previous
"""Trainium2 Bass kernel for nn_Encoder_base (5x ChebConv GNN + pool + MLP).

Distribution over 8 NeuronCores (v2, bf16 data path):
  - level-0 prop1: edge-sharded by destination; source rows PRE-GATHERED on
    the host (x is a kernel input), selection matrices host-built in bf16
  - level-0 prop2 + level-1 props: destination-sharded with full-width
    (all-batch) node rows -> few fat dma_gather indices instead of many thin
    ones; AllGather of z1/t1 between stages
  - einsums (channel mixes) node-sharded, 8 batch-group passes each
  - levels 2-3: batch-sharded dense-S matmuls (as v1) in bf16
  - MLP: output-feature sharded bf16 weights (FWL), BatchNorm local per
    feature; final BN in fp32
"""
import numpy as np
import concourse.bass as bass
import concourse.bacc as bacc
import concourse.tile as tile
from concourse import mybir, bass_utils
from concourse.masks import make_identity

F32 = mybir.dt.float32
BF = mybir.dt.float16
I32 = mybir.dt.int32
I16 = mybir.dt.int16
NPBF = mybir.dt.np(BF)
AF = mybir.ActivationFunctionType
ALU = mybir.AluOpType
AX = mybir.AxisListType
RG = [list(range(8))]
NCORES = 8
N0, N1, N2, N3 = 16384, 4096, 1024, 128
EPS = 1e-5

_CACHE = {}


# ---------------------------------------------------------------- host prep
def _prep_prop(row, col, we, n_dest, n_shard):
    """Sorted-by-dest edges -> 128-dest windows, 128-edge chunks, padded so
    chunk counts per window match across shards (one SPMD program)."""
    window = 128
    order = np.argsort(row, kind="stable")
    row, col, we = row[order], col[order], we[order]
    per = n_dest // n_shard
    nwin = per // window
    counts = np.zeros((n_shard, nwin), np.int64)
    lists = {}
    for s in range(n_shard):
        lo = s * per
        for wi in range(nwin):
            wlo = lo + wi * window
            a = np.searchsorted(row, wlo, side="left")
            b = np.searchsorted(row, wlo + window, side="left")
            lists[(s, wi)] = (row[a:b] - wlo, col[a:b], we[a:b])
            counts[s, wi] = (b - a + 127) // 128
    ncw = np.maximum(counts.max(axis=0), 1)
    C = int(ncw.sum())
    src = np.zeros((n_shard, C, 128), np.int32)
    dst = np.full((n_shard, C, 128), 200.0, np.float32)
    wea = np.zeros((n_shard, C, 128), np.float32)
    for s in range(n_shard):
        base = 0
        for wi in range(nwin):
            dl, cl, wl = lists[(s, wi)]
            n = len(dl)
            k = int(ncw[wi])
            src[s, base:base + k].reshape(-1)[:n] = cl
            dst[s, base:base + k].reshape(-1)[:n] = dl
            wea[s, base:base + k].reshape(-1)[:n] = wl
            base += k
    return [int(x) for x in ncw], src, dst, wea


def _edge_we(e, n):
    row, col = np.asarray(e[0], np.int64), np.asarray(e[1], np.int64)
    deg = np.bincount(row, minlength=n).astype(np.float32)
    dis = np.where(deg > 0, 1.0 / np.sqrt(np.maximum(deg, 1.0)), 0.0).astype(np.float32)
    return row, col, -(dis[row] * dis[col]).astype(np.float32)


def _sub_edges(row, col, we, pool_idx):
    order = np.argsort(row, kind="stable")
    row, col, we = row[order], col[order], we[order]
    starts = np.searchsorted(row, pool_idx, side="left")
    ends = np.searchsorted(row, pool_idx, side="right")
    nr, ncl, nw = [], [], []
    for i in range(len(pool_idx)):
        s, e = starts[i], ends[i]
        if e > s:
            nr.append(np.full(e - s, i, np.int64))
            ncl.append(col[s:e])
            nw.append(we[s:e])
    return np.concatenate(nr), np.concatenate(ncl), np.concatenate(nw)


def _dense_s(row, col, we, n):
    s = np.zeros((n, n), np.float32)
    np.add.at(s, (row, col), we)
    return s


def _tile_w(w, pack):
    """[K, M] -> [K//(128*pack) * 128, pack*M]: pack K-blocks side by side."""
    k, m = w.shape
    nb = k // 128
    t = w.reshape(nb // pack, pack, 128, m).transpose(0, 2, 1, 3)
    return np.ascontiguousarray(t.reshape((nb // pack) * 128, pack * m))


def _sel_pack(dst, wea):
    """dst/wea [C, 128] -> bf16 selection blocks [128, C*128]."""
    C = dst.shape[0]
    sel = np.zeros((C, 128, 128), np.float32)
    c_idx, p_idx = np.meshgrid(np.arange(C), np.arange(128), indexing="ij")
    valid = dst < 128
    sel[c_idx[valid], p_idx[valid], dst[valid].astype(np.int64)] = wea[valid]
    return np.ascontiguousarray(
        sel.transpose(1, 0, 2).reshape(128, C * 128)).astype(NPBF)


def _rows_pack(table, src, width):
    """Pre-gathered rows: table [N, width], src [C, 128] -> [128, C*width]."""
    C = src.shape[0]
    g = table[src.reshape(-1)].reshape(C, 128, width)
    return np.ascontiguousarray(
        g.transpose(1, 0, 2).reshape(128, C * width)).astype(NPBF)


def _idx_pack(flat):
    return np.ascontiguousarray(
        np.tile(flat.astype(np.int16).reshape(-1, 16).T, (8, 1)))


def _host_prep(inputs):
    d = {k: np.asarray(v) for k, v in inputs.items()}
    x = d["x"].astype(np.float32)
    l0 = np.asarray(d["l0"], np.int64)
    l1 = np.asarray(d["l1"], np.int64)
    l2 = np.asarray(d["l2"], np.int64)

    X0 = np.ascontiguousarray(x.transpose(1, 0, 2).reshape(N0, 96))
    X0bf = X0.astype(NPBF)

    r0, c0, w0 = _edge_we(d["e0"], N0)
    ncw_p1, src_p1, dst_p1, we_p1 = _prep_prop(r0, c0, w0, N0, NCORES)
    r0s, c0s, w0s = _sub_edges(r0, c0, w0, l0)
    ncw_p2, src_p2, dst_p2, we_p2 = _prep_prop(r0s, c0s, w0s, N1, NCORES)

    r1, c1, w1 = _edge_we(d["e1"], N1)
    ncw_q1, src_q1, dst_q1, we_q1 = _prep_prop(r1, c1, w1, N1, NCORES)
    r1s, c1s, w1s = _sub_edges(r1, c1, w1, l1)
    ncw_q2, src_q2, dst_q2, we_q2 = _prep_prop(r1s, c1s, w1s, N2, NCORES)

    r2, c2, w2 = _edge_we(d["e2"], N2)
    S2 = _dense_s(r2, c2, w2, N2)
    S2T = _tile_w(np.ascontiguousarray(S2.T), 8).astype(NPBF)       # [128, 8192]
    S2l2T = _tile_w(np.ascontiguousarray(S2[l2].T), 8).astype(NPBF)  # [128, 1024]
    P_l2 = np.zeros((N2, 128), np.float32)
    P_l2[l2, np.arange(128)] = 1.0
    P_l2 = _tile_w(P_l2, 8).astype(NPBF)                             # [128, 1024]

    r3, c3, w3 = _edge_we(d["e3"], N3)
    S3T = np.ascontiguousarray(_dense_s(r3, c3, w3, N3).T).astype(NPBF)

    def wmod(W):
        return W[0] - W[2], W[1], 2.0 * W[2]

    Wm1 = wmod(d["Wc1"].astype(np.float32))
    Wm = [wmod(d[f"Wc{i}"].astype(np.float32)) for i in (2, 3, 4, 5)]
    eye4 = np.eye(4, dtype=np.float32)

    per_core = []
    for k in range(NCORES):
        m = {}
        m["epsv"] = np.full((128, 1), EPS, np.float32)
        # ---- p1: host-gathered x rows + host sel blocks
        m["p1_xg"] = _rows_pack(X0bf, src_p1[k], 96)
        m["p1_sel"] = _sel_pack(dst_p1[k], we_p1[k])
        # ---- p2: gather idx (into tx1_all) + sel
        m["p2_idx"] = _idx_pack(src_p2[k].reshape(-1))
        m["p2_sel"] = _sel_pack(dst_p2[k], we_p2[k])
        # ---- q1 / q2
        m["q1_idx"] = _idx_pack(src_q1[k].reshape(-1))
        m["q1_sel"] = _sel_pack(dst_q1[k], we_q1[k])
        m["q2_idx"] = _idx_pack(src_q2[k].reshape(-1))
        m["q2_sel"] = _sel_pack(dst_q2[k], we_q2[k])
        # ---- einsum l0 (node shard 512k..512k+512)
        l0s = l0[512 * k:512 * (k + 1)]
        m["g0T"] = np.ascontiguousarray(X0[l0s].T).astype(NPBF)  # [96, 512]
        m["l0_idx"] = _idx_pack(l0s)
        for g in range(8):
            for t in range(3):
                bw = np.zeros((96, 128), np.float32)
                for j in range(4):
                    b = 4 * g + j
                    bw[3 * b:3 * b + 3, 32 * j:32 * j + 32] = Wm1[t]
                m[f"bw0_{g}_{t}"] = bw.astype(NPBF)
        # ---- einsum l1 (node shard 128k..128k+128)
        m["l1_idx"] = _idx_pack(l1[128 * k:128 * (k + 1)])
        for lev in range(4):
            for t in range(3):
                m[f"bigw{lev + 1}_{t}"] = np.kron(eye4, Wm[lev][t]).astype(NPBF)
        for lev, nm in ((1, "b1"), (2, "b2"), (3, "b3"), (4, "b4"), (5, "b5")):
            m[f"bias{lev}"] = np.tile(d[nm].astype(np.float32), 4).reshape(128, 1)
        # ---- level 2/3 dense
        m["S2T"] = S2T
        m["S2l2T"] = S2l2T
        m["P_l2"] = P_l2
        m["S3T"] = S3T
        # ---- MLP (feature shard 512k..512k+512)
        for li in (6, 7, 8):
            W = d[f"W{li}"].astype(np.float32)[:, 512 * k:512 * k + 512]
            m[f"w{li}"] = _tile_w(W, 8).astype(NPBF)  # [512, 4096]
            m[f"g{li}"] = np.ascontiguousarray(
                d[f"g{li}"].astype(np.float32)[512 * k:512 * k + 512].reshape(4, 128).T)
            m[f"be{li}"] = np.ascontiguousarray(
                d[f"be{li}"].astype(np.float32)[512 * k:512 * k + 512].reshape(4, 128).T)
        m["w9"] = _tile_w(
            d["W9"].astype(np.float32)[512 * k:512 * k + 512], 4).astype(NPBF)
        per_core.append(m)

    meta = {"p1": ncw_p1, "p2": ncw_p2, "q1": ncw_q1, "q2": ncw_q2}
    return per_core, meta


# ---------------------------------------------------------------- device program
def _build_nc(meta, shapes):
    nc = bacc.Bacc("TRN2", target_bir_lowering=False, debug=False, num_devices=NCORES)
    dtmap = {np.dtype(np.int32): I32, np.dtype(np.int16): I16,
             np.dtype(NPBF): BF, np.dtype(np.float32): F32}
    ein = {}
    for name, arr in shapes.items():
        ein[name] = nc.dram_tensor(name, list(arr.shape), dtmap[arr.dtype],
                                   kind="ExternalInput")
    out_mu = nc.dram_tensor("mu", [128, 32], F32, kind="ExternalOutput")

    tx1_loc = nc.dram_tensor("tx1_loc", [N0 // 8, 128], BF)
    tx1_all = nc.dram_tensor("tx1_all", [N0, 128], BF, addr_space="Shared")
    z1_loc = nc.dram_tensor("z1_loc", [512, 1024], BF)
    z1_all = nc.dram_tensor("z1_all", [N1, 1024], BF, addr_space="Shared")
    t1_loc = nc.dram_tensor("t1_loc", [512, 1024], BF)
    t1_all = nc.dram_tensor("t1_all", [N1, 1024], BF, addr_space="Shared")
    z2_a2a_in = nc.dram_tensor("z2_a2a_in", [1024, 128], BF)
    z2_a2a_out = nc.dram_tensor("z2_a2a_out", [1024, 128], BF)
    x6_loc = nc.dram_tensor("x6_loc", [4096, 4], BF)
    x6_all = nc.dram_tensor("x6_all", [8 * 4096, 4], BF, addr_space="Shared")
    h6_loc = nc.dram_tensor("h6_loc", [512, 32], BF)
    h6_all = nc.dram_tensor("h6_all", [4096, 32], BF, addr_space="Shared")
    h7_loc = nc.dram_tensor("h7_loc", [512, 32], BF)
    h7_all = nc.dram_tensor("h7_all", [4096, 32], BF, addr_space="Shared")
    mu_loc = nc.dram_tensor("mu_loc", [128, 32], F32)
    mu_all = nc.dram_tensor("mu_all", [8 * 128, 32], F32, addr_space="Shared")

    C1 = sum(meta["p1"])
    C2 = sum(meta["p2"])
    C3 = sum(meta["q1"])
    C4 = sum(meta["q2"])

    with tile.TileContext(nc) as tc:
        with (
            tc.tile_pool(name="const", bufs=1) as cpool,
            tc.tile_pool(name="grp", bufs=2) as gpool,
            tc.tile_pool(name="zb", bufs=3) as zpool,
            tc.tile_pool(name="work", bufs=3) as wpool,
            tc.tile_pool(name="wbig", bufs=2) as wbpool,
            tc.tile_pool(name="ps_s", bufs=2, space="PSUM") as pps,
        ):
            identf = cpool.tile([128, 128], F32, tag="identf", name="identf")
            make_identity(nc, identf[:])
            identb = cpool.tile([128, 128], BF, tag="identb", name="identb")
            nc.vector.tensor_copy(identb[:], identf[:])
            eps_t = cpool.tile([128, 1], F32, tag="epsv", name="epsv")
            nc.sync.dma_start(out=eps_t[:], in_=ein["epsv"][:, :])

            # big weight preloads ride the SWDGE queue: transfers overlap the
            # GNN phase without blocking the HWDGE rings that feed it
            def preload_w(nm):
                t = wbpool.tile([128, 16384], BF, tag="wbig", name="wbig")
                nc.gpsimd.dma_start(
                    out=t[:].rearrange("p (i f) -> p i f", f=4096),
                    in_=ein[nm].ap().rearrange("(i p) f -> p i f", p=128))
                return t

            w6sb = preload_w("w6")
            s2t_sb = cpool.tile([128, 8192], BF, tag="S2T", name="S2T")
            nc.gpsimd.dma_start(out=s2t_sb[:], in_=ein["S2T"][:, :])

            def load_const(name, dt=BF):
                t = cpool.tile(list(shapes[name].shape), dt, tag=name)
                nc.sync.dma_start(out=t[:], in_=ein[name][:, :])
                return t

            def load_idx(name, ncols):
                t = cpool.tile([128, ncols], I16, tag=name, name=name)
                nc.sync.dma_start(out=t[:], in_=ein[name][:, :])
                return t

            # group loader for host-packed per-chunk arrays ([128, C*w] in DRAM)
            def mk_loader(ein_name, w, nchunks, grp, tag, eng):
                tiles = {}

                def get(cc):
                    g0 = (cc // grp) * grp
                    if g0 not in tiles:
                        gc = min(grp, nchunks - g0)
                        t = gpool.tile([128, grp * w], BF, tag=tag, name=tag)
                        eng.dma_start(out=t[:, :gc * w],
                                      in_=ein[ein_name][:, g0 * w:(g0 + gc) * w])
                        tiles[g0] = t
                    return tiles[g0], (cc % grp) * w
                return get

            # gather groups: idx_sb [128, nchunks*8] (128 idx per chunk)
            def mk_gather(idx_sb, src_dram, w, nchunks, grp, tag, bufs=3):
                tiles = {}

                def get(cc):
                    g0 = (cc // grp) * grp
                    if g0 not in tiles:
                        gc = min(grp, nchunks - g0)
                        t = zpool.tile([128, grp * w], BF, tag=tag, name=tag, bufs=bufs)
                        nc.gpsimd.dma_gather(
                            out_ap=t[:, :gc * w].rearrange("p (c e) -> p c e", e=w),
                            in_ap=src_dram[:, :],
                            idxs_ap=idx_sb[:, g0 * 8:(g0 + gc) * 8],
                            num_idxs=gc * 128, num_idxs_reg=gc * 128, elem_size=w,
                            single_packet=False)
                        tiles[g0] = t
                    return tiles[g0], (cc % grp) * w
                return get

            def transp(src_ap, dst_ap):
                p, f = src_ap.shape
                ps = pps.tile([128, 128], BF, tag="tps", name="tps")
                nc.tensor.transpose(out=ps[:f, :p], in_=src_ap, identity=identb[:])
                nc.scalar.activation(out=dst_ap, in_=ps[:f, :p], func=AF.Copy)

            # ================= LEVEL 0: prop1 (host-gathered sources) ========
            with nc.named_scope("l0_prop1"):
                xg = mk_loader("p1_xg", 96, C1, 16, "p1xg", nc.sync)
                sl = mk_loader("p1_sel", 128, C1, 16, "p1sel", nc.scalar)
                base = 0
                for wi, nch in enumerate(meta["p1"]):
                    ps = pps.tile([128, 512], F32, tag="pp1", name="pp1")
                    for c in range(nch):
                        cc = base + c
                        xt, xo = xg(cc)
                        st, so = sl(cc)
                        nc.tensor.matmul(out=ps[:, :96],
                                         lhsT=st[:, so:so + 128],
                                         rhs=xt[:, xo:xo + 96],
                                         start=(c == 0), stop=(c == nch - 1))
                    ev = wpool.tile([128, 96], BF, tag="p1ev", name="p1ev", bufs=4)
                    nc.vector.tensor_copy(ev[:], ps[:, :96])
                    nc.sync.dma_start(out=tx1_loc[wi * 128:(wi + 1) * 128, :96], in_=ev[:])
                    base += nch
            with nc.named_scope("ag1"):
                nc.gpsimd.collective_compute(
                    "AllGather", ALU.bypass, replica_groups=RG,
                    ins=[tx1_loc.ap().opt()], outs=[tx1_all.ap().opt()])

            # ================= LEVEL 0: prop2 (dest = own l0 shard) ==========
            p2T_sb = cpool.tile([96, 512], BF, tag="p2T_sb", name="p2T_sb")
            with nc.named_scope("l0_prop2"):
                p2i = load_idx("p2_idx", C2 * 8)
                sl2 = mk_loader("p2_sel", 128, C2, 16, "p2sel", nc.scalar)
                gz = mk_gather(p2i, tx1_all, 128, C2, 16, "p2zb", bufs=2)
                base = 0
                for wi, nch in enumerate(meta["p2"]):
                    ps = pps.tile([128, 512], F32, tag="pp1", name="pp1")
                    for c in range(nch):
                        cc = base + c
                        zt, zo = gz(cc)
                        st, so = sl2(cc)
                        nc.tensor.matmul(out=ps[:96, :128],
                                         lhsT=zt[:, zo:zo + 96],
                                         rhs=st[:, so:so + 128],
                                         start=(c == 0), stop=(c == nch - 1))
                    nc.scalar.activation(out=p2T_sb[:, wi * 128:(wi + 1) * 128],
                                         in_=ps[:96, :128], func=AF.Copy)
                    base += nch

            # ================= LEVEL 0: einsum -> z1 =========================
            with nc.named_scope("l0_einsum"):
                g0T = load_const("g0T")                      # [96, 512]
                l0i = load_idx("l0_idx", 32)
                gz1 = mk_gather(l0i, tx1_all, 128, 4, 4, "g1zb", bufs=1)
                g1T = cpool.tile([96, 512], BF, tag="g1T", name="g1T")
                for c in range(4):
                    zt, zo = gz1(c)
                    transp(zt[:, zo:zo + 96], g1T[:, c * 128:(c + 1) * 128])
                bias1 = load_const("bias1", F32)
                for g in range(8):
                    bw = [load_const(f"bw0_{g}_{t}") for t in range(3)]
                    ps = pps.tile([128, 512], F32, tag="pp1", name="pp1")
                    for t, tap in enumerate((g0T, g1T, p2T_sb)):
                        nc.tensor.matmul(out=ps[:, :512], lhsT=bw[t][:, :],
                                         rhs=tap[:, :], start=(t == 0), stop=(t == 2))
                    z1Tg = wpool.tile([128, 512], BF, tag="z1Tg", name="z1Tg", bufs=2)
                    nc.scalar.activation(out=z1Tg[:], in_=ps[:, :512],
                                         func=AF.Identity, bias=bias1[:, 0:1])
                    z1g = wpool.tile([128, 512], BF, tag="z1g", name="z1g", bufs=2)
                    for c in range(4):
                        transp(z1Tg[:, c * 128:(c + 1) * 128],
                               z1g[:, c * 128:(c + 1) * 128])
                    nc.sync.dma_start(
                        out=z1_loc[:, g * 128:(g + 1) * 128].rearrange(
                            "(c p) f -> p c f", p=128),
                        in_=z1g[:].rearrange("p (c f) -> p c f", f=128))
            with nc.named_scope("ag_z1"):
                nc.gpsimd.collective_compute(
                    "AllGather", ALU.bypass, replica_groups=RG,
                    ins=[z1_loc.ap().opt()], outs=[z1_all.ap().opt()])

            # ================= LEVEL 1: prop1 (dest-sharded, fat rows) =======
            with nc.named_scope("l1_prop1"):
                q1i = load_idx("q1_idx", C3 * 8)
                slq1 = mk_loader("q1_sel", 128, C3, 8, "q1sel", nc.scalar)
                gq1 = mk_gather(q1i, z1_all, 1024, C3, 4, "q1zb", bufs=2)
                base = 0
                for wi, nch in enumerate(meta["q1"]):
                    psh = [pps.tile([128, 512], F32, tag="pp1", name="pp1")
                           for _ in range(2)]
                    for c in range(nch):
                        cc = base + c
                        zt, zo = gq1(cc)
                        st, so = slq1(cc)
                        for h in range(2):
                            nc.tensor.matmul(
                                out=psh[h][:, :512],
                                lhsT=st[:, so:so + 128],
                                rhs=zt[:, zo + h * 512:zo + (h + 1) * 512],
                                start=(c == 0), stop=(c == nch - 1))
                    ev = wpool.tile([128, 1024], BF, tag="q1ev", name="q1ev", bufs=2)
                    for h in range(2):
                        nc.scalar.activation(out=ev[:, h * 512:(h + 1) * 512],
                                             in_=psh[h][:, :512], func=AF.Copy)
                    nc.sync.dma_start(out=t1_loc[wi * 128:(wi + 1) * 128, :], in_=ev[:])
                    base += nch
                # z1 einsum taps don't depend on t1: gather + transpose them
                # (and start the w7 preload) before the ag_t1 wait blocks gpsimd
                l1i = load_idx("l1_idx", 8)
                gzl1 = mk_gather(l1i, z1_all, 1024, 1, 1, "el1a", bufs=1)
                z1l1T = cpool.tile([128, 1024], BF, tag="z1l1T", name="z1l1T")
                zt_l1, _ = gzl1(0)
                for g in range(8):
                    transp(zt_l1[:, g * 128:(g + 1) * 128],
                           z1l1T[:, g * 128:(g + 1) * 128])
                w7sb = preload_w("w7")
            with nc.named_scope("ag_t1"):
                nc.gpsimd.collective_compute(
                    "AllGather", ALU.bypass, replica_groups=RG,
                    ins=[t1_loc.ap().opt()], outs=[t1_all.ap().opt()])

            # ================= LEVEL 1: prop2 (dest = own l1 shard) ==========
            p2q = cpool.tile([128, 1024], BF, tag="p2q", name="p2q")
            with nc.named_scope("l1_prop2"):
                q2i = load_idx("q2_idx", C4 * 8)
                slq2 = mk_loader("q2_sel", 128, C4, 8, "q2sel", nc.scalar)
                gq2 = mk_gather(q2i, t1_all, 1024, C4, 4, "q1zb", bufs=2)
                psh = [pps.tile([128, 512], F32, tag="pp1", name="pp1")
                       for _ in range(2)]
                for c in range(C4):
                    zt, zo = gq2(c)
                    st, so = slq2(c)
                    for h in range(2):
                        nc.tensor.matmul(
                            out=psh[h][:, :512],
                            lhsT=st[:, so:so + 128],
                            rhs=zt[:, zo + h * 512:zo + (h + 1) * 512],
                            start=(c == 0), stop=(c == C4 - 1))
                for h in range(2):
                    nc.scalar.activation(out=p2q[:, h * 512:(h + 1) * 512],
                                         in_=psh[h][:, :512], func=AF.Copy)

            # ================= LEVEL 1: einsum -> z2 =========================
            with nc.named_scope("l1_einsum"):
                gtl1 = mk_gather(l1i, t1_all, 1024, 1, 1, "el1b", bufs=1)
                t1l1T = cpool.tile([128, 1024], BF, tag="t1l1T", name="t1l1T")
                p2qT = cpool.tile([128, 1024], BF, tag="p2qT", name="p2qT")
                tt, _ = gtl1(0)
                w8sb = preload_w("w8")
                for g in range(8):
                    transp(tt[:, g * 128:(g + 1) * 128], t1l1T[:, g * 128:(g + 1) * 128])
                    transp(p2q[:, g * 128:(g + 1) * 128], p2qT[:, g * 128:(g + 1) * 128])
                bw1 = [load_const(f"bigw1_{t}") for t in range(3)]
                bias2 = load_const("bias2", F32)
                for g in range(8):
                    ps = pps.tile([128, 512], F32, tag="pp1", name="pp1")
                    for t, tap in enumerate((z1l1T, t1l1T, p2qT)):
                        nc.tensor.matmul(out=ps[:, :128], lhsT=bw1[t][:, :],
                                         rhs=tap[:, g * 128:(g + 1) * 128],
                                         start=(t == 0), stop=(t == 2))
                    z2Tg = wpool.tile([128, 128], BF, tag="z2Tg", name="z2Tg")
                    nc.scalar.activation(out=z2Tg[:], in_=ps[:, :128],
                                         func=AF.Tanh, bias=bias2[:, 0:1])
                    z2ng = wpool.tile([128, 128], BF, tag="z2ng", name="z2ng", bufs=4)
                    transp(z2Tg[:], z2ng[:])
                    nc.sync.dma_start(out=z2_a2a_in[g * 128:(g + 1) * 128, :],
                                      in_=z2ng[:])
            with nc.named_scope("a2a_z2"):
                nc.gpsimd.collective_compute(
                    "AllToAll", ALU.bypass, replica_groups=RG,
                    ins=[z2_a2a_in.ap().opt()], outs=[z2_a2a_out.ap().opt()])

            # ================= LEVEL 2 (dense, batch-sharded) ================
            with nc.named_scope("l2"):
                # z2n: [128 nodes, 128 own-feats] tiles, k-tile kc
                z2n = cpool.tile([128, 1024], BF, tag="z2n", name="z2n")
                nc.sync.dma_start(
                    out=z2n[:].rearrange("p (c f) -> p c f", f=128),
                    in_=z2_a2a_out.ap().rearrange("(c p) f -> p c f", p=128))
                t1_l2 = cpool.tile([128, 1024], BF, tag="t1_l2", name="t1_l2")
                s2t = s2t_sb
                for dc in range(8):
                    ps = pps.tile([128, 512], F32, tag="pp1", name="pp1")
                    for kc in range(8):
                        nc.tensor.matmul(
                            out=ps[:, :128],
                            lhsT=s2t[:, kc * 1024 + dc * 128: kc * 1024 + dc * 128 + 128],
                            rhs=z2n[:, kc * 128:(kc + 1) * 128],
                            start=(kc == 0), stop=(kc == 7))
                    nc.scalar.activation(out=t1_l2[:, dc * 128:(dc + 1) * 128],
                                         in_=ps[:, :128], func=AF.Copy)
                s2l2 = load_const("S2l2T")
                ps = pps.tile([128, 512], F32, tag="pp1", name="pp1")
                for kc in range(8):
                    nc.tensor.matmul(out=ps[:, :128], lhsT=s2l2[:, kc * 128:(kc + 1) * 128],
                                     rhs=t1_l2[:, kc * 128:(kc + 1) * 128],
                                     start=(kc == 0), stop=(kc == 7))
                p2n_l2 = wpool.tile([128, 128], BF, tag="p2n_l2", name="p2n_l2")
                nc.scalar.activation(out=p2n_l2[:], in_=ps[:, :128], func=AF.Copy)
                pl2 = load_const("P_l2")
                z2l2T = wpool.tile([128, 128], BF, tag="z2l2T", name="z2l2T")
                psg = pps.tile([128, 512], F32, tag="pp1", name="pp1")
                for kc in range(8):
                    nc.tensor.matmul(out=psg[:, :128], lhsT=z2n[:, kc * 128:(kc + 1) * 128],
                                     rhs=pl2[:, kc * 128:(kc + 1) * 128],
                                     start=(kc == 0), stop=(kc == 7))
                nc.scalar.activation(out=z2l2T[:], in_=psg[:, :128], func=AF.Copy)
                t1l2T = wpool.tile([128, 128], BF, tag="t1l2T", name="t1l2T")
                psg2 = pps.tile([128, 512], F32, tag="pp1", name="pp1")
                for kc in range(8):
                    nc.tensor.matmul(out=psg2[:, :128], lhsT=t1_l2[:, kc * 128:(kc + 1) * 128],
                                     rhs=pl2[:, kc * 128:(kc + 1) * 128],
                                     start=(kc == 0), stop=(kc == 7))
                nc.scalar.activation(out=t1l2T[:], in_=psg2[:, :128], func=AF.Copy)
                p2l2T = wpool.tile([128, 128], BF, tag="p2l2T", name="p2l2T")
                transp(p2n_l2[:], p2l2T[:])
                bw2 = [load_const(f"bigw2_{t}") for t in range(3)]
                bias3 = load_const("bias3", F32)
                ps3 = pps.tile([128, 512], F32, tag="pp1", name="pp1")
                for t, tap in enumerate((z2l2T, t1l2T, p2l2T)):
                    nc.tensor.matmul(out=ps3[:, :128], lhsT=bw2[t][:, :], rhs=tap[:],
                                     start=(t == 0), stop=(t == 2))
                z3T = wpool.tile([128, 128], BF, tag="z3T", name="z3T")
                nc.scalar.activation(out=z3T[:], in_=ps3[:, :128],
                                     func=AF.Tanh, bias=bias3[:, 0:1])
                z3n = wpool.tile([128, 128], BF, tag="z3n", name="z3n")
                transp(z3T[:], z3n[:])

            # ================= LEVEL 3 =================
            with nc.named_scope("l3"):
                s3t = load_const("S3T")
                bias4 = load_const("bias4", F32)
                bias5 = load_const("bias5", F32)

                def conv_l3(zn, zT, bw_pref, bias_t, func, keep):
                    t1T = wpool.tile([128, 128], BF, tag=keep + "t1T", name=keep + "t1T")
                    ps = pps.tile([128, 512], F32, tag="pp1", name="pp1")
                    nc.tensor.matmul(out=ps[:, :128], lhsT=zn, rhs=s3t[:], start=True, stop=True)
                    nc.scalar.activation(out=t1T[:], in_=ps[:, :128], func=AF.Copy)
                    t1n_ = wpool.tile([128, 128], BF, tag=keep + "t1n", name=keep + "t1n")
                    transp(t1T[:], t1n_[:])
                    p2T_ = wpool.tile([128, 128], BF, tag=keep + "p2T", name=keep + "p2T")
                    ps2 = pps.tile([128, 512], F32, tag="pp1", name="pp1")
                    nc.tensor.matmul(out=ps2[:, :128], lhsT=t1n_[:], rhs=s3t[:], start=True, stop=True)
                    nc.scalar.activation(out=p2T_[:], in_=ps2[:, :128], func=AF.Copy)
                    bw = [load_const(f"{bw_pref}_{t}") for t in range(3)]
                    outT = wpool.tile([128, 128], BF, tag=keep + "oT", name=keep + "oT")
                    ps4 = pps.tile([128, 512], F32, tag="pp1", name="pp1")
                    for t, tap in enumerate((zT, t1T[:], p2T_[:])):
                        nc.tensor.matmul(out=ps4[:, :128], lhsT=bw[t][:, :], rhs=tap,
                                         start=(t == 0), stop=(t == 2))
                    f2 = AF.Identity if func == AF.Copy else func
                    nc.scalar.activation(out=outT[:], in_=ps4[:, :128], func=f2,
                                         bias=bias_t[:, 0:1])
                    outn = wpool.tile([128, 128], BF, tag=keep + "on", name=keep + "on")
                    transp(outT[:], outn[:])
                    return outn, outT

                z4n, z4T = conv_l3(z3n[:], z3T[:], "bigw3", bias4, AF.Tanh, "c4")
                o5n, o5T = conv_l3(z4n[:], z4T[:], "bigw4", bias5, AF.Copy, "c5")

            # ================= MLP input assembly =================
            with nc.named_scope("mlp_in"):
                for j in range(4):
                    ap_out = x6_loc.ap()[:, j:j + 1].rearrange("(n h) o -> n (h o)", h=32)
                    nc.sync.dma_start(out=ap_out, in_=o5n[:, 32 * j:32 * j + 32])
                nc.gpsimd.collective_compute(
                    "AllGather", ALU.bypass, replica_groups=RG,
                    ins=[x6_loc.ap().opt()], outs=[x6_all.ap().opt()])

            # ================= MLP =================
            def mlp_layer(nm, src_sb, out_sb, wsb):
                g_t = load_const("g" + nm[1], F32)
                be_t = load_const("be" + nm[1], F32)
                # single PSUM bank for all 4 m-slices: start=True only on the
                # very first matmul (clears the whole bank's has_written bits);
                # each slice's first write then overwrites, later ones add.
                acc = pps.tile([128, 128], F32, tag="macc", name="macc", bufs=1)
                for i in range(4):
                    for a in range(8):
                        kc = i * 8 + a
                        for mm in range(4):
                            o = i * 4096 + a * 512 + mm * 128
                            nc.tensor.matmul(
                                out=acc[:, 32 * mm:32 * mm + 32],
                                lhsT=wsb[:, o:o + 128],
                                rhs=src_sb[:, 32 * kc:32 * kc + 32],
                                start=(kc == 0 and mm == 0), stop=(kc == 31))
                for mm in range(4):
                    t = wpool.tile([128, 32], F32, tag="b_t", name="b_t", bufs=4)
                    nc.vector.tensor_copy(t[:], acc[:, 32 * mm:32 * mm + 32])
                    s1 = wpool.tile([128, 1], F32, tag="b_s1", name="b_s1", bufs=4)
                    nc.vector.tensor_reduce(out=s1[:], in_=t[:], axis=AX.X, op=ALU.add)
                    mu_ = wpool.tile([128, 1], F32, tag="b_mu", name="b_mu", bufs=4)
                    nc.vector.tensor_scalar_mul(mu_[:], s1[:], 1.0 / 32.0)
                    sq = wpool.tile([128, 32], F32, tag="b_sq", name="b_sq", bufs=4)
                    nc.vector.tensor_mul(sq[:], t[:], t[:])
                    s2_ = wpool.tile([128, 1], F32, tag="b_s2", name="b_s2", bufs=4)
                    nc.vector.tensor_reduce(out=s2_[:], in_=sq[:], axis=AX.X, op=ALU.add)
                    var = wpool.tile([128, 1], F32, tag="b_var", name="b_var", bufs=4)
                    nc.vector.scalar_tensor_tensor(out=var[:], in0=mu_[:], scalar=-1.0,
                                                   in1=mu_[:], op0=ALU.mult, op1=ALU.mult)
                    nc.vector.scalar_tensor_tensor(out=var[:], in0=s2_[:], scalar=1.0 / 32.0,
                                                   in1=var[:], op0=ALU.mult, op1=ALU.add)
                    sd = wpool.tile([128, 1], F32, tag="b_sd", name="b_sd", bufs=4)
                    nc.scalar.activation(out=sd[:], in_=var[:], func=AF.Sqrt, bias=eps_t[:, 0:1])
                    rs = wpool.tile([128, 1], F32, tag="b_rs", name="b_rs", bufs=4)
                    nc.vector.reciprocal(rs[:], sd[:])
                    a_ = wpool.tile([128, 1], F32, tag="b_a", name="b_a", bufs=4)
                    nc.vector.tensor_mul(a_[:], rs[:], g_t[:, mm:mm + 1])
                    sh = wpool.tile([128, 1], F32, tag="b_sh", name="b_sh", bufs=4)
                    nc.vector.scalar_tensor_tensor(out=sh[:], in0=mu_[:], scalar=-1.0,
                                                   in1=a_[:], op0=ALU.mult, op1=ALU.mult)
                    nc.vector.tensor_add(sh[:], sh[:], be_t[:, mm:mm + 1])
                    nc.scalar.activation(out=out_sb[:, 32 * mm:32 * mm + 32], in_=t[:],
                                         func=AF.Relu, scale=a_[:, 0:1], bias=sh[:, 0:1])

            with nc.named_scope("mlp6"):
                x6T = cpool.tile([128, 1024], BF, tag="x6T", name="x6T")
                for kk in range(8):
                    nc.sync.dma_start(
                        out=x6T[:].rearrange("p (c r) -> p c r", r=32)[:, :, 4 * kk:4 * kk + 4],
                        in_=x6_all[4096 * kk:4096 * (kk + 1), :].rearrange(
                            "(c p) j -> p c j", p=128))
                h6 = cpool.tile([128, 128], BF, tag="h6sb", name="h6sb")
                mlp_layer("w6", x6T, h6, w6sb)
                nc.sync.dma_start(out=h6_loc.ap().rearrange("(m p) b -> p m b", p=128),
                                  in_=h6[:].rearrange("p (m b) -> p m b", b=32))
                nc.gpsimd.collective_compute(
                    "AllGather", ALU.bypass, replica_groups=RG,
                    ins=[h6_loc.ap().opt()], outs=[h6_all.ap().opt()])
            with nc.named_scope("mlp7"):
                x7T = cpool.tile([128, 1024], BF, tag="x7T", name="x7T")
                nc.sync.dma_start(out=x7T[:].rearrange("p (c b) -> p c b", b=32),
                                  in_=h6_all[:, :].rearrange("(c p) b -> p c b", p=128))
                h7 = cpool.tile([128, 128], BF, tag="h7sb", name="h7sb")
                mlp_layer("w7", x7T, h7, w7sb)
                nc.sync.dma_start(out=h7_loc.ap().rearrange("(m p) b -> p m b", p=128),
                                  in_=h7[:].rearrange("p (m b) -> p m b", b=32))
                nc.gpsimd.collective_compute(
                    "AllGather", ALU.bypass, replica_groups=RG,
                    ins=[h7_loc.ap().opt()], outs=[h7_all.ap().opt()])
            with nc.named_scope("mlp8"):
                x8T = cpool.tile([128, 1024], BF, tag="x8T", name="x8T")
                nc.sync.dma_start(out=x8T[:].rearrange("p (c b) -> p c b", b=32),
                                  in_=h7_all[:, :].rearrange("(c p) b -> p c b", p=128))
                h8 = cpool.tile([128, 128], BF, tag="h8sb", name="h8sb")
                mlp_layer("w8", x8T, h8, w8sb)

            with nc.named_scope("mlp9"):
                w9t = load_const("w9")
                ps9 = pps.tile([128, 128], F32, tag="macc", name="macc", bufs=1)
                for kc in range(4):
                    nc.tensor.matmul(out=ps9[:, :32], lhsT=w9t[:, kc * 128:(kc + 1) * 128],
                                     rhs=h8[:, 32 * kc:32 * kc + 32],
                                     start=(kc == 0), stop=(kc == 3))
                mu_sb = wpool.tile([128, 32], F32, tag="mu_sb", name="mu_sb")
                nc.vector.tensor_copy(mu_sb[:], ps9[:, :32])
                nc.sync.dma_start(out=mu_loc[:, :], in_=mu_sb[:])
                nc.gpsimd.collective_compute(
                    "AllGather", ALU.bypass, replica_groups=RG,
                    ins=[mu_loc.ap().opt()], outs=[mu_all.ap().opt()])
                mall = wpool.tile([128, 256], F32, tag="f_mall", name="f_mall")
                nc.sync.dma_start(
                    out=mall[:].rearrange("p (k b) -> p k b", b=32),
                    in_=mu_all.ap().rearrange("(k p) b -> p k b", p=128))
                tot = wpool.tile([128, 32], F32, tag="f_tot", name="f_tot")
                nc.vector.tensor_copy(tot[:], mall[:, 0:32])
                for k in range(1, 8):
                    nc.vector.tensor_add(tot[:], tot[:], mall[:, 32 * k:32 * k + 32])
                s1 = wpool.tile([128, 1], F32, tag="f_s1", name="f_s1")
                nc.vector.tensor_reduce(out=s1[:], in_=tot[:], axis=AX.X, op=ALU.add)
                mu_ = wpool.tile([128, 1], F32, tag="f_mu", name="f_mu")
                nc.vector.tensor_scalar_mul(mu_[:], s1[:], 1.0 / 32.0)
                sq = wpool.tile([128, 32], F32, tag="f_sq", name="f_sq")
                nc.vector.tensor_mul(sq[:], tot[:], tot[:])
                s2_ = wpool.tile([128, 1], F32, tag="f_s2", name="f_s2")
                nc.vector.tensor_reduce(out=s2_[:], in_=sq[:], axis=AX.X, op=ALU.add)
                var = wpool.tile([128, 1], F32, tag="f_var", name="f_var")
                nc.vector.scalar_tensor_tensor(out=var[:], in0=mu_[:], scalar=-1.0,
                                               in1=mu_[:], op0=ALU.mult, op1=ALU.mult)
                nc.vector.scalar_tensor_tensor(out=var[:], in0=s2_[:], scalar=1.0 / 32.0,
                                               in1=var[:], op0=ALU.mult, op1=ALU.add)
                sdf = wpool.tile([128, 1], F32, tag="f_sd", name="f_sd")
                nc.scalar.activation(out=sdf[:], in_=var[:], func=AF.Sqrt, bias=eps_t[:, 0:1])
                rs = wpool.tile([128, 1], F32, tag="f_rs", name="f_rs")
                nc.vector.reciprocal(rs[:], sdf[:])
                neg = wpool.tile([128, 1], F32, tag="f_neg", name="f_neg")
                nc.vector.scalar_tensor_tensor(out=neg[:], in0=mu_[:], scalar=-1.0,
                                               in1=rs[:], op0=ALU.mult, op1=ALU.mult)
                outt = wpool.tile([128, 32], F32, tag="f_out", name="f_out")
                nc.scalar.activation(out=outt[:], in_=tot[:], func=AF.Identity,
                                     scale=rs[:, 0:1], bias=neg[:, 0:1])
                nc.sync.dma_start(out=out_mu[:, :], in_=outt[:])

    nc.compile()
    return nc


# ---------------------------------------------------------------- entry point
def kernel(**inputs) -> np.ndarray:
    per_core, meta = _host_prep(inputs)
    if "prog" not in _CACHE:
        _CACHE["prog"] = _build_nc(meta, per_core[0])
    nc = _CACHE["prog"]
    res = bass_utils.run_bass_kernel_spmd(nc, per_core, core_ids=list(range(NCORES)))
    return np.ascontiguousarray(res.results[0]["mu"].T)


# revision 33
# speedup vs baseline: 2.5483x; 1.2270x over previous
"""Trainium2 Bass kernel for nn_Encoder_base (5x ChebConv GNN + pool + MLP).

Distribution over 8 NeuronCores (v2, bf16 data path):
  - level-0 prop1: edge-sharded by destination; source rows PRE-GATHERED on
    the host (x is a kernel input), selection matrices host-built in bf16
  - level-0 prop2 + level-1 props: destination-sharded with full-width
    (all-batch) node rows -> few fat dma_gather indices instead of many thin
    ones; AllGather of z1/t1 between stages
  - einsums (channel mixes) node-sharded, 8 batch-group passes each
  - levels 2-3: batch-sharded dense-S matmuls (as v1) in bf16
  - MLP: output-feature sharded bf16 weights (FWL), BatchNorm local per
    feature; final BN in fp32
"""
import numpy as np
import concourse.bass as bass
import concourse.bacc as bacc
import concourse.tile as tile
from concourse import mybir, bass_utils
from concourse.masks import make_identity

F32 = mybir.dt.float32
BF = mybir.dt.float16
I32 = mybir.dt.int32
I16 = mybir.dt.int16
NPBF = mybir.dt.np(BF)
AF = mybir.ActivationFunctionType
ALU = mybir.AluOpType
AX = mybir.AxisListType
RG = [list(range(8))]
NCORES = 8
N0, N1, N2, N3 = 16384, 4096, 1024, 128
EPS = 1e-5

_CACHE = {}


# ---------------------------------------------------------------- host prep
def _prep_prop(row, col, we, n_dest, n_shard):
    """Sorted-by-dest edges -> 128-dest windows, 128-edge chunks, padded so
    chunk counts per window match across shards (one SPMD program)."""
    window = 128
    order = np.argsort(row, kind="stable")
    row, col, we = row[order], col[order], we[order]
    per = n_dest // n_shard
    nwin = per // window
    counts = np.zeros((n_shard, nwin), np.int64)
    lists = {}
    for s in range(n_shard):
        lo = s * per
        for wi in range(nwin):
            wlo = lo + wi * window
            a = np.searchsorted(row, wlo, side="left")
            b = np.searchsorted(row, wlo + window, side="left")
            lists[(s, wi)] = (row[a:b] - wlo, col[a:b], we[a:b])
            counts[s, wi] = (b - a + 127) // 128
    ncw = np.maximum(counts.max(axis=0), 1)
    C = int(ncw.sum())
    src = np.zeros((n_shard, C, 128), np.int32)
    dst = np.full((n_shard, C, 128), 200.0, np.float32)
    wea = np.zeros((n_shard, C, 128), np.float32)
    for s in range(n_shard):
        base = 0
        for wi in range(nwin):
            dl, cl, wl = lists[(s, wi)]
            n = len(dl)
            k = int(ncw[wi])
            src[s, base:base + k].reshape(-1)[:n] = cl
            dst[s, base:base + k].reshape(-1)[:n] = dl
            wea[s, base:base + k].reshape(-1)[:n] = wl
            base += k
    return [int(x) for x in ncw], src, dst, wea


def _edge_we(e, n):
    row, col = np.asarray(e[0], np.int64), np.asarray(e[1], np.int64)
    deg = np.bincount(row, minlength=n).astype(np.float32)
    dis = np.where(deg > 0, 1.0 / np.sqrt(np.maximum(deg, 1.0)), 0.0).astype(np.float32)
    return row, col, -(dis[row] * dis[col]).astype(np.float32)


def _sub_edges(row, col, we, pool_idx):
    order = np.argsort(row, kind="stable")
    row, col, we = row[order], col[order], we[order]
    starts = np.searchsorted(row, pool_idx, side="left")
    ends = np.searchsorted(row, pool_idx, side="right")
    nr, ncl, nw = [], [], []
    for i in range(len(pool_idx)):
        s, e = starts[i], ends[i]
        if e > s:
            nr.append(np.full(e - s, i, np.int64))
            ncl.append(col[s:e])
            nw.append(we[s:e])
    return np.concatenate(nr), np.concatenate(ncl), np.concatenate(nw)


def _dense_s(row, col, we, n):
    s = np.zeros((n, n), np.float32)
    np.add.at(s, (row, col), we)
    return s


def _tile_w(w, pack):
    """[K, M] -> [K//(128*pack) * 128, pack*M]: pack K-blocks side by side."""
    k, m = w.shape
    nb = k // 128
    t = w.reshape(nb // pack, pack, 128, m).transpose(0, 2, 1, 3)
    return np.ascontiguousarray(t.reshape((nb // pack) * 128, pack * m))


def _sel_pack(dst, wea):
    """dst/wea [C, 128] -> bf16 selection blocks [128, C*128]."""
    C = dst.shape[0]
    sel = np.zeros((C, 128, 128), np.float32)
    c_idx, p_idx = np.meshgrid(np.arange(C), np.arange(128), indexing="ij")
    valid = dst < 128
    sel[c_idx[valid], p_idx[valid], dst[valid].astype(np.int64)] = wea[valid]
    return np.ascontiguousarray(
        sel.transpose(1, 0, 2).reshape(128, C * 128)).astype(NPBF)


def _rows_pack(table, src, width):
    """Pre-gathered rows: table [N, width], src [C, 128] -> [128, C*width]."""
    C = src.shape[0]
    g = table[src.reshape(-1)].reshape(C, 128, width)
    return np.ascontiguousarray(
        g.transpose(1, 0, 2).reshape(128, C * width)).astype(NPBF)


def _idx_pack(flat):
    return np.ascontiguousarray(
        np.tile(flat.astype(np.int16).reshape(-1, 16).T, (8, 1)))


def _host_prep(inputs):
    d = {k: np.asarray(v) for k, v in inputs.items()}
    x = d["x"].astype(np.float32)
    l0 = np.asarray(d["l0"], np.int64)
    l1 = np.asarray(d["l1"], np.int64)
    l2 = np.asarray(d["l2"], np.int64)

    X0 = np.ascontiguousarray(x.transpose(1, 0, 2).reshape(N0, 96))
    X0bf = X0.astype(NPBF)

    r0, c0, w0 = _edge_we(d["e0"], N0)
    ncw_p1, src_p1, dst_p1, we_p1 = _prep_prop(r0, c0, w0, N0, NCORES)
    r0s, c0s, w0s = _sub_edges(r0, c0, w0, l0)
    ncw_p2, src_p2, dst_p2, we_p2 = _prep_prop(r0s, c0s, w0s, N1, NCORES)

    r1, c1, w1 = _edge_we(d["e1"], N1)
    ncw_q1, src_q1, dst_q1, we_q1 = _prep_prop(r1, c1, w1, N1, NCORES)
    r1s, c1s, w1s = _sub_edges(r1, c1, w1, l1)
    ncw_q2, src_q2, dst_q2, we_q2 = _prep_prop(r1s, c1s, w1s, N2, NCORES)

    r2, c2, w2 = _edge_we(d["e2"], N2)
    S2 = _dense_s(r2, c2, w2, N2)
    S2T = _tile_w(np.ascontiguousarray(S2.T), 8).astype(NPBF)       # [128, 8192]
    S2l2T = _tile_w(np.ascontiguousarray(S2[l2].T), 8).astype(NPBF)  # [128, 1024]
    P_l2 = np.zeros((N2, 128), np.float32)
    P_l2[l2, np.arange(128)] = 1.0
    P_l2 = _tile_w(P_l2, 8).astype(NPBF)                             # [128, 1024]

    r3, c3, w3 = _edge_we(d["e3"], N3)
    S3T = np.ascontiguousarray(_dense_s(r3, c3, w3, N3).T).astype(NPBF)

    def wmod(W):
        return W[0] - W[2], W[1], 2.0 * W[2]

    Wm1 = wmod(d["Wc1"].astype(np.float32))
    Wm = [wmod(d[f"Wc{i}"].astype(np.float32)) for i in (2, 3, 4, 5)]
    eye4 = np.eye(4, dtype=np.float32)

    per_core = []
    for k in range(NCORES):
        m = {}
        m["epsv"] = np.full((128, 1), EPS, np.float32)
        # ---- p1: host-gathered x rows + host sel blocks
        m["p1_xg"] = _rows_pack(X0bf, src_p1[k], 96)
        m["p1_sel"] = _sel_pack(dst_p1[k], we_p1[k])
        # ---- p2: gather idx (into tx1_all) + sel
        m["p2_idx"] = _idx_pack(src_p2[k].reshape(-1))
        m["p2_sel"] = _sel_pack(dst_p2[k], we_p2[k])
        # ---- q1 / q2
        m["q1_idx"] = _idx_pack(src_q1[k].reshape(-1))
        m["q1_sel"] = _sel_pack(dst_q1[k], we_q1[k])
        m["q2_idx"] = _idx_pack(src_q2[k].reshape(-1))
        m["q2_sel"] = _sel_pack(dst_q2[k], we_q2[k])
        # ---- einsum l0 (node shard 512k..512k+512)
        l0s = l0[512 * k:512 * (k + 1)]
        m["g0T"] = np.ascontiguousarray(X0[l0s].T).astype(NPBF)  # [96, 512]
        m["l0_idx"] = _idx_pack(l0s)
        for g in range(8):
            for t in range(3):
                bw = np.zeros((96, 128), np.float32)
                for j in range(4):
                    b = 4 * g + j
                    bw[3 * b:3 * b + 3, 32 * j:32 * j + 32] = Wm1[t]
                m[f"bw0_{g}_{t}"] = bw.astype(NPBF)
        # ---- einsum l1 (node shard 128k..128k+128)
        m["l1_idx"] = _idx_pack(l1[128 * k:128 * (k + 1)])
        for lev in range(4):
            for t in range(3):
                m[f"bigw{lev + 1}_{t}"] = np.kron(eye4, Wm[lev][t]).astype(NPBF)
        for lev, nm in ((1, "b1"), (2, "b2"), (3, "b3"), (4, "b4"), (5, "b5")):
            m[f"bias{lev}"] = np.tile(d[nm].astype(np.float32), 4).reshape(128, 1)
        # ---- level 2/3 dense
        m["S2T"] = S2T
        m["S2l2T"] = S2l2T
        m["P_l2"] = P_l2
        m["S3T"] = S3T
        # ---- MLP (feature shard 512k..512k+512)
        for li in (6, 7, 8):
            W = d[f"W{li}"].astype(np.float32)[:, 512 * k:512 * k + 512]
            m[f"w{li}"] = _tile_w(W, 8).astype(NPBF)  # [512, 4096]
            m[f"g{li}"] = np.ascontiguousarray(
                d[f"g{li}"].astype(np.float32)[512 * k:512 * k + 512].reshape(4, 128).T)
            m[f"be{li}"] = np.ascontiguousarray(
                d[f"be{li}"].astype(np.float32)[512 * k:512 * k + 512].reshape(4, 128).T)
        m["w9"] = _tile_w(
            d["W9"].astype(np.float32)[512 * k:512 * k + 512], 4).astype(NPBF)
        # transpose-gather index tables for MLP activations
        m["x6g_idx"] = _idx_pack(np.array(
            [b * 4 + q for q in range(4) for b in range(32)], np.int64))
        m["h_idx"] = _idx_pack(np.arange(256, dtype=np.int64))
        per_core.append(m)

    meta = {"p1": ncw_p1, "p2": ncw_p2, "q1": ncw_q1, "q2": ncw_q2}
    return per_core, meta


# ---------------------------------------------------------------- device program
def _build_nc(meta, shapes):
    nc = bacc.Bacc("TRN2", target_bir_lowering=False, debug=False, num_devices=NCORES)
    dtmap = {np.dtype(np.int32): I32, np.dtype(np.int16): I16,
             np.dtype(NPBF): BF, np.dtype(np.float32): F32}
    ein = {}
    for name, arr in shapes.items():
        ein[name] = nc.dram_tensor(name, list(arr.shape), dtmap[arr.dtype],
                                   kind="ExternalInput")
    out_mu = nc.dram_tensor("mu", [128, 32], F32, kind="ExternalOutput")

    tx1_loc = nc.dram_tensor("tx1_loc", [N0 // 8, 128], BF)
    tx1_all = nc.dram_tensor("tx1_all", [N0, 128], BF, addr_space="Shared")
    z1_loc = nc.dram_tensor("z1_loc", [512, 1024], BF)
    z1_all = nc.dram_tensor("z1_all", [N1, 1024], BF, addr_space="Shared")
    t1_loc = nc.dram_tensor("t1_loc", [512, 1024], BF)
    t1_all = nc.dram_tensor("t1_all", [N1, 1024], BF, addr_space="Shared")
    z2_a2a_in = nc.dram_tensor("z2_a2a_in", [1024, 128], BF)
    z2_a2a_out = nc.dram_tensor("z2_a2a_out", [1024, 128], BF)
    x6_loc = nc.dram_tensor("x6_loc", [16, 1024], BF)
    x6_all = nc.dram_tensor("x6_all", [128, 1024], BF, addr_space="Shared")
    h6_loc = nc.dram_tensor("h6_loc", [32, 512], BF)
    h6_all = nc.dram_tensor("h6_all", [256, 512], BF, addr_space="Shared")
    h7_loc = nc.dram_tensor("h7_loc", [32, 512], BF)
    h7_all = nc.dram_tensor("h7_all", [256, 512], BF, addr_space="Shared")
    mu_loc = nc.dram_tensor("mu_loc", [128, 32], F32)
    mu_all = nc.dram_tensor("mu_all", [8 * 128, 32], F32, addr_space="Shared")

    C1 = sum(meta["p1"])
    C2 = sum(meta["p2"])
    C3 = sum(meta["q1"])
    C4 = sum(meta["q2"])

    with tile.TileContext(nc) as tc:
        with (
            tc.tile_pool(name="const", bufs=1) as cpool,
            tc.tile_pool(name="grp", bufs=2) as gpool,
            tc.tile_pool(name="zb", bufs=3) as zpool,
            tc.tile_pool(name="work", bufs=3) as wpool,
            tc.tile_pool(name="wbig", bufs=2) as wbpool,
            tc.tile_pool(name="ps_s", bufs=2, space="PSUM") as pps,
        ):
            identf = cpool.tile([128, 128], F32, tag="identf", name="identf")
            make_identity(nc, identf[:])
            identb = cpool.tile([128, 128], BF, tag="identb", name="identb")
            nc.vector.tensor_copy(identb[:], identf[:])
            eps_t = cpool.tile([128, 1], F32, tag="epsv", name="epsv")
            nc.sync.dma_start(out=eps_t[:], in_=ein["epsv"][:, :])

            # big weight preloads ride the SWDGE queue: transfers overlap the
            # GNN phase without blocking the HWDGE rings that feed it
            def preload_w(nm):
                t = wbpool.tile([128, 16384], BF, tag="wbig", name="wbig")
                nc.gpsimd.dma_start(
                    out=t[:].rearrange("p (i f) -> p i f", f=4096),
                    in_=ein[nm].ap().rearrange("(i p) f -> p i f", p=128))
                return t

            w6sb = preload_w("w6")
            s2t_sb = cpool.tile([128, 8192], BF, tag="S2T", name="S2T")
            nc.gpsimd.dma_start(out=s2t_sb[:], in_=ein["S2T"][:, :])

            def load_const(name, dt=BF):
                t = cpool.tile(list(shapes[name].shape), dt, tag=name)
                nc.sync.dma_start(out=t[:], in_=ein[name][:, :])
                return t

            def load_idx(name, ncols):
                t = cpool.tile([128, ncols], I16, tag=name, name=name)
                nc.sync.dma_start(out=t[:], in_=ein[name][:, :])
                return t

            # group loader for host-packed per-chunk arrays ([128, C*w] in DRAM)
            def mk_loader(ein_name, w, nchunks, grp, tag, eng):
                tiles = {}

                def get(cc):
                    g0 = (cc // grp) * grp
                    if g0 not in tiles:
                        gc = min(grp, nchunks - g0)
                        t = gpool.tile([128, grp * w], BF, tag=tag, name=tag)
                        eng.dma_start(out=t[:, :gc * w],
                                      in_=ein[ein_name][:, g0 * w:(g0 + gc) * w])
                        tiles[g0] = t
                    return tiles[g0], (cc % grp) * w
                return get

            # gather groups: idx_sb [128, nchunks*8] (128 idx per chunk)
            def mk_gather(idx_sb, src_dram, w, nchunks, grp, tag, bufs=3):
                tiles = {}

                def get(cc):
                    g0 = (cc // grp) * grp
                    if g0 not in tiles:
                        gc = min(grp, nchunks - g0)
                        t = zpool.tile([128, grp * w], BF, tag=tag, name=tag, bufs=bufs)
                        nc.gpsimd.dma_gather(
                            out_ap=t[:, :gc * w].rearrange("p (c e) -> p c e", e=w),
                            in_ap=src_dram[:, :],
                            idxs_ap=idx_sb[:, g0 * 8:(g0 + gc) * 8],
                            num_idxs=gc * 128, num_idxs_reg=gc * 128, elem_size=w,
                            single_packet=False)
                        tiles[g0] = t
                    return tiles[g0], (cc % grp) * w
                return get

            def transp(src_ap, dst_ap):
                p, f = src_ap.shape
                b0 = src_ap.base_partition()
                ps = pps.tile([128, 128], BF, tag="tps", name="tps")
                nc.tensor.transpose(out=ps[:f, :p], in_=src_ap,
                                    identity=identb[b0:b0 + p, b0:b0 + p])
                nc.scalar.activation(out=dst_ap, in_=ps[:f, :p], func=AF.Copy)

            # ================= LEVEL 0: prop1 (host-gathered sources) ========
            with nc.named_scope("l0_prop1"):
                xg = mk_loader("p1_xg", 96, C1, 16, "p1xg", nc.sync)
                sl = mk_loader("p1_sel", 128, C1, 16, "p1sel", nc.scalar)
                base = 0
                for wi, nch in enumerate(meta["p1"]):
                    ps = pps.tile([128, 512], F32, tag="pp1", name="pp1")
                    for c in range(nch):
                        cc = base + c
                        xt, xo = xg(cc)
                        st, so = sl(cc)
                        nc.tensor.matmul(out=ps[:, :96],
                                         lhsT=st[:, so:so + 128],
                                         rhs=xt[:, xo:xo + 96],
                                         start=(c == 0), stop=(c == nch - 1))
                    ev = wpool.tile([128, 96], BF, tag="p1ev", name="p1ev", bufs=4)
                    nc.vector.tensor_copy(ev[:], ps[:, :96])
                    nc.sync.dma_start(out=tx1_loc[wi * 128:(wi + 1) * 128, :96], in_=ev[:])
                    base += nch
            with nc.named_scope("ag1"):
                nc.gpsimd.collective_compute(
                    "AllGather", ALU.bypass, replica_groups=RG,
                    ins=[tx1_loc.ap().opt()], outs=[tx1_all.ap().opt()])

            # ================= LEVEL 0: prop2 (dest = own l0 shard) ==========
            p2T_sb = cpool.tile([96, 512], BF, tag="p2T_sb", name="p2T_sb")
            with nc.named_scope("l0_prop2"):
                p2i = load_idx("p2_idx", C2 * 8)
                sl2 = mk_loader("p2_sel", 128, C2, 16, "p2sel", nc.scalar)
                gz = mk_gather(p2i, tx1_all, 128, C2, 16, "p2zb", bufs=2)
                base = 0
                for wi, nch in enumerate(meta["p2"]):
                    ps = pps.tile([128, 512], F32, tag="pp1", name="pp1")
                    for c in range(nch):
                        cc = base + c
                        zt, zo = gz(cc)
                        st, so = sl2(cc)
                        nc.tensor.matmul(out=ps[:96, :128],
                                         lhsT=zt[:, zo:zo + 96],
                                         rhs=st[:, so:so + 128],
                                         start=(c == 0), stop=(c == nch - 1))
                    nc.scalar.activation(out=p2T_sb[:, wi * 128:(wi + 1) * 128],
                                         in_=ps[:96, :128], func=AF.Copy)
                    base += nch

            # ================= LEVEL 0: einsum -> z1 =========================
            with nc.named_scope("l0_einsum"):
                g0T = load_const("g0T")                      # [96, 512]
                l0i = load_idx("l0_idx", 32)
                gz1 = mk_gather(l0i, tx1_all, 128, 4, 4, "g1zb", bufs=1)
                g1T = cpool.tile([96, 512], BF, tag="g1T", name="g1T")
                for c in range(4):
                    zt, zo = gz1(c)
                    transp(zt[:, zo:zo + 96], g1T[:, c * 128:(c + 1) * 128])
                bias1 = load_const("bias1", F32)
                for g in range(8):
                    bw = [load_const(f"bw0_{g}_{t}") for t in range(3)]
                    ps = pps.tile([128, 512], F32, tag="pp1", name="pp1")
                    for t, tap in enumerate((g0T, g1T, p2T_sb)):
                        nc.tensor.matmul(out=ps[:, :512], lhsT=bw[t][:, :],
                                         rhs=tap[:, :], start=(t == 0), stop=(t == 2))
                    z1Tg = wpool.tile([128, 512], BF, tag="z1Tg", name="z1Tg", bufs=2)
                    nc.scalar.activation(out=z1Tg[:], in_=ps[:, :512],
                                         func=AF.Identity, bias=bias1[:, 0:1])
                    z1g = wpool.tile([128, 512], BF, tag="z1g", name="z1g", bufs=2)
                    for c in range(4):
                        transp(z1Tg[:, c * 128:(c + 1) * 128],
                               z1g[:, c * 128:(c + 1) * 128])
                    nc.sync.dma_start(
                        out=z1_loc[:, g * 128:(g + 1) * 128].rearrange(
                            "(c p) f -> p c f", p=128),
                        in_=z1g[:].rearrange("p (c f) -> p c f", f=128))
            with nc.named_scope("ag_z1"):
                nc.gpsimd.collective_compute(
                    "AllGather", ALU.bypass, replica_groups=RG,
                    ins=[z1_loc.ap().opt()], outs=[z1_all.ap().opt()])

            # ================= LEVEL 1: prop1 (dest-sharded, fat rows) =======
            with nc.named_scope("l1_prop1"):
                q1i = load_idx("q1_idx", C3 * 8)
                slq1 = mk_loader("q1_sel", 128, C3, 8, "q1sel", nc.scalar)
                gq1 = mk_gather(q1i, z1_all, 1024, C3, 4, "q1zb", bufs=2)
                base = 0
                for wi, nch in enumerate(meta["q1"]):
                    psh = [pps.tile([128, 512], F32, tag="pp1", name="pp1")
                           for _ in range(2)]
                    for c in range(nch):
                        cc = base + c
                        zt, zo = gq1(cc)
                        st, so = slq1(cc)
                        for h in range(2):
                            nc.tensor.matmul(
                                out=psh[h][:, :512],
                                lhsT=st[:, so:so + 128],
                                rhs=zt[:, zo + h * 512:zo + (h + 1) * 512],
                                start=(c == 0), stop=(c == nch - 1))
                    ev = wpool.tile([128, 1024], BF, tag="q1ev", name="q1ev", bufs=2)
                    for h in range(2):
                        nc.scalar.activation(out=ev[:, h * 512:(h + 1) * 512],
                                             in_=psh[h][:, :512], func=AF.Copy)
                    nc.sync.dma_start(out=t1_loc[wi * 128:(wi + 1) * 128, :], in_=ev[:])
                    base += nch
                # z1 einsum taps don't depend on t1: gather + transpose them
                # (and start the w7 preload) before the ag_t1 wait blocks gpsimd
                l1i = load_idx("l1_idx", 8)
                gzl1 = mk_gather(l1i, z1_all, 1024, 1, 1, "el1a", bufs=1)
                z1l1T = cpool.tile([128, 1024], BF, tag="z1l1T", name="z1l1T")
                zt_l1, _ = gzl1(0)
                for g in range(8):
                    transp(zt_l1[:, g * 128:(g + 1) * 128],
                           z1l1T[:, g * 128:(g + 1) * 128])
                w7sb = preload_w("w7")
            with nc.named_scope("ag_t1"):
                nc.gpsimd.collective_compute(
                    "AllGather", ALU.bypass, replica_groups=RG,
                    ins=[t1_loc.ap().opt()], outs=[t1_all.ap().opt()])

            # ================= LEVEL 1: prop2 (dest = own l1 shard) ==========
            p2q = cpool.tile([128, 1024], BF, tag="p2q", name="p2q")
            with nc.named_scope("l1_prop2"):
                # t1 einsum tap first on the gpsimd queue (tiny, unblocks
                # the l1_einsum transposes while q2 runs)
                gtl1 = mk_gather(l1i, t1_all, 1024, 1, 1, "el1b", bufs=1)
                tt_l1, _ = gtl1(0)
                q2i = load_idx("q2_idx", C4 * 8)
                slq2 = mk_loader("q2_sel", 128, C4, 8, "q2sel", nc.scalar)
                gq2 = mk_gather(q2i, t1_all, 1024, C4, 4, "q1zb", bufs=2)
                psh = [pps.tile([128, 512], F32, tag="pp1", name="pp1")
                       for _ in range(2)]
                for c in range(C4):
                    zt, zo = gq2(c)
                    st, so = slq2(c)
                    for h in range(2):
                        nc.tensor.matmul(
                            out=psh[h][:, :512],
                            lhsT=st[:, so:so + 128],
                            rhs=zt[:, zo + h * 512:zo + (h + 1) * 512],
                            start=(c == 0), stop=(c == C4 - 1))
                for h in range(2):
                    nc.scalar.activation(out=p2q[:, h * 512:(h + 1) * 512],
                                         in_=psh[h][:, :512], func=AF.Copy)

            # ================= LEVEL 1: einsum -> z2 =========================
            with nc.named_scope("l1_einsum"):
                w8sb = preload_w("w8")
                t1l1T = cpool.tile([128, 1024], BF, tag="t1l1T", name="t1l1T")
                p2qT = cpool.tile([128, 1024], BF, tag="p2qT", name="p2qT")
                for g in range(8):
                    transp(tt_l1[:, g * 128:(g + 1) * 128], t1l1T[:, g * 128:(g + 1) * 128])
                    transp(p2q[:, g * 128:(g + 1) * 128], p2qT[:, g * 128:(g + 1) * 128])
                bw1 = [load_const(f"bigw1_{t}") for t in range(3)]
                bias2 = load_const("bias2", F32)
                for g in range(8):
                    ps = pps.tile([128, 512], F32, tag="pp1", name="pp1")
                    for t, tap in enumerate((z1l1T, t1l1T, p2qT)):
                        nc.tensor.matmul(out=ps[:, :128], lhsT=bw1[t][:, :],
                                         rhs=tap[:, g * 128:(g + 1) * 128],
                                         start=(t == 0), stop=(t == 2))
                    z2Tg = wpool.tile([128, 128], BF, tag="z2Tg", name="z2Tg")
                    nc.scalar.activation(out=z2Tg[:], in_=ps[:, :128],
                                         func=AF.Tanh, bias=bias2[:, 0:1])
                    z2ng = wpool.tile([128, 128], BF, tag="z2ng", name="z2ng", bufs=4)
                    transp(z2Tg[:], z2ng[:])
                    nc.sync.dma_start(out=z2_a2a_in[g * 128:(g + 1) * 128, :],
                                      in_=z2ng[:])
            with nc.named_scope("a2a_z2"):
                nc.gpsimd.collective_compute(
                    "AllToAll", ALU.bypass, replica_groups=RG,
                    ins=[z2_a2a_in.ap().opt()], outs=[z2_a2a_out.ap().opt()])

            # ================= LEVEL 2 (dense, batch-sharded) ================
            with nc.named_scope("l2"):
                # z2n: [128 nodes, 128 own-feats] tiles, k-tile kc
                z2n = cpool.tile([128, 1024], BF, tag="z2n", name="z2n")
                nc.sync.dma_start(
                    out=z2n[:].rearrange("p (c f) -> p c f", f=128),
                    in_=z2_a2a_out.ap().rearrange("(c p) f -> p c f", p=128))
                t1_l2 = cpool.tile([128, 1024], BF, tag="t1_l2", name="t1_l2")
                s2t = s2t_sb
                for dc in range(8):
                    ps = pps.tile([128, 512], F32, tag="pp1", name="pp1")
                    for kc in range(8):
                        nc.tensor.matmul(
                            out=ps[:, :128],
                            lhsT=s2t[:, kc * 1024 + dc * 128: kc * 1024 + dc * 128 + 128],
                            rhs=z2n[:, kc * 128:(kc + 1) * 128],
                            start=(kc == 0), stop=(kc == 7))
                    nc.scalar.activation(out=t1_l2[:, dc * 128:(dc + 1) * 128],
                                         in_=ps[:, :128], func=AF.Copy)
                s2l2 = load_const("S2l2T")
                ps = pps.tile([128, 512], F32, tag="pp1", name="pp1")
                for kc in range(8):
                    nc.tensor.matmul(out=ps[:, :128], lhsT=s2l2[:, kc * 128:(kc + 1) * 128],
                                     rhs=t1_l2[:, kc * 128:(kc + 1) * 128],
                                     start=(kc == 0), stop=(kc == 7))
                p2n_l2 = wpool.tile([128, 128], BF, tag="p2n_l2", name="p2n_l2")
                nc.scalar.activation(out=p2n_l2[:], in_=ps[:, :128], func=AF.Copy)
                pl2 = load_const("P_l2")
                z2l2T = wpool.tile([128, 128], BF, tag="z2l2T", name="z2l2T")
                psg = pps.tile([128, 512], F32, tag="pp1", name="pp1")
                for kc in range(8):
                    nc.tensor.matmul(out=psg[:, :128], lhsT=z2n[:, kc * 128:(kc + 1) * 128],
                                     rhs=pl2[:, kc * 128:(kc + 1) * 128],
                                     start=(kc == 0), stop=(kc == 7))
                nc.scalar.activation(out=z2l2T[:], in_=psg[:, :128], func=AF.Copy)
                t1l2T = wpool.tile([128, 128], BF, tag="t1l2T", name="t1l2T")
                psg2 = pps.tile([128, 512], F32, tag="pp1", name="pp1")
                for kc in range(8):
                    nc.tensor.matmul(out=psg2[:, :128], lhsT=t1_l2[:, kc * 128:(kc + 1) * 128],
                                     rhs=pl2[:, kc * 128:(kc + 1) * 128],
                                     start=(kc == 0), stop=(kc == 7))
                nc.scalar.activation(out=t1l2T[:], in_=psg2[:, :128], func=AF.Copy)
                p2l2T = wpool.tile([128, 128], BF, tag="p2l2T", name="p2l2T")
                transp(p2n_l2[:], p2l2T[:])
                bw2 = [load_const(f"bigw2_{t}") for t in range(3)]
                bias3 = load_const("bias3", F32)
                ps3 = pps.tile([128, 512], F32, tag="pp1", name="pp1")
                for t, tap in enumerate((z2l2T, t1l2T, p2l2T)):
                    nc.tensor.matmul(out=ps3[:, :128], lhsT=bw2[t][:, :], rhs=tap[:],
                                     start=(t == 0), stop=(t == 2))
                z3T = wpool.tile([128, 128], BF, tag="z3T", name="z3T")
                nc.scalar.activation(out=z3T[:], in_=ps3[:, :128],
                                     func=AF.Tanh, bias=bias3[:, 0:1])
                z3n = wpool.tile([128, 128], BF, tag="z3n", name="z3n")
                transp(z3T[:], z3n[:])

            # ================= LEVEL 3 =================
            with nc.named_scope("l3"):
                s3t = load_const("S3T")
                bias4 = load_const("bias4", F32)
                bias5 = load_const("bias5", F32)

                def conv_l3(zn, zT, bw_pref, bias_t, func, keep):
                    t1T = wpool.tile([128, 128], BF, tag=keep + "t1T", name=keep + "t1T")
                    ps = pps.tile([128, 512], F32, tag="pp1", name="pp1")
                    nc.tensor.matmul(out=ps[:, :128], lhsT=zn, rhs=s3t[:], start=True, stop=True)
                    nc.scalar.activation(out=t1T[:], in_=ps[:, :128], func=AF.Copy)
                    t1n_ = wpool.tile([128, 128], BF, tag=keep + "t1n", name=keep + "t1n")
                    transp(t1T[:], t1n_[:])
                    p2T_ = wpool.tile([128, 128], BF, tag=keep + "p2T", name=keep + "p2T")
                    ps2 = pps.tile([128, 512], F32, tag="pp1", name="pp1")
                    nc.tensor.matmul(out=ps2[:, :128], lhsT=t1n_[:], rhs=s3t[:], start=True, stop=True)
                    nc.scalar.activation(out=p2T_[:], in_=ps2[:, :128], func=AF.Copy)
                    bw = [load_const(f"{bw_pref}_{t}") for t in range(3)]
                    outT = wpool.tile([128, 128], BF, tag=keep + "oT", name=keep + "oT")
                    ps4 = pps.tile([128, 512], F32, tag="pp1", name="pp1")
                    for t, tap in enumerate((zT, t1T[:], p2T_[:])):
                        nc.tensor.matmul(out=ps4[:, :128], lhsT=bw[t][:, :], rhs=tap,
                                         start=(t == 0), stop=(t == 2))
                    f2 = AF.Identity if func == AF.Copy else func
                    nc.scalar.activation(out=outT[:], in_=ps4[:, :128], func=f2,
                                         bias=bias_t[:, 0:1])
                    outn = wpool.tile([128, 128], BF, tag=keep + "on", name=keep + "on")
                    transp(outT[:], outn[:])
                    return outn, outT

                z4n, z4T = conv_l3(z3n[:], z3T[:], "bigw3", bias4, AF.Tanh, "c4")
                o5n, o5T = conv_l3(z4n[:], z4T[:], "bigw4", bias5, AF.Copy, "c5")

            # ================= MLP input assembly (batch-major rows) =========
            with nc.named_scope("mlp_in"):
                # x6_loc rows 4*jb+q (jb=own batch, q=node quarter), 1024 feats
                for jb in range(4):
                    nc.sync.dma_start(
                        out=x6_loc.ap()[4 * jb:4 * jb + 4, :].rearrange(
                            "q (nn c) -> (q nn) c", c=32),
                        in_=o5n[:, 32 * jb:32 * jb + 32])
                nc.gpsimd.collective_compute(
                    "AllGather", ALU.bypass, replica_groups=RG,
                    ins=[x6_loc.ap().opt()], outs=[x6_all.ap().opt()])

            # ================= MLP =================
            def mlp_layer(nm, rhs_of, out_sb, wsb):
                g_t = load_const("g" + nm[1], F32)
                be_t = load_const("be" + nm[1], F32)
                # single PSUM bank for all 4 m-slices: start=True only on the
                # very first matmul (clears the whole bank's has_written bits);
                # each slice's first write then overwrites, later ones add.
                acc = pps.tile([128, 128], F32, tag="macc", name="macc", bufs=1)
                for i in range(4):
                    for a in range(8):
                        kc = i * 8 + a
                        for mm in range(4):
                            o = i * 4096 + a * 512 + mm * 128
                            nc.tensor.matmul(
                                out=acc[:, 32 * mm:32 * mm + 32],
                                lhsT=wsb[:, o:o + 128],
                                rhs=rhs_of(kc),
                                start=(kc == 0 and mm == 0), stop=(kc == 31))
                for mm in range(4):
                    t = wpool.tile([128, 32], F32, tag="b_t", name="b_t", bufs=4)
                    nc.vector.tensor_copy(t[:], acc[:, 32 * mm:32 * mm + 32])
                    s1 = wpool.tile([128, 1], F32, tag="b_s1", name="b_s1", bufs=4)
                    nc.vector.tensor_reduce(out=s1[:], in_=t[:], axis=AX.X, op=ALU.add)
                    mu_ = wpool.tile([128, 1], F32, tag="b_mu", name="b_mu", bufs=4)
                    nc.vector.tensor_scalar_mul(mu_[:], s1[:], 1.0 / 32.0)
                    sq = wpool.tile([128, 32], F32, tag="b_sq", name="b_sq", bufs=4)
                    nc.vector.tensor_mul(sq[:], t[:], t[:])
                    s2_ = wpool.tile([128, 1], F32, tag="b_s2", name="b_s2", bufs=4)
                    nc.vector.tensor_reduce(out=s2_[:], in_=sq[:], axis=AX.X, op=ALU.add)
                    var = wpool.tile([128, 1], F32, tag="b_var", name="b_var", bufs=4)
                    nc.vector.scalar_tensor_tensor(out=var[:], in0=mu_[:], scalar=-1.0,
                                                   in1=mu_[:], op0=ALU.mult, op1=ALU.mult)
                    nc.vector.scalar_tensor_tensor(out=var[:], in0=s2_[:], scalar=1.0 / 32.0,
                                                   in1=var[:], op0=ALU.mult, op1=ALU.add)
                    sd = wpool.tile([128, 1], F32, tag="b_sd", name="b_sd", bufs=4)
                    nc.scalar.activation(out=sd[:], in_=var[:], func=AF.Sqrt, bias=eps_t[:, 0:1])
                    rs = wpool.tile([128, 1], F32, tag="b_rs", name="b_rs", bufs=4)
                    nc.vector.reciprocal(rs[:], sd[:])
                    a_ = wpool.tile([128, 1], F32, tag="b_a", name="b_a", bufs=4)
                    nc.vector.tensor_mul(a_[:], rs[:], g_t[:, mm:mm + 1])
                    sh = wpool.tile([128, 1], F32, tag="b_sh", name="b_sh", bufs=4)
                    nc.vector.scalar_tensor_tensor(out=sh[:], in0=mu_[:], scalar=-1.0,
                                                   in1=a_[:], op0=ALU.mult, op1=ALU.mult)
                    nc.vector.tensor_add(sh[:], sh[:], be_t[:, mm:mm + 1])
                    nc.scalar.activation(out=out_sb[:, 32 * mm:32 * mm + 32], in_=t[:],
                                         func=AF.Relu, scale=a_[:, 0:1], bias=sh[:, 0:1])

            def h_to_batch_major(h_sb, loc_dram):
                hb = wpool.tile([32, 512], BF, tag="hB", name="hB", bufs=2)
                for m in range(4):
                    transp(h_sb[:, 32 * m:32 * m + 32], hb[:, m * 128:(m + 1) * 128])
                nc.sync.dma_start(out=loc_dram[:, :], in_=hb[:])

            def gather_xT(idx_t, table, elem, n_idx, tag):
                t = wpool.tile([128, 1024], BF, tag="xg", name=tag, bufs=2)
                nc.gpsimd.dma_gather(
                    out_ap=t[:].rearrange("p (c i) -> p c i", i=n_idx),
                    in_ap=table[:, :], idxs_ap=idx_t[:, :],
                    num_idxs=n_idx, num_idxs_reg=n_idx, elem_size=elem,
                    transpose=True, single_packet=False)
                return t

            with nc.named_scope("mlp6"):
                x6gi = load_idx("x6g_idx", 8)
                x6g = gather_xT(x6gi, x6_all, 1024, 128, "x6g")
                h6 = cpool.tile([128, 128], BF, tag="h6sb", name="h6sb")
                mlp_layer("w6", lambda kc: x6g[:, (kc % 8) * 128 + (kc // 8) * 32:
                                               (kc % 8) * 128 + (kc // 8) * 32 + 32],
                          h6, w6sb)
                h_to_batch_major(h6, h6_loc)
                nc.gpsimd.collective_compute(
                    "AllGather", ALU.bypass, replica_groups=RG,
                    ins=[h6_loc.ap().opt()], outs=[h6_all.ap().opt()])
            with nc.named_scope("mlp7"):
                hgi = load_idx("h_idx", 16)
                x7g = gather_xT(hgi, h6_all, 512, 256, "x7g")
                h7 = cpool.tile([128, 128], BF, tag="h7sb", name="h7sb")
                mlp_layer("w7", lambda kc: x7g[:, (kc % 4) * 256 + (kc // 4) * 32:
                                               (kc % 4) * 256 + (kc // 4) * 32 + 32],
                          h7, w7sb)
                h_to_batch_major(h7, h7_loc)
                nc.gpsimd.collective_compute(
                    "AllGather", ALU.bypass, replica_groups=RG,
                    ins=[h7_loc.ap().opt()], outs=[h7_all.ap().opt()])
            with nc.named_scope("mlp8"):
                x8g = gather_xT(hgi, h7_all, 512, 256, "x8g")
                h8 = cpool.tile([128, 128], BF, tag="h8sb", name="h8sb")
                mlp_layer("w8", lambda kc: x8g[:, (kc % 4) * 256 + (kc // 4) * 32:
                                               (kc % 4) * 256 + (kc // 4) * 32 + 32],
                          h8, w8sb)

            with nc.named_scope("mlp9"):
                w9t = load_const("w9")
                ps9 = pps.tile([128, 128], F32, tag="macc", name="macc", bufs=1)
                for kc in range(4):
                    nc.tensor.matmul(out=ps9[:, :32], lhsT=w9t[:, kc * 128:(kc + 1) * 128],
                                     rhs=h8[:, 32 * kc:32 * kc + 32],
                                     start=(kc == 0), stop=(kc == 3))
                mu_sb = wpool.tile([128, 32], F32, tag="mu_sb", name="mu_sb")
                nc.vector.tensor_copy(mu_sb[:], ps9[:, :32])
                nc.sync.dma_start(out=mu_loc[:, :], in_=mu_sb[:])
                nc.gpsimd.collective_compute(
                    "AllGather", ALU.bypass, replica_groups=RG,
                    ins=[mu_loc.ap().opt()], outs=[mu_all.ap().opt()])
                mall = wpool.tile([128, 256], F32, tag="f_mall", name="f_mall")
                nc.sync.dma_start(
                    out=mall[:].rearrange("p (k b) -> p k b", b=32),
                    in_=mu_all.ap().rearrange("(k p) b -> p k b", p=128))
                tot = wpool.tile([128, 32], F32, tag="f_tot", name="f_tot")
                nc.vector.tensor_copy(tot[:], mall[:, 0:32])
                for k in range(1, 8):
                    nc.vector.tensor_add(tot[:], tot[:], mall[:, 32 * k:32 * k + 32])
                s1 = wpool.tile([128, 1], F32, tag="f_s1", name="f_s1")
                nc.vector.tensor_reduce(out=s1[:], in_=tot[:], axis=AX.X, op=ALU.add)
                mu_ = wpool.tile([128, 1], F32, tag="f_mu", name="f_mu")
                nc.vector.tensor_scalar_mul(mu_[:], s1[:], 1.0 / 32.0)
                sq = wpool.tile([128, 32], F32, tag="f_sq", name="f_sq")
                nc.vector.tensor_mul(sq[:], tot[:], tot[:])
                s2_ = wpool.tile([128, 1], F32, tag="f_s2", name="f_s2")
                nc.vector.tensor_reduce(out=s2_[:], in_=sq[:], axis=AX.X, op=ALU.add)
                var = wpool.tile([128, 1], F32, tag="f_var", name="f_var")
                nc.vector.scalar_tensor_tensor(out=var[:], in0=mu_[:], scalar=-1.0,
                                               in1=mu_[:], op0=ALU.mult, op1=ALU.mult)
                nc.vector.scalar_tensor_tensor(out=var[:], in0=s2_[:], scalar=1.0 / 32.0,
                                               in1=var[:], op0=ALU.mult, op1=ALU.add)
                sdf = wpool.tile([128, 1], F32, tag="f_sd", name="f_sd")
                nc.scalar.activation(out=sdf[:], in_=var[:], func=AF.Sqrt, bias=eps_t[:, 0:1])
                rs = wpool.tile([128, 1], F32, tag="f_rs", name="f_rs")
                nc.vector.reciprocal(rs[:], sdf[:])
                neg = wpool.tile([128, 1], F32, tag="f_neg", name="f_neg")
                nc.vector.scalar_tensor_tensor(out=neg[:], in0=mu_[:], scalar=-1.0,
                                               in1=rs[:], op0=ALU.mult, op1=ALU.mult)
                outt = wpool.tile([128, 32], F32, tag="f_out", name="f_out")
                nc.scalar.activation(out=outt[:], in_=tot[:], func=AF.Identity,
                                     scale=rs[:, 0:1], bias=neg[:, 0:1])
                nc.sync.dma_start(out=out_mu[:, :], in_=outt[:])

    nc.compile()
    return nc


# ---------------------------------------------------------------- entry point
def kernel(**inputs) -> np.ndarray:
    per_core, meta = _host_prep(inputs)
    if "prog" not in _CACHE:
        _CACHE["prog"] = _build_nc(meta, per_core[0])
    nc = _CACHE["prog"]
    res = bass_utils.run_bass_kernel_spmd(nc, per_core, core_ids=list(range(NCORES)))
    return np.ascontiguousarray(res.results[0]["mu"].T)


# revision 39
# speedup vs baseline: 2.5616x; 1.0052x over previous
"""Trainium2 Bass kernel for nn_Encoder_base (5x ChebConv GNN + pool + MLP).

Distribution over 8 NeuronCores (v2, bf16 data path):
  - level-0 prop1: edge-sharded by destination; source rows PRE-GATHERED on
    the host (x is a kernel input), selection matrices host-built in bf16
  - level-0 prop2 + level-1 props: destination-sharded with full-width
    (all-batch) node rows -> few fat dma_gather indices instead of many thin
    ones; AllGather of z1/t1 between stages
  - einsums (channel mixes) node-sharded, 8 batch-group passes each
  - levels 2-3: batch-sharded dense-S matmuls (as v1) in bf16
  - MLP: output-feature sharded bf16 weights (FWL), BatchNorm local per
    feature; final BN in fp32
"""
import numpy as np
import concourse.bass as bass
import concourse.bacc as bacc
import concourse.tile as tile
from concourse import mybir, bass_utils
from concourse.masks import make_identity

F32 = mybir.dt.float32
BF = mybir.dt.float16
I32 = mybir.dt.int32
I16 = mybir.dt.int16
NPBF = mybir.dt.np(BF)
AF = mybir.ActivationFunctionType
ALU = mybir.AluOpType
AX = mybir.AxisListType
RG = [list(range(8))]
NCORES = 8
N0, N1, N2, N3 = 16384, 4096, 1024, 128
EPS = 1e-5

_CACHE = {}


# ---------------------------------------------------------------- host prep
def _prep_prop(row, col, we, n_dest, n_shard):
    """Sorted-by-dest edges -> 128-dest windows, 128-edge chunks, padded so
    chunk counts per window match across shards (one SPMD program)."""
    window = 128
    order = np.argsort(row, kind="stable")
    row, col, we = row[order], col[order], we[order]
    per = n_dest // n_shard
    nwin = per // window
    counts = np.zeros((n_shard, nwin), np.int64)
    lists = {}
    for s in range(n_shard):
        lo = s * per
        for wi in range(nwin):
            wlo = lo + wi * window
            a = np.searchsorted(row, wlo, side="left")
            b = np.searchsorted(row, wlo + window, side="left")
            lists[(s, wi)] = (row[a:b] - wlo, col[a:b], we[a:b])
            counts[s, wi] = (b - a + 127) // 128
    ncw = np.maximum(counts.max(axis=0), 1)
    C = int(ncw.sum())
    src = np.zeros((n_shard, C, 128), np.int32)
    dst = np.full((n_shard, C, 128), 200.0, np.float32)
    wea = np.zeros((n_shard, C, 128), np.float32)
    for s in range(n_shard):
        base = 0
        for wi in range(nwin):
            dl, cl, wl = lists[(s, wi)]
            n = len(dl)
            k = int(ncw[wi])
            src[s, base:base + k].reshape(-1)[:n] = cl
            dst[s, base:base + k].reshape(-1)[:n] = dl
            wea[s, base:base + k].reshape(-1)[:n] = wl
            base += k
    return [int(x) for x in ncw], src, dst, wea


def _edge_we(e, n):
    row, col = np.asarray(e[0], np.int64), np.asarray(e[1], np.int64)
    deg = np.bincount(row, minlength=n).astype(np.float32)
    dis = np.where(deg > 0, 1.0 / np.sqrt(np.maximum(deg, 1.0)), 0.0).astype(np.float32)
    return row, col, -(dis[row] * dis[col]).astype(np.float32)


def _sub_edges(row, col, we, pool_idx):
    order = np.argsort(row, kind="stable")
    row, col, we = row[order], col[order], we[order]
    starts = np.searchsorted(row, pool_idx, side="left")
    ends = np.searchsorted(row, pool_idx, side="right")
    nr, ncl, nw = [], [], []
    for i in range(len(pool_idx)):
        s, e = starts[i], ends[i]
        if e > s:
            nr.append(np.full(e - s, i, np.int64))
            ncl.append(col[s:e])
            nw.append(we[s:e])
    return np.concatenate(nr), np.concatenate(ncl), np.concatenate(nw)


def _dense_s(row, col, we, n):
    s = np.zeros((n, n), np.float32)
    np.add.at(s, (row, col), we)
    return s


def _tile_w(w, pack):
    """[K, M] -> [K//(128*pack) * 128, pack*M]: pack K-blocks side by side."""
    k, m = w.shape
    nb = k // 128
    t = w.reshape(nb // pack, pack, 128, m).transpose(0, 2, 1, 3)
    return np.ascontiguousarray(t.reshape((nb // pack) * 128, pack * m))


def _sel_pack(dst, wea):
    """dst/wea [C, 128] -> bf16 selection blocks [128, C*128]."""
    C = dst.shape[0]
    sel = np.zeros((C, 128, 128), np.float32)
    c_idx, p_idx = np.meshgrid(np.arange(C), np.arange(128), indexing="ij")
    valid = dst < 128
    sel[c_idx[valid], p_idx[valid], dst[valid].astype(np.int64)] = wea[valid]
    return np.ascontiguousarray(
        sel.transpose(1, 0, 2).reshape(128, C * 128)).astype(NPBF)


def _rows_pack(table, src, width):
    """Pre-gathered rows: table [N, width], src [C, 128] -> [128, C*width]."""
    C = src.shape[0]
    g = table[src.reshape(-1)].reshape(C, 128, width)
    return np.ascontiguousarray(
        g.transpose(1, 0, 2).reshape(128, C * width)).astype(NPBF)


def _idx_pack(flat):
    return np.ascontiguousarray(
        np.tile(flat.astype(np.int16).reshape(-1, 16).T, (8, 1)))


def _host_prep(inputs):
    d = {k: np.asarray(v) for k, v in inputs.items()}
    x = d["x"].astype(np.float32)
    l0 = np.asarray(d["l0"], np.int64)
    l1 = np.asarray(d["l1"], np.int64)
    l2 = np.asarray(d["l2"], np.int64)

    X0 = np.ascontiguousarray(x.transpose(1, 0, 2).reshape(N0, 96))
    X0bf = X0.astype(NPBF)

    r0, c0, w0 = _edge_we(d["e0"], N0)
    ncw_p1, src_p1, dst_p1, we_p1 = _prep_prop(r0, c0, w0, N0, NCORES)
    r0s, c0s, w0s = _sub_edges(r0, c0, w0, l0)
    ncw_p2, src_p2, dst_p2, we_p2 = _prep_prop(r0s, c0s, w0s, N1, NCORES)

    r1, c1, w1 = _edge_we(d["e1"], N1)
    ncw_q1, src_q1, dst_q1, we_q1 = _prep_prop(r1, c1, w1, N1, NCORES)
    r1s, c1s, w1s = _sub_edges(r1, c1, w1, l1)
    ncw_q2, src_q2, dst_q2, we_q2 = _prep_prop(r1s, c1s, w1s, N2, NCORES)

    r2, c2, w2 = _edge_we(d["e2"], N2)
    S2 = _dense_s(r2, c2, w2, N2)
    S2T = _tile_w(np.ascontiguousarray(S2.T), 8).astype(NPBF)       # [128, 8192]
    S2l2T = _tile_w(np.ascontiguousarray(S2[l2].T), 8).astype(NPBF)  # [128, 1024]
    P_l2 = np.zeros((N2, 128), np.float32)
    P_l2[l2, np.arange(128)] = 1.0
    P_l2 = _tile_w(P_l2, 8).astype(NPBF)                             # [128, 1024]

    r3, c3, w3 = _edge_we(d["e3"], N3)
    S3T = np.ascontiguousarray(_dense_s(r3, c3, w3, N3).T).astype(NPBF)

    def wmod(W):
        return W[0] - W[2], W[1], 2.0 * W[2]

    Wm1 = wmod(d["Wc1"].astype(np.float32))
    Wm = [wmod(d[f"Wc{i}"].astype(np.float32)) for i in (2, 3, 4, 5)]
    eye4 = np.eye(4, dtype=np.float32)

    per_core = []
    for k in range(NCORES):
        m = {}
        m["epsv"] = np.full((128, 1), EPS, np.float32)
        # ---- p1: host-gathered x rows + host sel blocks
        m["p1_xg"] = _rows_pack(X0bf, src_p1[k], 96)
        m["p1_sel"] = _sel_pack(dst_p1[k], we_p1[k])
        # ---- p2: gather idx (into tx1_all) + sel
        m["p2_idx"] = _idx_pack(src_p2[k].reshape(-1))
        m["p2_sel"] = _sel_pack(dst_p2[k], we_p2[k])
        # ---- q1 / q2
        m["q1_idx"] = _idx_pack(src_q1[k].reshape(-1))
        m["q1_sel"] = _sel_pack(dst_q1[k], we_q1[k])
        m["q2_idx"] = _idx_pack(src_q2[k].reshape(-1))
        m["q2_sel"] = _sel_pack(dst_q2[k], we_q2[k])
        # ---- einsum l0 (node shard 512k..512k+512)
        l0s = l0[512 * k:512 * (k + 1)]
        m["g0T"] = np.ascontiguousarray(X0[l0s].T).astype(NPBF)  # [96, 512]
        m["l0_idx"] = _idx_pack(l0s)
        for g in range(8):
            for t in range(3):
                bw = np.zeros((96, 128), np.float32)
                for j in range(4):
                    b = 4 * g + j
                    bw[3 * b:3 * b + 3, 32 * j:32 * j + 32] = Wm1[t]
                m[f"bw0_{g}_{t}"] = bw.astype(NPBF)
        # ---- einsum l1 (node shard 128k..128k+128)
        m["l1_idx"] = _idx_pack(l1[128 * k:128 * (k + 1)])
        for lev in range(4):
            for t in range(3):
                m[f"bigw{lev + 1}_{t}"] = np.kron(eye4, Wm[lev][t]).astype(NPBF)
        for lev, nm in ((1, "b1"), (2, "b2"), (3, "b3"), (4, "b4"), (5, "b5")):
            m[f"bias{lev}"] = np.tile(d[nm].astype(np.float32), 4).reshape(128, 1)
        # ---- level 2/3 dense
        m["S2T"] = S2T
        m["S2l2T"] = S2l2T
        m["P_l2"] = P_l2
        m["S3T"] = S3T
        # ---- MLP (feature shard 512k..512k+512)
        m["ones32"] = np.ones((32, 1), np.float32)
        m["one1x32"] = np.ones((1, 32), np.float32)
        for li in (6, 7, 8):
            W = d[f"W{li}"].astype(np.float32)[:, 512 * k:512 * k + 512]
            m[f"w{li}"] = _tile_w(W, 8).astype(NPBF)  # [512, 4096]
            m[f"gb{li}"] = d[f"g{li}"].astype(np.float32)[
                512 * k:512 * k + 512].reshape(1, 512).copy()
            m[f"bb{li}"] = d[f"be{li}"].astype(np.float32)[
                512 * k:512 * k + 512].reshape(1, 512).copy()
        m["w9"] = _tile_w(
            d["W9"].astype(np.float32)[512 * k:512 * k + 512], 4).astype(NPBF)
        # transpose-gather index tables for MLP activations
        m["x6g_idx"] = _idx_pack(np.array(
            [b * 4 + q for q in range(4) for b in range(32)], np.int64))
        m["h_idx"] = _idx_pack(np.arange(256, dtype=np.int64))
        per_core.append(m)

    meta = {"p1": ncw_p1, "p2": ncw_p2, "q1": ncw_q1, "q2": ncw_q2}
    return per_core, meta


# ---------------------------------------------------------------- device program
def _build_nc(meta, shapes):
    nc = bacc.Bacc("TRN2", target_bir_lowering=False, debug=False, num_devices=NCORES)
    dtmap = {np.dtype(np.int32): I32, np.dtype(np.int16): I16,
             np.dtype(NPBF): BF, np.dtype(np.float32): F32}
    ein = {}
    for name, arr in shapes.items():
        ein[name] = nc.dram_tensor(name, list(arr.shape), dtmap[arr.dtype],
                                   kind="ExternalInput")
    out_mu = nc.dram_tensor("mu", [128, 32], F32, kind="ExternalOutput")

    tx1_loc = nc.dram_tensor("tx1_loc", [N0 // 8, 128], BF)
    tx1_all = nc.dram_tensor("tx1_all", [N0, 128], BF, addr_space="Shared")
    z1_loc = nc.dram_tensor("z1_loc", [512, 1024], BF)
    z1_all = nc.dram_tensor("z1_all", [N1, 1024], BF, addr_space="Shared")
    t1_loc = nc.dram_tensor("t1_loc", [512, 1024], BF)
    t1_all = nc.dram_tensor("t1_all", [N1, 1024], BF, addr_space="Shared")
    z2_a2a_in = nc.dram_tensor("z2_a2a_in", [1024, 128], BF)
    z2_a2a_out = nc.dram_tensor("z2_a2a_out", [1024, 128], BF)
    x6_loc = nc.dram_tensor("x6_loc", [16, 1024], BF)
    x6_all = nc.dram_tensor("x6_all", [128, 1024], BF, addr_space="Shared")
    h6_loc = nc.dram_tensor("h6_loc", [32, 512], BF)
    h6_all = nc.dram_tensor("h6_all", [256, 512], BF, addr_space="Shared")
    h7_loc = nc.dram_tensor("h7_loc", [32, 512], BF)
    h7_all = nc.dram_tensor("h7_all", [256, 512], BF, addr_space="Shared")
    dmy_loc = nc.dram_tensor("dmy_loc", [16, 16], BF)
    dmy_all = nc.dram_tensor("dmy_all", [128, 16], BF, addr_space="Shared")
    mu_loc = nc.dram_tensor("mu_loc", [128, 32], F32)
    mu_all = nc.dram_tensor("mu_all", [8 * 128, 32], F32, addr_space="Shared")

    C1 = sum(meta["p1"])
    C2 = sum(meta["p2"])
    C3 = sum(meta["q1"])
    C4 = sum(meta["q2"])

    with tile.TileContext(nc) as tc:
        with (
            tc.tile_pool(name="const", bufs=1) as cpool,
            tc.tile_pool(name="grp", bufs=2) as gpool,
            tc.tile_pool(name="zb", bufs=3) as zpool,
            tc.tile_pool(name="work", bufs=3) as wpool,
            tc.tile_pool(name="wbig", bufs=2) as wbpool,
            tc.tile_pool(name="ps_s", bufs=2, space="PSUM") as pps,
        ):
            identf = cpool.tile([128, 128], F32, tag="identf", name="identf")
            make_identity(nc, identf[:])
            identb = cpool.tile([128, 128], BF, tag="identb", name="identb")
            nc.vector.tensor_copy(identb[:], identf[:])
            eps_t = cpool.tile([128, 1], F32, tag="epsv", name="epsv")
            nc.sync.dma_start(out=eps_t[:], in_=ein["epsv"][:, :])

            # big weight preloads ride the SWDGE queue: transfers overlap the
            # GNN phase without blocking the HWDGE rings that feed it
            def preload_w(nm):
                halves = []
                for hh in range(2):
                    t = wbpool.tile([128, 8192], BF, tag="wbig", name="wbig")
                    nc.gpsimd.dma_start(
                        out=t[:].rearrange("p (i f) -> p i f", f=4096),
                        in_=ein[nm].ap().rearrange("(i p) f -> p i f", p=128)
                        [:, 2 * hh:2 * hh + 2, :])
                    halves.append(t)
                return halves

            w6sb = preload_w("w6")
            s2t_sb = cpool.tile([128, 8192], BF, tag="S2T", name="S2T")
            nc.gpsimd.dma_start(out=s2t_sb[:], in_=ein["S2T"][:, :])
            # fire a tiny collective immediately: absorbs the one-time
            # collective-comm init barrier (~60us) behind the level-0 compute
            nc.gpsimd.collective_compute(
                "AllGather", ALU.bypass, replica_groups=RG,
                ins=[dmy_loc.ap().opt()], outs=[dmy_all.ap().opt()])

            def load_const(name, dt=BF):
                t = cpool.tile(list(shapes[name].shape), dt, tag=name)
                nc.sync.dma_start(out=t[:], in_=ein[name][:, :])
                return t

            def load_idx(name, ncols):
                t = cpool.tile([128, ncols], I16, tag=name, name=name)
                nc.sync.dma_start(out=t[:], in_=ein[name][:, :])
                return t

            # group loader for host-packed per-chunk arrays ([128, C*w] in DRAM)
            def mk_loader(ein_name, w, nchunks, grp, tag, eng):
                tiles = {}

                def get(cc):
                    g0 = (cc // grp) * grp
                    if g0 not in tiles:
                        gc = min(grp, nchunks - g0)
                        t = gpool.tile([128, grp * w], BF, tag=tag, name=tag)
                        eng.dma_start(out=t[:, :gc * w],
                                      in_=ein[ein_name][:, g0 * w:(g0 + gc) * w])
                        tiles[g0] = t
                    return tiles[g0], (cc % grp) * w
                return get

            # gather groups: idx_sb [128, nchunks*8] (128 idx per chunk)
            def mk_gather(idx_sb, src_dram, w, nchunks, grp, tag, bufs=3):
                tiles = {}

                def get(cc):
                    g0 = (cc // grp) * grp
                    if g0 not in tiles:
                        gc = min(grp, nchunks - g0)
                        t = zpool.tile([128, grp * w], BF, tag=tag, name=tag, bufs=bufs)
                        nc.gpsimd.dma_gather(
                            out_ap=t[:, :gc * w].rearrange("p (c e) -> p c e", e=w),
                            in_ap=src_dram[:, :],
                            idxs_ap=idx_sb[:, g0 * 8:(g0 + gc) * 8],
                            num_idxs=gc * 128, num_idxs_reg=gc * 128, elem_size=w,
                            single_packet=False)
                        tiles[g0] = t
                    return tiles[g0], (cc % grp) * w
                return get

            def transp(src_ap, dst_ap):
                p, f = src_ap.shape
                b0 = src_ap.base_partition()
                ps = pps.tile([128, 128], BF, tag="tps", name="tps")
                nc.tensor.transpose(out=ps[:f, :p], in_=src_ap,
                                    identity=identb[b0:b0 + p, b0:b0 + p])
                nc.scalar.activation(out=dst_ap, in_=ps[:f, :p], func=AF.Copy)

            # ================= LEVEL 0: prop1 (host-gathered sources) ========
            with nc.named_scope("l0_prop1"):
                xg = mk_loader("p1_xg", 96, C1, 16, "p1xg", nc.sync)
                sl = mk_loader("p1_sel", 128, C1, 16, "p1sel", nc.scalar)
                base = 0
                for wi, nch in enumerate(meta["p1"]):
                    ps = pps.tile([128, 512], F32, tag="pp1", name="pp1")
                    for c in range(nch):
                        cc = base + c
                        xt, xo = xg(cc)
                        st, so = sl(cc)
                        nc.tensor.matmul(out=ps[:, :96],
                                         lhsT=st[:, so:so + 128],
                                         rhs=xt[:, xo:xo + 96],
                                         start=(c == 0), stop=(c == nch - 1))
                    ev = wpool.tile([128, 96], BF, tag="p1ev", name="p1ev", bufs=4)
                    nc.vector.tensor_copy(ev[:], ps[:, :96])
                    nc.sync.dma_start(out=tx1_loc[wi * 128:(wi + 1) * 128, :96], in_=ev[:])
                    base += nch
            with nc.named_scope("ag1"):
                nc.gpsimd.collective_compute(
                    "AllGather", ALU.bypass, replica_groups=RG,
                    ins=[tx1_loc.ap().opt()], outs=[tx1_all.ap().opt()])

            # ================= LEVEL 0: prop2 (dest = own l0 shard) ==========
            p2T_sb = cpool.tile([96, 512], BF, tag="p2T_sb", name="p2T_sb")
            with nc.named_scope("l0_prop2"):
                p2i = load_idx("p2_idx", C2 * 8)
                sl2 = mk_loader("p2_sel", 128, C2, 16, "p2sel", nc.scalar)
                gz = mk_gather(p2i, tx1_all, 128, C2, 16, "p2zb", bufs=2)
                base = 0
                for wi, nch in enumerate(meta["p2"]):
                    ps = pps.tile([128, 512], F32, tag="pp1", name="pp1")
                    for c in range(nch):
                        cc = base + c
                        zt, zo = gz(cc)
                        st, so = sl2(cc)
                        nc.tensor.matmul(out=ps[:96, :128],
                                         lhsT=zt[:, zo:zo + 96],
                                         rhs=st[:, so:so + 128],
                                         start=(c == 0), stop=(c == nch - 1))
                    nc.scalar.activation(out=p2T_sb[:, wi * 128:(wi + 1) * 128],
                                         in_=ps[:96, :128], func=AF.Copy)
                    base += nch

            # ================= LEVEL 0: einsum -> z1 =========================
            with nc.named_scope("l0_einsum"):
                g0T = load_const("g0T")                      # [96, 512]
                l0i = load_idx("l0_idx", 32)
                gz1 = mk_gather(l0i, tx1_all, 128, 4, 4, "g1zb", bufs=1)
                g1T = cpool.tile([96, 512], BF, tag="g1T", name="g1T")
                for c in range(4):
                    zt, zo = gz1(c)
                    transp(zt[:, zo:zo + 96], g1T[:, c * 128:(c + 1) * 128])
                bias1 = load_const("bias1", F32)
                for g in range(8):
                    bw = [load_const(f"bw0_{g}_{t}") for t in range(3)]
                    ps = pps.tile([128, 512], F32, tag="pp1", name="pp1")
                    for t, tap in enumerate((g0T, g1T, p2T_sb)):
                        nc.tensor.matmul(out=ps[:, :512], lhsT=bw[t][:, :],
                                         rhs=tap[:, :], start=(t == 0), stop=(t == 2))
                    z1Tg = wpool.tile([128, 512], BF, tag="z1Tg", name="z1Tg", bufs=2)
                    nc.scalar.activation(out=z1Tg[:], in_=ps[:, :512],
                                         func=AF.Identity, bias=bias1[:, 0:1])
                    z1g = wpool.tile([128, 512], BF, tag="z1g", name="z1g", bufs=2)
                    for c in range(4):
                        transp(z1Tg[:, c * 128:(c + 1) * 128],
                               z1g[:, c * 128:(c + 1) * 128])
                    nc.sync.dma_start(
                        out=z1_loc[:, g * 128:(g + 1) * 128].rearrange(
                            "(c p) f -> p c f", p=128),
                        in_=z1g[:].rearrange("p (c f) -> p c f", f=128))
            with nc.named_scope("ag_z1"):
                nc.gpsimd.collective_compute(
                    "AllGather", ALU.bypass, replica_groups=RG,
                    ins=[z1_loc.ap().opt()], outs=[z1_all.ap().opt()])

            # ================= LEVEL 1: prop1 (dest-sharded, fat rows) =======
            with nc.named_scope("l1_prop1"):
                q1i = load_idx("q1_idx", C3 * 8)
                slq1 = mk_loader("q1_sel", 128, C3, 8, "q1sel", nc.scalar)
                gq1 = mk_gather(q1i, z1_all, 1024, C3, 4, "q1zb", bufs=2)
                base = 0
                for wi, nch in enumerate(meta["q1"]):
                    psh = [pps.tile([128, 512], F32, tag="pp1", name="pp1")
                           for _ in range(2)]
                    for c in range(nch):
                        cc = base + c
                        zt, zo = gq1(cc)
                        st, so = slq1(cc)
                        for h in range(2):
                            nc.tensor.matmul(
                                out=psh[h][:, :512],
                                lhsT=st[:, so:so + 128],
                                rhs=zt[:, zo + h * 512:zo + (h + 1) * 512],
                                start=(c == 0), stop=(c == nch - 1))
                    ev = wpool.tile([128, 1024], BF, tag="q1ev", name="q1ev", bufs=2)
                    for h in range(2):
                        nc.scalar.activation(out=ev[:, h * 512:(h + 1) * 512],
                                             in_=psh[h][:, :512], func=AF.Copy)
                    nc.sync.dma_start(out=t1_loc[wi * 128:(wi + 1) * 128, :], in_=ev[:])
                    base += nch
                # z1 einsum taps don't depend on t1: gather + transpose them
                # (and start the w7 preload) before the ag_t1 wait blocks gpsimd
                l1i = load_idx("l1_idx", 8)
                gzl1 = mk_gather(l1i, z1_all, 1024, 1, 1, "el1a", bufs=1)
                z1l1T = cpool.tile([128, 1024], BF, tag="z1l1T", name="z1l1T")
                zt_l1, _ = gzl1(0)
                for g in range(8):
                    transp(zt_l1[:, g * 128:(g + 1) * 128],
                           z1l1T[:, g * 128:(g + 1) * 128])
            with nc.named_scope("ag_t1"):
                nc.gpsimd.collective_compute(
                    "AllGather", ALU.bypass, replica_groups=RG,
                    ins=[t1_loc.ap().opt()], outs=[t1_all.ap().opt()])

            # ================= LEVEL 1: prop2 (dest = own l1 shard) ==========
            p2q = cpool.tile([128, 1024], BF, tag="p2q", name="p2q")
            with nc.named_scope("l1_prop2"):
                # t1 einsum tap first on the gpsimd queue (tiny, unblocks
                # the l1_einsum transposes while q2 runs)
                gtl1 = mk_gather(l1i, t1_all, 1024, 1, 1, "el1b", bufs=1)
                tt_l1, _ = gtl1(0)
                q2i = load_idx("q2_idx", C4 * 8)
                slq2 = mk_loader("q2_sel", 128, C4, 8, "q2sel", nc.scalar)
                gq2 = mk_gather(q2i, t1_all, 1024, C4, 4, "q1zb", bufs=2)
                psh = [pps.tile([128, 512], F32, tag="pp1", name="pp1")
                       for _ in range(2)]
                for c in range(C4):
                    zt, zo = gq2(c)
                    st, so = slq2(c)
                    for h in range(2):
                        nc.tensor.matmul(
                            out=psh[h][:, :512],
                            lhsT=st[:, so:so + 128],
                            rhs=zt[:, zo + h * 512:zo + (h + 1) * 512],
                            start=(c == 0), stop=(c == C4 - 1))
                for h in range(2):
                    nc.scalar.activation(out=p2q[:, h * 512:(h + 1) * 512],
                                         in_=psh[h][:, :512], func=AF.Copy)

            # ================= LEVEL 1: einsum -> z2 =========================
            with nc.named_scope("l1_einsum"):
                t1l1T = cpool.tile([128, 1024], BF, tag="t1l1T", name="t1l1T")
                p2qT = cpool.tile([128, 1024], BF, tag="p2qT", name="p2qT")
                for g in range(8):
                    transp(tt_l1[:, g * 128:(g + 1) * 128], t1l1T[:, g * 128:(g + 1) * 128])
                    transp(p2q[:, g * 128:(g + 1) * 128], p2qT[:, g * 128:(g + 1) * 128])
                bw1 = [load_const(f"bigw1_{t}") for t in range(3)]
                bias2 = load_const("bias2", F32)
                for g in range(8):
                    ps = pps.tile([128, 512], F32, tag="pp1", name="pp1")
                    for t, tap in enumerate((z1l1T, t1l1T, p2qT)):
                        nc.tensor.matmul(out=ps[:, :128], lhsT=bw1[t][:, :],
                                         rhs=tap[:, g * 128:(g + 1) * 128],
                                         start=(t == 0), stop=(t == 2))
                    z2Tg = wpool.tile([128, 128], BF, tag="z2Tg", name="z2Tg")
                    nc.scalar.activation(out=z2Tg[:], in_=ps[:, :128],
                                         func=AF.Tanh, bias=bias2[:, 0:1])
                    z2ng = wpool.tile([128, 128], BF, tag="z2ng", name="z2ng", bufs=4)
                    transp(z2Tg[:], z2ng[:])
                    nc.sync.dma_start(out=z2_a2a_in[g * 128:(g + 1) * 128, :],
                                      in_=z2ng[:])
            with nc.named_scope("a2a_z2"):
                nc.gpsimd.collective_compute(
                    "AllToAll", ALU.bypass, replica_groups=RG,
                    ins=[z2_a2a_in.ap().opt()], outs=[z2_a2a_out.ap().opt()])

            # ================= LEVEL 2 (dense, batch-sharded) ================
            with nc.named_scope("l2"):
                # z2n: [128 nodes, 128 own-feats] tiles, k-tile kc
                z2n = cpool.tile([128, 1024], BF, tag="z2n", name="z2n")
                nc.sync.dma_start(
                    out=z2n[:].rearrange("p (c f) -> p c f", f=128),
                    in_=z2_a2a_out.ap().rearrange("(c p) f -> p c f", p=128))
                t1_l2 = cpool.tile([128, 1024], BF, tag="t1_l2", name="t1_l2")
                s2t = s2t_sb
                for dc in range(8):
                    ps = pps.tile([128, 512], F32, tag="pp1", name="pp1")
                    for kc in range(8):
                        nc.tensor.matmul(
                            out=ps[:, :128],
                            lhsT=s2t[:, kc * 1024 + dc * 128: kc * 1024 + dc * 128 + 128],
                            rhs=z2n[:, kc * 128:(kc + 1) * 128],
                            start=(kc == 0), stop=(kc == 7))
                    nc.scalar.activation(out=t1_l2[:, dc * 128:(dc + 1) * 128],
                                         in_=ps[:, :128], func=AF.Copy)
                s2l2 = load_const("S2l2T")
                ps = pps.tile([128, 512], F32, tag="pp1", name="pp1")
                for kc in range(8):
                    nc.tensor.matmul(out=ps[:, :128], lhsT=s2l2[:, kc * 128:(kc + 1) * 128],
                                     rhs=t1_l2[:, kc * 128:(kc + 1) * 128],
                                     start=(kc == 0), stop=(kc == 7))
                p2n_l2 = wpool.tile([128, 128], BF, tag="p2n_l2", name="p2n_l2")
                nc.scalar.activation(out=p2n_l2[:], in_=ps[:, :128], func=AF.Copy)
                pl2 = load_const("P_l2")
                z2l2T = wpool.tile([128, 128], BF, tag="z2l2T", name="z2l2T")
                psg = pps.tile([128, 512], F32, tag="pp1", name="pp1")
                for kc in range(8):
                    nc.tensor.matmul(out=psg[:, :128], lhsT=z2n[:, kc * 128:(kc + 1) * 128],
                                     rhs=pl2[:, kc * 128:(kc + 1) * 128],
                                     start=(kc == 0), stop=(kc == 7))
                nc.scalar.activation(out=z2l2T[:], in_=psg[:, :128], func=AF.Copy)
                t1l2T = wpool.tile([128, 128], BF, tag="t1l2T", name="t1l2T")
                psg2 = pps.tile([128, 512], F32, tag="pp1", name="pp1")
                for kc in range(8):
                    nc.tensor.matmul(out=psg2[:, :128], lhsT=t1_l2[:, kc * 128:(kc + 1) * 128],
                                     rhs=pl2[:, kc * 128:(kc + 1) * 128],
                                     start=(kc == 0), stop=(kc == 7))
                nc.scalar.activation(out=t1l2T[:], in_=psg2[:, :128], func=AF.Copy)
                p2l2T = wpool.tile([128, 128], BF, tag="p2l2T", name="p2l2T")
                transp(p2n_l2[:], p2l2T[:])
                bw2 = [load_const(f"bigw2_{t}") for t in range(3)]
                bias3 = load_const("bias3", F32)
                ps3 = pps.tile([128, 512], F32, tag="pp1", name="pp1")
                for t, tap in enumerate((z2l2T, t1l2T, p2l2T)):
                    nc.tensor.matmul(out=ps3[:, :128], lhsT=bw2[t][:, :], rhs=tap[:],
                                     start=(t == 0), stop=(t == 2))
                z3T = wpool.tile([128, 128], BF, tag="z3T", name="z3T")
                nc.scalar.activation(out=z3T[:], in_=ps3[:, :128],
                                     func=AF.Tanh, bias=bias3[:, 0:1])
                z3n = wpool.tile([128, 128], BF, tag="z3n", name="z3n")
                transp(z3T[:], z3n[:])

            # ================= LEVEL 3 =================
            with nc.named_scope("l3"):
                s3t = load_const("S3T")
                bias4 = load_const("bias4", F32)
                bias5 = load_const("bias5", F32)

                def conv_l3(zn, zT, bw_pref, bias_t, func, keep):
                    t1T = wpool.tile([128, 128], BF, tag=keep + "t1T", name=keep + "t1T")
                    ps = pps.tile([128, 512], F32, tag="pp1", name="pp1")
                    nc.tensor.matmul(out=ps[:, :128], lhsT=zn, rhs=s3t[:], start=True, stop=True)
                    nc.scalar.activation(out=t1T[:], in_=ps[:, :128], func=AF.Copy)
                    t1n_ = wpool.tile([128, 128], BF, tag=keep + "t1n", name=keep + "t1n")
                    transp(t1T[:], t1n_[:])
                    p2T_ = wpool.tile([128, 128], BF, tag=keep + "p2T", name=keep + "p2T")
                    ps2 = pps.tile([128, 512], F32, tag="pp1", name="pp1")
                    nc.tensor.matmul(out=ps2[:, :128], lhsT=t1n_[:], rhs=s3t[:], start=True, stop=True)
                    nc.scalar.activation(out=p2T_[:], in_=ps2[:, :128], func=AF.Copy)
                    bw = [load_const(f"{bw_pref}_{t}") for t in range(3)]
                    outT = wpool.tile([128, 128], BF, tag=keep + "oT", name=keep + "oT")
                    ps4 = pps.tile([128, 512], F32, tag="pp1", name="pp1")
                    for t, tap in enumerate((zT, t1T[:], p2T_[:])):
                        nc.tensor.matmul(out=ps4[:, :128], lhsT=bw[t][:, :], rhs=tap,
                                         start=(t == 0), stop=(t == 2))
                    f2 = AF.Identity if func == AF.Copy else func
                    nc.scalar.activation(out=outT[:], in_=ps4[:, :128], func=f2,
                                         bias=bias_t[:, 0:1])
                    outn = wpool.tile([128, 128], BF, tag=keep + "on", name=keep + "on")
                    transp(outT[:], outn[:])
                    return outn, outT

                z4n, z4T = conv_l3(z3n[:], z3T[:], "bigw3", bias4, AF.Tanh, "c4")
                o5n, o5T = conv_l3(z4n[:], z4T[:], "bigw4", bias5, AF.Copy, "c5")

            # ================= MLP input assembly (batch-major rows) =========
            with nc.named_scope("mlp_in"):
                # x6_loc rows 4*jb+q (jb=own batch, q=node quarter), 1024 feats
                for jb in range(4):
                    nc.sync.dma_start(
                        out=x6_loc.ap()[4 * jb:4 * jb + 4, :].rearrange(
                            "q (nn c) -> (q nn) c", c=32),
                        in_=o5n[:, 32 * jb:32 * jb + 32])
                nc.gpsimd.collective_compute(
                    "AllGather", ALU.bypass, replica_groups=RG,
                    ins=[x6_loc.ap().opt()], outs=[x6_all.ap().opt()])

            # ================= MLP =================
            ones32 = load_const("ones32", F32)
            one1x32 = load_const("one1x32", F32)

            def gather_xT(idx_t, table, elem, n_idx, tag):
                t = wpool.tile([128, 1024], BF, tag="xg", name=tag, bufs=2)
                nc.gpsimd.dma_gather(
                    out_ap=t[:].rearrange("p (c i) -> p c i", i=n_idx),
                    in_ap=table[:, :], idxs_ap=idx_t[:, :],
                    num_idxs=n_idx, num_idxs_reg=n_idx, elem_size=elem,
                    transpose=True, single_packet=False)
                return t

            def mlp_layer(nm, xg_of, wsb):
                gb = load_const("gb" + nm[1], F32)
                bb = load_const("bb" + nm[1], F32)
                acc = pps.tile([32, 512], F32, tag="macc", name="macc", bufs=1)
                for kc in range(32):
                    wt = wsb[kc // 16]
                    nc.tensor.matmul(out=acc[:, :512], lhsT=xg_of(kc),
                                     rhs=wt[:, (kc % 16) * 512:(kc % 16 + 1) * 512],
                                     start=(kc == 0), stop=(kc == 31))
                h = wpool.tile([32, 512], F32, tag="mh", name="mh", bufs=1)
                nc.vector.tensor_copy(h[:], acc[:, :512])
                hsq = wpool.tile([32, 512], F32, tag="mhsq", name="mhsq", bufs=1)
                nc.vector.tensor_mul(hsq[:], h[:], h[:])
                st1 = pps.tile([1, 512], F32, tag="mst", name="mst", bufs=2)
                nc.tensor.matmul(out=st1[:, :512], lhsT=ones32[:, :], rhs=h[:],
                                 start=True, stop=True)
                st2 = pps.tile([1, 512], F32, tag="mst", name="mst", bufs=2)
                nc.tensor.matmul(out=st2[:, :512], lhsT=ones32[:, :], rhs=hsq[:],
                                 start=True, stop=True)
                # stats all on partition 0: [mu | var | a | c]
                s = wpool.tile([1, 2048], F32, tag="mstat", name="mstat", bufs=1)
                mu_, va_, aa_, cc_ = (s[0:1, 512 * i:512 * (i + 1)] for i in range(4))
                nc.vector.tensor_scalar_mul(mu_, st1[:1, :512], 1.0 / 32.0)
                nc.vector.tensor_scalar_mul(va_, st2[:1, :512], 1.0 / 32.0)
                nc.vector.tensor_mul(aa_, mu_, mu_)
                nc.vector.tensor_tensor(va_, va_, aa_, op=ALU.subtract)
                nc.scalar.activation(out=aa_, in_=va_, func=AF.Sqrt,
                                     bias=eps_t[0:1, 0:1])
                nc.vector.reciprocal(aa_, aa_)
                nc.vector.tensor_mul(aa_, aa_, gb[:])
                nc.vector.tensor_mul(cc_, mu_, aa_)
                nc.vector.tensor_tensor(cc_, bb[:], cc_, op=ALU.subtract)
                # broadcast a/c to 32 batch partitions via K=1 matmuls
                pb = pps.tile([32, 512], F32, tag="macc", name="macc", bufs=1)
                nc.tensor.matmul(out=pb[:, :512], lhsT=one1x32[:, :], rhs=aa_,
                                 start=True, stop=True)
                ab = wpool.tile([32, 512], F32, tag="mab", name="mab", bufs=1)
                nc.vector.tensor_copy(ab[:], pb[:, :512])
                pb2 = pps.tile([32, 512], F32, tag="macc", name="macc", bufs=1)
                nc.tensor.matmul(out=pb2[:, :512], lhsT=one1x32[:, :], rhs=cc_,
                                 start=True, stop=True)
                cb = wpool.tile([32, 512], F32, tag="mcb", name="mcb", bufs=1)
                nc.vector.tensor_copy(cb[:], pb2[:, :512])
                ha = wpool.tile([32, 512], F32, tag="mha", name="mha", bufs=1)
                nc.vector.tensor_mul(ha[:], h[:], ab[:])
                nc.vector.tensor_add(ha[:], ha[:], cb[:])
                h16 = wpool.tile([32, 512], BF, tag="mh16", name="mh16", bufs=2)
                nc.vector.tensor_scalar(out=h16[:], in0=ha[:], scalar1=0.0,
                                        scalar2=None, op0=ALU.max)
                return h16

            with nc.named_scope("mlp6"):
                x6gi = load_idx("x6g_idx", 8)
                x6g = gather_xT(x6gi, x6_all, 1024, 128, "x6g")
                w7sb = preload_w("w7")
                h6 = mlp_layer("w6", lambda kc: x6g[:, (kc % 8) * 128 + (kc // 8) * 32:
                                                    (kc % 8) * 128 + (kc // 8) * 32 + 32],
                               w6sb)
                nc.sync.dma_start(out=h6_loc[:, :], in_=h6[:])
                nc.gpsimd.collective_compute(
                    "AllGather", ALU.bypass, replica_groups=RG,
                    ins=[h6_loc.ap().opt()], outs=[h6_all.ap().opt()])
            with nc.named_scope("mlp7"):
                hgi = load_idx("h_idx", 16)
                x7g = gather_xT(hgi, h6_all, 512, 256, "x7g")
                w8sb = preload_w("w8")
                h7 = mlp_layer("w7", lambda kc: x7g[:, (kc % 4) * 256 + (kc // 4) * 32:
                                                    (kc % 4) * 256 + (kc // 4) * 32 + 32],
                               w7sb)
                nc.sync.dma_start(out=h7_loc[:, :], in_=h7[:])
                nc.gpsimd.collective_compute(
                    "AllGather", ALU.bypass, replica_groups=RG,
                    ins=[h7_loc.ap().opt()], outs=[h7_all.ap().opt()])
            with nc.named_scope("mlp8"):
                x8g = gather_xT(hgi, h7_all, 512, 256, "x8g")
                h8 = mlp_layer("w8", lambda kc: x8g[:, (kc % 4) * 256 + (kc // 4) * 32:
                                                    (kc % 4) * 256 + (kc // 4) * 32 + 32],
                               w8sb)

            with nc.named_scope("mlp9"):
                w9t = load_const("w9")
                x9 = wpool.tile([128, 128], BF, tag="x9", name="x9")
                for kc in range(4):
                    transp(h8[:, kc * 128:(kc + 1) * 128], x9[:, 32 * kc:32 * kc + 32])
                ps9 = pps.tile([128, 128], F32, tag="mac9", name="mac9", bufs=1)
                for kc in range(4):
                    nc.tensor.matmul(out=ps9[:, :32], lhsT=w9t[:, kc * 128:(kc + 1) * 128],
                                     rhs=x9[:, 32 * kc:32 * kc + 32],
                                     start=(kc == 0), stop=(kc == 3))
                mu_sb = wpool.tile([128, 32], F32, tag="mu_sb", name="mu_sb")
                nc.vector.tensor_copy(mu_sb[:], ps9[:, :32])
                nc.sync.dma_start(out=mu_loc[:, :], in_=mu_sb[:])
                nc.gpsimd.collective_compute(
                    "AllGather", ALU.bypass, replica_groups=RG,
                    ins=[mu_loc.ap().opt()], outs=[mu_all.ap().opt()])
                mall = wpool.tile([128, 256], F32, tag="f_mall", name="f_mall")
                nc.sync.dma_start(
                    out=mall[:].rearrange("p (k b) -> p k b", b=32),
                    in_=mu_all.ap().rearrange("(k p) b -> p k b", p=128))
                tot = wpool.tile([128, 32], F32, tag="f_tot", name="f_tot")
                nc.vector.tensor_copy(tot[:], mall[:, 0:32])
                for k in range(1, 8):
                    nc.vector.tensor_add(tot[:], tot[:], mall[:, 32 * k:32 * k + 32])
                s1 = wpool.tile([128, 1], F32, tag="f_s1", name="f_s1")
                nc.vector.tensor_reduce(out=s1[:], in_=tot[:], axis=AX.X, op=ALU.add)
                mu_ = wpool.tile([128, 1], F32, tag="f_mu", name="f_mu")
                nc.vector.tensor_scalar_mul(mu_[:], s1[:], 1.0 / 32.0)
                sq = wpool.tile([128, 32], F32, tag="f_sq", name="f_sq")
                nc.vector.tensor_mul(sq[:], tot[:], tot[:])
                s2_ = wpool.tile([128, 1], F32, tag="f_s2", name="f_s2")
                nc.vector.tensor_reduce(out=s2_[:], in_=sq[:], axis=AX.X, op=ALU.add)
                var = wpool.tile([128, 1], F32, tag="f_var", name="f_var")
                nc.vector.scalar_tensor_tensor(out=var[:], in0=mu_[:], scalar=-1.0,
                                               in1=mu_[:], op0=ALU.mult, op1=ALU.mult)
                nc.vector.scalar_tensor_tensor(out=var[:], in0=s2_[:], scalar=1.0 / 32.0,
                                               in1=var[:], op0=ALU.mult, op1=ALU.add)
                sdf = wpool.tile([128, 1], F32, tag="f_sd", name="f_sd")
                nc.scalar.activation(out=sdf[:], in_=var[:], func=AF.Sqrt, bias=eps_t[:, 0:1])
                rs = wpool.tile([128, 1], F32, tag="f_rs", name="f_rs")
                nc.vector.reciprocal(rs[:], sdf[:])
                neg = wpool.tile([128, 1], F32, tag="f_neg", name="f_neg")
                nc.vector.scalar_tensor_tensor(out=neg[:], in0=mu_[:], scalar=-1.0,
                                               in1=rs[:], op0=ALU.mult, op1=ALU.mult)
                outt = wpool.tile([128, 32], F32, tag="f_out", name="f_out")
                nc.scalar.activation(out=outt[:], in_=tot[:], func=AF.Identity,
                                     scale=rs[:, 0:1], bias=neg[:, 0:1])
                nc.sync.dma_start(out=out_mu[:, :], in_=outt[:])

    nc.compile()
    return nc


# ---------------------------------------------------------------- entry point
def kernel(**inputs) -> np.ndarray:
    per_core, meta = _host_prep(inputs)
    if "prog" not in _CACHE:
        _CACHE["prog"] = _build_nc(meta, per_core[0])
    nc = _CACHE["prog"]
    res = bass_utils.run_bass_kernel_spmd(nc, per_core, core_ids=list(range(NCORES)))
    return np.ascontiguousarray(res.results[0]["mu"].T)
